# revision 1
# baseline (speedup 1.0000x reference)
"""AttentionResblock on 8 NeuronCores (Trainium2, Bass/Tile).

Sharding: query-token blocks of 512 (T_PAD=4096 = 8 x 512), two launches:
  Phase 1 (per core c): LayerNorm + Q/K/V projections for token rows
    [512c, 512c+512). Emits qT/kT (head-dim-major, bf16) and v (token-major,
    bf16) for its block. Host concatenates kT/v across cores.
  Phase 2 (per core c): full attention for its 512 query rows over all 4096
    keys (16 heads), output projection + residual. Host concatenates rows.

Numerics: all matmuls bf16 (PSUM f32); softmax as exp(s)*exp(bias) with
f32 scores from PE; denominators accumulated in f32 via ones-matmuls.
Final residual add in f32. Output error is dominated by the f32 residual
path since Wc scales the attention branch by ~1e-3.
"""

import sys

sys.path.insert(0, "/opt/trn_rl_repo")

from contextlib import ExitStack  # noqa: E402

import numpy as np  # noqa: E402
import ml_dtypes  # noqa: E402

import concourse.bass as bass  # noqa: E402
import concourse.bacc as bacc  # noqa: E402
import concourse.tile as tile  # noqa: E402
from concourse import mybir  # noqa: E402
from concourse.bass_utils import run_bass_kernel_spmd  # noqa: E402
from concourse.masks import make_identity  # noqa: E402

F32 = mybir.dt.float32
BF16 = mybir.dt.bfloat16
AF = mybir.ActivationFunctionType
ALU = mybir.AluOpType

N_STATE = 1024
N_HEADS = 16
D_HEAD = 64
N_CTX = 4080
T_PAD = 4096
N_CORES = 8
TOK = T_PAD // N_CORES  # 512 tokens per core
P = 128
LN_EPS = 1e-5
QK_SCALE = 0.125  # 1/sqrt(D_HEAD)

NSC = N_STATE // P  # 8 state chunks
NTC = TOK // P  # 4 token chunks per core
NKC = T_PAD // P  # 32 key chunks
NPAIR = N_HEADS // 2  # 8 head pairs


def _build_phase1() -> bass.Bass:
    nc = bacc.Bacc("TRN2", target_bir_lowering=False, debug=False, num_devices=N_CORES)
    m_blk = nc.dram_tensor("m_blk", [TOK, N_STATE], F32, kind="ExternalInput")
    gamma = nc.dram_tensor("gamma", [N_STATE], F32, kind="ExternalInput")
    Wq = nc.dram_tensor("Wq", [N_STATE, N_STATE], F32, kind="ExternalInput")
    Wk = nc.dram_tensor("Wk", [N_STATE, N_STATE], F32, kind="ExternalInput")
    Wv = nc.dram_tensor("Wv", [N_STATE, N_STATE], F32, kind="ExternalInput")
    bq = nc.dram_tensor("bq", [N_STATE], F32, kind="ExternalInput")
    bv = nc.dram_tensor("bv", [N_STATE], F32, kind="ExternalInput")
    qT_out = nc.dram_tensor("qT_out", [N_STATE, TOK], BF16, kind="ExternalOutput")
    kT_out = nc.dram_tensor("kT_out", [N_STATE, TOK], BF16, kind="ExternalOutput")
    v_out = nc.dram_tensor("v_out", [TOK, N_STATE], BF16, kind="ExternalOutput")

    with ExitStack() as ctx:
        tc = ctx.enter_context(tile.TileContext(nc))
        consts = ctx.enter_context(tc.tile_pool(name="consts", bufs=1))
        work = ctx.enter_context(tc.tile_pool(name="work", bufs=2))
        small = ctx.enter_context(tc.tile_pool(name="small", bufs=4))
        psum = ctx.enter_context(tc.tile_pool(name="psum", bufs=2, space="PSUM"))

        ident = consts.tile([P, P], F32)
        make_identity(nc, ident)
        ones1 = consts.tile([1, P], BF16)
        nc.vector.memset(ones1, 1.0)

        gamma_sb = consts.tile([P, NSC], F32)
        nc.sync.dma_start(out=gamma_sb, in_=gamma.rearrange("(sc p) -> p sc", p=P))
        bq_sb = consts.tile([P, NSC], F32)
        nc.sync.dma_start(out=bq_sb, in_=bq.rearrange("(sc p) -> p sc", p=P))
        bv_bf = consts.tile([1, N_STATE], BF16)
        nc.gpsimd.dma_start(out=bv_bf, in_=bv[None, :])
        eps_sb = consts.tile([P, 1], F32)
        nc.vector.memset(eps_sb, LN_EPS)

        # m first (LN is the head of the dependency chain), chunked per tok-chunk
        m_sb = consts.tile([P, NTC, N_STATE], F32)
        for tcn in range(NTC):
            nc.sync.dma_start(
                out=m_sb[:, tcn, :],
                in_=m_blk.rearrange("(c p) s -> p c s", p=P)[:, tcn, :],
            )

        # Weights straight to bf16 via casting SWDGE DMAs, layout [P, sc, out]
        w_bf = {}
        for name, w in (("Wq", Wq), ("Wk", Wk), ("Wv", Wv)):
            wb = consts.tile([P, NSC, N_STATE], BF16, name=f"{name}_bf")
            for sc in range(NSC):
                nc.gpsimd.dma_start(
                    out=wb[:, sc, :],
                    in_=w.rearrange("(sc p) o -> p sc o", p=P)[:, sc, :],
                )
            w_bf[name] = wb

        # LayerNorm (token-partition layout) -> xn (normalized, no gamma yet)
        xn_sb = consts.tile([P, NTC, N_STATE], F32)
        for tcn in range(NTC):
            ssum = small.tile([P, 1], F32, tag="ssum")
            nc.vector.reduce_sum(ssum, m_sb[:, tcn, :], axis=mybir.AxisListType.X)
            negmean = small.tile([P, 1], F32, tag="negmean")
            nc.scalar.mul(negmean, ssum, -1.0 / N_STATE)
            nc.vector.tensor_scalar_add(xn_sb[:, tcn, :], m_sb[:, tcn, :], negmean)
            sq = work.tile([P, N_STATE], F32, tag="sq")
            sqsum = small.tile([P, 1], F32, tag="sqsum")
            nc.scalar.activation(
                out=sq, in_=xn_sb[:, tcn, :], func=AF.Square, accum_out=sqsum
            )
            std = small.tile([P, 1], F32, tag="std")
            nc.scalar.activation(
                out=std, in_=sqsum, func=AF.Sqrt, bias=eps_sb, scale=1.0 / N_STATE
            )
            rstd = small.tile([P, 1], F32, tag="rstd")
            nc.vector.reciprocal(rstd, std)
            nc.vector.tensor_scalar_mul(xn_sb[:, tcn, :], xn_sb[:, tcn, :], rstd)

        # rT = gamma * xn^T  (state-partition layout), bf16
        rT_sb = consts.tile([P, NSC, TOK], BF16)
        for sc in range(NSC):
            pst = psum.tile([P, TOK], F32, tag="ptr")
            for tcn in range(NTC):
                nc.tensor.transpose(
                    pst[:, tcn * P : (tcn + 1) * P],
                    xn_sb[:, tcn, sc * P : (sc + 1) * P],
                    ident,
                )
            nc.scalar.activation(
                out=rT_sb[:, sc, :],
                in_=pst,
                func=AF.Copy,
                scale=gamma_sb[:, sc : sc + 1],
            )

        # qT = (Wq^T r^T + bq) * QK_SCALE ; kT = Wk^T r^T   (bf16, [P, hd_chunk, TOK])
        qT_sb = consts.tile([P, NSC, TOK], BF16)
        kT_sb = consts.tile([P, NSC, TOK], BF16)
        for j in range(NSC):
            psq = psum.tile([P, TOK], F32, tag="pq")
            psk = psum.tile([P, TOK], F32, tag="pk")
            for sc in range(NSC):
                nc.tensor.matmul(
                    psq,
                    lhsT=w_bf["Wq"][:, sc, j * P : (j + 1) * P],
                    rhs=rT_sb[:, sc, :],
                    start=(sc == 0),
                    stop=(sc == NSC - 1),
                )
            for sc in range(NSC):
                nc.tensor.matmul(
                    psk,
                    lhsT=w_bf["Wk"][:, sc, j * P : (j + 1) * P],
                    rhs=rT_sb[:, sc, :],
                    start=(sc == 0),
                    stop=(sc == NSC - 1),
                )
            nc.vector.tensor_scalar(
                out=qT_sb[:, j, :],
                in0=psq,
                scalar1=bq_sb[:, j : j + 1],
                scalar2=QK_SCALE,
                op0=ALU.add,
                op1=ALU.mult,
            )
            nc.scalar.copy(kT_sb[:, j, :], psk)

        # v = r @ Wv + bv  (token-partition layout) bf16
        v_sb = consts.tile([P, NTC, N_STATE], BF16)
        for tcn in range(NTC):
            for pc in range(2):
                psv = psum.tile([P, 512], F32, tag="pv")
                for sc in range(NSC):
                    nc.tensor.matmul(
                        psv,
                        lhsT=rT_sb[:, sc, tcn * P : (tcn + 1) * P],
                        rhs=w_bf["Wv"][:, sc, pc * 512 : (pc + 1) * 512],
                        start=(sc == 0),
                        stop=False,
                    )
                nc.tensor.matmul(
                    psv,
                    lhsT=ones1,
                    rhs=bv_bf[:, pc * 512 : (pc + 1) * 512],
                    start=False,
                    stop=True,
                )
                nc.scalar.copy(v_sb[:, tcn, pc * 512 : (pc + 1) * 512], psv)

        for j in range(NSC):
            nc.sync.dma_start(
                out=qT_out.rearrange("(j p) t -> p j t", p=P)[:, j, :],
                in_=qT_sb[:, j, :],
            )
            nc.sync.dma_start(
                out=kT_out.rearrange("(j p) t -> p j t", p=P)[:, j, :],
                in_=kT_sb[:, j, :],
            )
        for tcn in range(NTC):
            nc.sync.dma_start(
                out=v_out.rearrange("(c p) s -> p c s", p=P)[:, tcn, :],
                in_=v_sb[:, tcn, :],
            )
    nc.compile()
    return nc


def _build_phase2() -> bass.Bass:
    nc = bacc.Bacc("TRN2", target_bir_lowering=False, debug=False, num_devices=N_CORES)
    qT_in = nc.dram_tensor("qT_in", [N_STATE, TOK], BF16, kind="ExternalInput")
    kT_full = nc.dram_tensor("kT_full", [N_STATE, T_PAD], BF16, kind="ExternalInput")
    v_full = nc.dram_tensor("v_full", [T_PAD, N_STATE], BF16, kind="ExternalInput")
    bias_blk = nc.dram_tensor("bias_blk", [TOK, T_PAD], F32, kind="ExternalInput")
    m_blk = nc.dram_tensor("m_blk", [TOK, N_STATE], F32, kind="ExternalInput")
    Wc = nc.dram_tensor("Wc", [N_STATE, N_STATE], F32, kind="ExternalInput")
    bc = nc.dram_tensor("bc", [N_STATE], F32, kind="ExternalInput")
    o_out = nc.dram_tensor("o_out", [TOK, N_STATE], F32, kind="ExternalOutput")

    with ExitStack() as ctx:
        tc = ctx.enter_context(tile.TileContext(nc))
        consts = ctx.enter_context(tc.tile_pool(name="consts", bufs=1))
        pairbuf = ctx.enter_context(tc.tile_pool(name="pairbuf", bufs=2))
        work = ctx.enter_context(tc.tile_pool(name="work", bufs=3))
        small = ctx.enter_context(tc.tile_pool(name="small", bufs=4))
        psqk = ctx.enter_context(tc.tile_pool(name="psqk", bufs=2, space="PSUM"))
        pspv = ctx.enter_context(tc.tile_pool(name="pspv", bufs=1, space="PSUM"))
        psmisc = ctx.enter_context(tc.tile_pool(name="psmisc", bufs=2, space="PSUM"))
        bpool = ctx.enter_context(tc.tile_pool(name="bpool", bufs=2))

        ident = consts.tile([P, P], F32)
        make_identity(nc, ident)
        ones64_f = consts.tile([1, D_HEAD], F32)
        nc.vector.memset(ones64_f, 1.0)
        ones1x128_bf = consts.tile([1, P], BF16)
        nc.vector.memset(ones1x128_bf, 1.0)

        # expb[k_part, kc, q] = exp(bias^T) bf16
        expb_sb = consts.tile([P, NKC, TOK], BF16)
        for g4 in range(NKC // 2):
            bstage = bpool.tile([P, NTC, 2 * P], F32, tag="bstage", bufs=4)
            nc.gpsimd.dma_start(
                out=bstage,
                in_=bias_blk[:, g4 * 2 * P : (g4 + 1) * 2 * P].rearrange(
                    "(qc p) k -> p qc k", p=P
                ),
            )
            for sub in range(2):
                kc = g4 * 2 + sub
                ps_t = psmisc.tile([P, TOK], F32, tag="mt")
                for qc in range(NTC):
                    nc.tensor.transpose(
                        ps_t[:, qc * P : (qc + 1) * P],
                        bstage[:, qc, sub * P : (sub + 1) * P],
                        ident,
                    )
                nc.scalar.activation(
                    out=expb_sb[:, kc, :], in_=ps_t, func=AF.Exp
                )

        # attention per head-pair; PV carries a ones column for the denominators
        attnT_sb = consts.tile([P, NSC, TOK], BF16)
        for j in range(NPAIR):
            kT_pair = pairbuf.tile([P, T_PAD], BF16, tag="kT")
            nc.sync.dma_start(out=kT_pair, in_=kT_full[j * P : (j + 1) * P, :])
            qT_pair = pairbuf.tile([P, TOK], BF16, tag="qT")
            nc.gpsimd.dma_start(out=qT_pair, in_=qT_in[j * P : (j + 1) * P, :])
            v_pair = pairbuf.tile([P, NKC, 130], BF16, tag="v")
            nc.gpsimd.memset(v_pair[:, :, 64:65], 1.0)
            nc.gpsimd.memset(v_pair[:, :, 129:130], 1.0)
            nc.gpsimd.dma_start(
                out=v_pair[:, :, 0:64],
                in_=v_full[:, j * P : j * P + 64].rearrange(
                    "(kc p) c -> p kc c", p=P
                ),
            )
            nc.gpsimd.dma_start(
                out=v_pair[:, :, 65:129],
                in_=v_full[:, j * P + 64 : (j + 1) * P].rearrange(
                    "(kc p) c -> p kc c", p=P
                ),
            )

            ps_pvA = pspv.tile([65, TOK], F32, tag="pvA")
            ps_pvB = pspv.tile([65, TOK], F32, tag="pvB")
            for kc in range(NKC):
                ps_qk = psqk.tile([P, 2 * TOK], F32, tag="qk")
                nc.tensor.matmul(
                    ps_qk[:, 0:TOK],
                    lhsT=kT_pair[0:64, kc * P : (kc + 1) * P],
                    rhs=qT_pair[0:64, :],
                    start=True,
                    stop=True,
                    tile_position=(0, 0),
                )
                nc.tensor.matmul(
                    ps_qk[:, TOK : 2 * TOK],
                    lhsT=kT_pair[64:128, kc * P : (kc + 1) * P],
                    rhs=qT_pair[64:128, :],
                    start=True,
                    stop=True,
                    tile_position=(64, 0),
                )
                pt = work.tile([P, 2 * TOK], BF16, tag="pt")
                nc.scalar.activation(out=pt, in_=ps_qk, func=AF.Exp)
                pr = work.tile([P, 2 * TOK], BF16, tag="pr")
                eb = expb_sb[:, kc, :].rearrange("p (o k) -> p o k", o=1)
                nc.vector.tensor_mul(
                    pr.rearrange("p (o k) -> p o k", o=2),
                    pt.rearrange("p (o k) -> p o k", o=2),
                    eb.broadcast_to([P, 2, TOK]),
                )
                nc.tensor.matmul(
                    ps_pvA,
                    lhsT=v_pair[:, kc, 0:65],
                    rhs=pr[:, 0:TOK],
                    start=(kc == 0),
                    stop=(kc == NKC - 1),
                )
                nc.tensor.matmul(
                    ps_pvB,
                    lhsT=v_pair[:, kc, 65:130],
                    rhs=pr[:, TOK : 2 * TOK],
                    start=(kc == 0),
                    stop=(kc == NKC - 1),
                )

            recipA = small.tile([1, TOK], F32, tag="recA")
            nc.vector.reciprocal(recipA, ps_pvA[64:65, :])
            recipB = small.tile([1, TOK], F32, tag="recB")
            nc.vector.reciprocal(recipB, ps_pvB[64:65, :])
            ps_bc = psmisc.tile([P, TOK], F32, tag="mt")
            nc.tensor.matmul(
                ps_bc[0:64, :],
                lhsT=ones64_f,
                rhs=recipA,
                start=True,
                stop=True,
                tile_position=(0, 0),
            )
            nc.tensor.matmul(
                ps_bc[64:128, :],
                lhsT=ones64_f,
                rhs=recipB,
                start=True,
                stop=True,
                tile_position=(0, 64),
            )
            bc_sb = bpool.tile([P, TOK], F32, tag="bcsb")
            nc.vector.tensor_copy(bc_sb, ps_bc)
            nc.vector.tensor_mul(
                attnT_sb[0:64, j, :], ps_pvA[0:64, :], bc_sb[0:64, :]
            )
            nc.vector.tensor_mul(
                attnT_sb[64:128, j, :], ps_pvB[0:64, :], bc_sb[64:128, :]
            )

        bc_bf = consts.tile([1, N_STATE], BF16)
        nc.gpsimd.dma_start(out=bc_bf, in_=bc[None, :])
        m_sb = consts.tile([P, NTC, N_STATE], F32)
        nc.sync.dma_start(out=m_sb, in_=m_blk.rearrange("(c p) s -> p c s", p=P))
        Wc_bf = consts.tile([P, NSC, N_STATE], BF16)
        nc.gpsimd.dma_start(out=Wc_bf, in_=Wc.rearrange("(sc p) o -> p sc o", p=P))

        # output projection + bias + residual
        o_sb = consts.tile([P, NTC, N_STATE], F32)
        for qc in range(NTC):
            for pc in range(2):
                gidx = qc * 2 + pc
                if gidx % 2 == 0:
                    ps_o_full = psqk.tile([P, 2 * TOK], F32, tag="qk", name="ps_o_full")
                    ps_o = ps_o_full[:, 0:512]
                else:
                    ps_o = psmisc.tile([P, 512], F32, tag="mt")
                for j in range(NSC):
                    nc.tensor.matmul(
                        ps_o,
                        lhsT=attnT_sb[:, j, qc * P : (qc + 1) * P],
                        rhs=Wc_bf[:, j, pc * 512 : (pc + 1) * 512],
                        start=(j == 0),
                        stop=False,
                    )
                nc.tensor.matmul(
                    ps_o,
                    lhsT=ones1x128_bf,
                    rhs=bc_bf[:, pc * 512 : (pc + 1) * 512],
                    start=False,
                    stop=True,
                )
                nc.vector.tensor_add(
                    o_sb[:, qc, pc * 512 : (pc + 1) * 512],
                    ps_o,
                    m_sb[:, qc, pc * 512 : (pc + 1) * 512],
                )
        for qc in range(NTC):
            nc.sync.dma_start(
                out=o_out.rearrange("(c p) s -> p c s", p=P)[:, qc, :],
                in_=o_sb[:, qc, :],
            )
    nc.compile()
    return nc


_NC_CACHE = {}


def _get_nc(which):
    if which not in _NC_CACHE:
        _NC_CACHE[which] = _build_phase1() if which == 1 else _build_phase2()
    return _NC_CACHE[which]


def kernel(m, bias, gamma, beta, Wq, bq, Wk, Wv, bv, Wc, bc, _want_timing=None):
    m = np.asarray(m, dtype=np.float32).reshape(N_CTX, N_STATE)
    m_pad = np.zeros((T_PAD, N_STATE), np.float32)
    m_pad[:N_CTX] = m
    gamma = np.asarray(gamma, np.float32)
    beta = np.asarray(beta, np.float32)
    bias = np.asarray(bias, np.float32)

    import sys as _sys
    def _log(*a):
        print("[kernel]", *a, file=_sys.stderr, flush=True)
    _log("building phase1")
    nc1 = _get_nc(1)
    _log("phase1 built")
    in_maps1 = []
    for c in range(N_CORES):
        in_maps1.append(
            {
                "m_blk": np.ascontiguousarray(m_pad[c * TOK : (c + 1) * TOK]),
                "gamma": np.asarray(gamma, np.float32),
                "Wq": np.asarray(Wq, np.float32),
                "Wk": np.asarray(Wk, np.float32),
                "Wv": np.asarray(Wv, np.float32),
                "bq": np.asarray(bq, np.float32),
                "bv": np.asarray(bv, np.float32),
            }
        )
    _log("running phase1")
    res1 = run_bass_kernel_spmd(nc1, in_maps1, core_ids=list(range(N_CORES)))
    _log("phase1 done")
    kT_full = np.concatenate([r["kT_out"] for r in res1.results], axis=1)
    v_full = np.concatenate([r["v_out"] for r in res1.results], axis=0)
    qT_blks = [r["qT_out"] for r in res1.results]
    # zero the padded key/value tokens (guards against pad-row LN artifacts)
    kT_full[:, N_CTX:] = 0
    v_full[N_CTX:, :] = 0

    nc2 = _get_nc(2)
    _log("phase2 built")
    in_maps2 = []
    for c in range(N_CORES):
        in_maps2.append(
            {
                "qT_in": np.ascontiguousarray(qT_blks[c]),
                "kT_full": kT_full,
                "v_full": v_full,
                "bias_blk": np.ascontiguousarray(bias[c * TOK : (c + 1) * TOK]),
                "m_blk": np.ascontiguousarray(m_pad[c * TOK : (c + 1) * TOK]),
                "Wc": np.asarray(Wc, np.float32),
                "bc": np.asarray(bc, np.float32),
            }
        )
    _log("running phase2")
    res2 = run_bass_kernel_spmd(nc2, in_maps2, core_ids=list(range(N_CORES)))
    _log("phase2 done")
    o = np.concatenate([r["o_out"] for r in res2.results], axis=0)[:N_CTX]
    if _want_timing is not None:
        _want_timing["res1"] = res1
        _want_timing["res2"] = res2
    return o.reshape(1, N_CTX, N_STATE).astype(np.float32)



# revision 8
# speedup vs baseline: 1.2582x; 1.2582x over previous
"""AttentionResblock on 8 NeuronCores (Trainium2, Bass/Tile) — fp8 edition.

Sharding: query-token blocks of 512 (T_PAD=4096 = 8 x 512), two launches:
  Phase 1 (per core c): LayerNorm + Q/K/V projections (fp8 DoubleRow matmuls)
    for token rows [512c, 512c+512). Emits q8/k8 in DoubleRow-ready
    [128, 2, 512] head-pair tiles and v8 token-major, all fp8-e4m3.
    gamma/beta/bv/bc are folded into weights/residual on the host.
  Phase 2 (per core c): 16-head attention for its 512 query rows over all
    4096 keys. QK via fp8 DoubleRow (2x32 contraction). Softmax weights are
    produced two ways, split across engines to balance the timeline:
      - ACT chunks: bias added in PSUM via fp8 ident-matmul, then true
        exp -> fp8-e4m3 (premultiplied by 512, shifted by C=9).
      - DVE chunks: fastexp bit trick - y = int8(alpha*s + [alpha*b +
        beta - alpha*C]) bitcast as fp8-e5m2 (2^(y/4-15) ~ 512*e^(s+b-9)).
    The scale/shift cancels in softmax: PV accumulates numerator and
    denominator (ones columns in the fp8 V tiles, value 0.5 = LAM_V/LAM_ATTN)
    with fp8 DoubleRow over key-chunk pairs. Normalize, fp8 DoubleRow output
    projection, f32 residual add.

Numerics validated against the reference in numpy (numerics2.py): rel err
~1.5e-4 vs the 2e-2 gate, dominated by fp8 quantization of the attention
branch, which is scaled by ~1e-3 through Wc so the f32 residual dominates.
"""

import sys

sys.path.insert(0, "/opt/trn_rl_repo")

from contextlib import ExitStack  # noqa: E402

import numpy as np  # noqa: E402
import ml_dtypes  # noqa: E402

import concourse.bass as bass  # noqa: E402
import concourse.bacc as bacc  # noqa: E402
import concourse.tile as tile  # noqa: E402
from concourse import mybir  # noqa: E402
from concourse.bass_utils import run_bass_kernel_spmd  # noqa: E402
from concourse.masks import make_identity  # noqa: E402

F32 = mybir.dt.float32
BF16 = mybir.dt.bfloat16
F8E4 = mybir.dt.float8e4
F8E5 = mybir.dt.float8e5
I8 = mybir.dt.int8
AF = mybir.ActivationFunctionType
ALU = mybir.AluOpType
DR = mybir.MatmulPerfMode.DoubleRow

E4NP = ml_dtypes.float8_e4m3
E5NP = ml_dtypes.float8_e5m2
BFNP = ml_dtypes.bfloat16

N_STATE = 1024
N_HEADS = 16
D_HEAD = 64
N_CTX = 4080
T_PAD = 4096
N_CORES = 8
TOK = 512
P = 128
LN_EPS = 1e-5
NSC = 8  # state chunks of 128
NTC = 4  # token chunks per core
NKC = 32  # key chunks of 128
NKCP = 16  # key-chunk pairs of 256
NPAIR = 8  # head pairs

# fp8 scale plan (see numerics2.py)
ALPHA = 4 * np.log2(np.e)  # logit scale in PSUM: psum = ALPHA*(s)
C_SHIFT = 9.0  # global logit shift (measured max 6.21)
PMULT = 512.0  # weights premultiplier (cancels in softmax)
BETA = 96.0  # 60 + 4*log2(PMULT)
LAM_R = 16.0  # LN output scale
LAM_W = 512.0  # Wq/Wk/Wv scale
LAM_Q = float(np.sqrt(ALPHA / 8.0))  # q/k scales; 8*LAM_Q*LAM_K = ALPHA
LAM_V = 16.0
LAM_B = 369.0  # ACT-path bias quant scale; ident diag g = 2^-6, g*LAM_B ~ ALPHA
G_IDENT = 2.0 ** -6
LAM_ATTN = 32.0
LAM_WC = 32768.0
ONES_VAL = LAM_V / LAM_ATTN  # 0.5, folded into denominator columns
GQ = LAM_Q / (LAM_R * LAM_W)
GV = LAM_V / (LAM_R * LAM_W)
G_OUT = 1.0 / (LAM_ATTN * LAM_WC)
EXP_BIAS = float(np.log(PMULT) - C_SHIFT)  # -2.7616
EXP_SCALE = float(1.0 / ALPHA)
STT_CLAMP = -40.0

# kc-pair -> engine assignment (per head-pair parity), tuned for balance
DVE_EVEN = (1, 3, 5, 7, 9, 11)
DVE_ODD = (1, 3, 5, 7, 9, 11, 13)
ACT_KC_LIST = [kc for kcp in range(NKCP) if kcp not in DVE_EVEN
               for kc in (2 * kcp, 2 * kcp + 1)]  # 20 kcs ever handled by ACT
DVE_KC_LIST = [kc for kcp in DVE_ODD for kc in (2 * kcp, 2 * kcp + 1)]  # 14 kcs
ACT_SLOT = {kc: i for i, kc in enumerate(ACT_KC_LIST)}
DVE_SLOT = {kc: i for i, kc in enumerate(DVE_KC_LIST)}
VW = NPAIR * 130 + 32  # v8 tile width: per-pair 130 cols + tail padding


def _build_phase1() -> bass.Bass:
    nc = bacc.Bacc("TRN2", target_bir_lowering=False, debug=False, num_devices=N_CORES)
    m_blk = nc.dram_tensor("m_blk", [TOK, N_STATE], F32, kind="ExternalInput")
    Wq8 = nc.dram_tensor("Wq8", [P, 4, 2, N_STATE], F8E4, kind="ExternalInput")
    Wk8 = nc.dram_tensor("Wk8", [P, 4, 2, N_STATE], F8E4, kind="ExternalInput")
    Wv8 = nc.dram_tensor("Wv8", [P, 4, 2, N_STATE], F8E4, kind="ExternalInput")
    bqs = nc.dram_tensor("bqs", [N_STATE], F32, kind="ExternalInput")
    q8_out = nc.dram_tensor("q8_out", [4, P, 2, TOK], F8E4, kind="ExternalOutput")
    k8_out = nc.dram_tensor("k8_out", [4, P, 2, TOK], F8E4, kind="ExternalOutput")
    v8_out = nc.dram_tensor("v8_out", [TOK, N_STATE], F8E4, kind="ExternalOutput")

    with ExitStack() as ctx:
        tc = ctx.enter_context(tile.TileContext(nc))
        consts = ctx.enter_context(tc.tile_pool(name="consts", bufs=1))
        small = ctx.enter_context(tc.tile_pool(name="small", bufs=4))
        work = ctx.enter_context(tc.tile_pool(name="work", bufs=2))
        psum = ctx.enter_context(tc.tile_pool(name="psum", bufs=2, space="PSUM"))
        pst_pool = ctx.enter_context(tc.tile_pool(name="pst", bufs=2, space="PSUM"))

        identB = consts.tile([P, P], BF16)
        make_identity(nc, identB)
        eps_sb = consts.tile([P, 1], F32)
        nc.vector.memset(eps_sb, LN_EPS)
        bqs_sb = consts.tile([P, NSC], F32)
        nc.sync.dma_start(out=bqs_sb, in_=bqs.rearrange("(j p) -> p j", p=P))

        m_sb = consts.tile([P, NTC, N_STATE], F32)
        for tcn in range(NTC):
            nc.sync.dma_start(
                out=m_sb[:, tcn, :],
                in_=m_blk.rearrange("(c p) s -> p c s", p=P)[:, tcn, :],
            )
        w_sb = {}
        for name, w in (("Wq8", Wq8), ("Wk8", Wk8), ("Wv8", Wv8)):
            wb = consts.tile([P, 4, 2, N_STATE], F8E4, name=f"{name}_sb")
            nc.sync.dma_start(out=wb, in_=w[:, :, :, :])
            w_sb[name] = wb

        # LayerNorm -> xcB = (m - mu) * rstd * LAM_R in bf16
        xcB = consts.tile([P, NTC, N_STATE], BF16)
        for tcn in range(NTC):
            ssum = small.tile([P, 1], F32, tag="ssum")
            nc.vector.reduce_sum(ssum, m_sb[:, tcn, :], axis=mybir.AxisListType.X)
            negmean = small.tile([P, 1], F32, tag="negmean")
            nc.scalar.mul(negmean, ssum, -1.0 / N_STATE)
            sqscr = work.tile([P, N_STATE], BF16, tag="sqscr")
            sqsum = small.tile([P, 1], F32, tag="sqsum")
            nc.scalar.activation(
                out=sqscr, in_=m_sb[:, tcn, :], func=AF.Square, accum_out=sqsum
            )
            mean = small.tile([P, 1], F32, tag="mean")
            nc.scalar.mul(mean, ssum, 1.0 / N_STATE)
            msq = small.tile([P, 1], F32, tag="msq")
            nc.scalar.activation(out=msq, in_=mean, func=AF.Square)
            exx = small.tile([P, 1], F32, tag="exx")
            nc.scalar.mul(exx, sqsum, 1.0 / N_STATE)
            var = small.tile([P, 1], F32, tag="var")
            nc.vector.tensor_tensor(out=var, in0=exx, in1=msq, op=ALU.subtract)
            std = small.tile([P, 1], F32, tag="std")
            nc.scalar.activation(out=std, in_=var, func=AF.Sqrt, bias=eps_sb)
            rstd = small.tile([P, 1], F32, tag="rstd")
            nc.vector.reciprocal(rstd, std)
            rstdl = small.tile([P, 1], F32, tag="rstdl")
            nc.scalar.mul(rstdl, rstd, LAM_R)
            nc.gpsimd.tensor_scalar(
                out=xcB[:, tcn, :],
                in0=m_sb[:, tcn, :],
                scalar1=negmean,
                scalar2=rstdl,
                op0=ALU.add,
                op1=ALU.mult,
            )

        # transpose to state-major and quantize: rT8 [128, sc, 512] e4m3
        rT8 = consts.tile([P, NSC, TOK], F8E4)
        for sc in range(NSC):
            pst = pst_pool.tile([P, TOK], BF16, tag="pst")
            for tcn in range(NTC):
                nc.tensor.transpose(
                    pst[:, tcn * P : (tcn + 1) * P],
                    xcB[:, tcn, sc * P : (sc + 1) * P],
                    identB,
                )
            nc.vector.tensor_copy(rT8[:, sc, :], pst)

        # q/k: DoubleRow fp8 matmuls, evacuate into [128, 2, 512] pair tiles
        q8g = [consts.tile([P, 2, TOK], F8E4, name=f"q8g{g}") for g in range(4)]
        k8g = [consts.tile([P, 2, TOK], F8E4, name=f"k8g{g}") for g in range(4)]
        for j in range(NSC):
            g, half = j // 2, j % 2
            psq = psum.tile([P, TOK], F32, tag="psq")
            psk = psum.tile([P, TOK], F32, tag="psk")
            for s in range(4):
                nc.tensor.matmul(
                    psq,
                    lhsT=w_sb["Wq8"][:, s, :, j * P : (j + 1) * P],
                    rhs=rT8[:, 2 * s : 2 * s + 2, :],
                    start=(s == 0),
                    stop=(s == 3),
                    perf_mode=DR,
                )
            for s in range(4):
                nc.tensor.matmul(
                    psk,
                    lhsT=w_sb["Wk8"][:, s, :, j * P : (j + 1) * P],
                    rhs=rT8[:, 2 * s : 2 * s + 2, :],
                    start=(s == 0),
                    stop=(s == 3),
                    perf_mode=DR,
                )
            for t in range(2):
                nc.scalar.activation(
                    out=q8g[g][64 * half : 64 * half + 64, t, :],
                    in_=psq[64 * t : 64 * t + 64, :],
                    func=AF.Identity,
                    bias=bqs_sb[64 * t : 64 * t + 64, j : j + 1],
                    scale=GQ,
                )
                nc.vector.tensor_scalar(
                    out=k8g[g][64 * half : 64 * half + 64, t, :],
                    in0=psk[64 * t : 64 * t + 64, :],
                    scalar1=GQ,
                    scalar2=None,
                    op0=ALU.mult,
                )

        # v: token-major, e4m3
        v8sb = consts.tile([P, NTC, N_STATE], F8E4)
        for tcn in range(NTC):
            psv = psum.tile([P, N_STATE], F32, tag="psv", bufs=1)
            for pc in range(2):
                for s in range(4):
                    nc.tensor.matmul(
                        psv[:, pc * TOK : (pc + 1) * TOK],
                        lhsT=rT8[:, 2 * s : 2 * s + 2, tcn * P : (tcn + 1) * P],
                        rhs=w_sb["Wv8"][:, s, :, pc * TOK : (pc + 1) * TOK],
                        start=(s == 0),
                        stop=(s == 3),
                        perf_mode=DR,
                    )
            nc.scalar.mul(v8sb[:, tcn, :], psv, GV)

        for g in range(4):
            nc.sync.dma_start(out=q8_out[g, :, :, :], in_=q8g[g])
            nc.sync.dma_start(out=k8_out[g, :, :, :], in_=k8g[g])
        for tcn in range(NTC):
            nc.sync.dma_start(
                out=v8_out.rearrange("(c p) s -> p c s", p=P)[:, tcn, :],
                in_=v8sb[:, tcn, :],
            )
    nc.compile()
    return nc


def _build_phase2() -> bass.Bass:
    nc = bacc.Bacc("TRN2", target_bir_lowering=False, debug=False, num_devices=N_CORES)
    q8_in = nc.dram_tensor("q8_in", [4, P, 2, TOK], F8E4, kind="ExternalInput")
    k8_in = nc.dram_tensor("k8_in", [4, P, 2, T_PAD], F8E4, kind="ExternalInput")
    v8_in = nc.dram_tensor("v8_in", [4, P, NKC // 4, VW], F8E4, kind="ExternalInput")
    b8_in = nc.dram_tensor("b8_in", [len(ACT_KC_LIST), P, 2, TOK], F8E4, kind="ExternalInput")
    bt_in = nc.dram_tensor("bt_in", [len(DVE_KC_LIST), P, TOK], BF16, kind="ExternalInput")
    mres = nc.dram_tensor("mres", [TOK, N_STATE], F32, kind="ExternalInput")
    Wc8 = nc.dram_tensor("Wc8", [P, 4, 2, N_STATE], F8E4, kind="ExternalInput")
    o_out = nc.dram_tensor("o_out", [TOK, N_STATE], F32, kind="ExternalOutput")

    with ExitStack() as ctx:
        tc = ctx.enter_context(tile.TileContext(nc))
        consts = ctx.enter_context(tc.tile_pool(name="consts", bufs=1))
        small = ctx.enter_context(tc.tile_pool(name="small", bufs=4))
        ppool = ctx.enter_context(tc.tile_pool(name="ppool", bufs=3))
        psqk = ctx.enter_context(tc.tile_pool(name="psqk", bufs=3, space="PSUM"))
        pspv = ctx.enter_context(tc.tile_pool(name="pspv", bufs=1, space="PSUM"))

        identg = consts.tile([P, 2, P], F8E4)
        nc.vector.memset(identg, 0.0)
        make_identity(nc, identg[:, 0, :])
        make_identity(nc, identg[:, 1, :])
        identg2 = consts.tile([P, 2, P], F8E4)
        nc.scalar.mul(identg2, identg, G_IDENT / 2.0)
        ones64 = consts.tile([1, 64], BF16)
        nc.vector.memset(ones64, 1.0)
        expb_ap = consts.tile([P, 1], F32)
        nc.vector.memset(expb_ap, EXP_BIAS)

        q8sb = consts.tile([P, 4, 2, TOK], F8E4)
        nc.sync.dma_start(out=q8sb, in_=q8_in[:, :, :, :].rearrange("g p t n -> p g t n"))
        k8sb = consts.tile([P, 4, 2, T_PAD], F8E4)
        for g in range(4):
            nc.sync.dma_start(out=k8sb[:, g, :, :], in_=k8_in[g, :, :, :])
        v8sb = consts.tile([P, NKC, VW], F8E4)
        for vq in range(4):
            nc.sync.dma_start(
                out=v8sb[:, vq * 8 : (vq + 1) * 8, :], in_=v8_in[vq, :, :, :]
            )
        b8sb = consts.tile([P, len(ACT_KC_LIST), 2, TOK], F8E4)
        nc.sync.dma_start(out=b8sb, in_=b8_in[:, :, :, :].rearrange("k p t n -> p k t n"))
        btsb = consts.tile([P, len(DVE_KC_LIST), TOK], BF16)
        nc.sync.dma_start(out=btsb, in_=bt_in[:, :, :].rearrange("k p n -> p k n"))
        m_sb = consts.tile([P, NTC, N_STATE], F32)
        nc.sync.dma_start(out=m_sb, in_=mres.rearrange("(c p) s -> p c s", p=P))
        wc_sb = consts.tile([P, 4, 2, N_STATE], F8E4)
        nc.sync.dma_start(out=wc_sb, in_=Wc8[:, :, :, :])

        attnT8 = consts.tile([P, NPAIR, TOK], F8E4)

        for j in range(NPAIR):
            g, half = j // 2, j % 2
            dve_set = DVE_ODD if (j % 2) else DVE_EVEN
            pvA = pspv.tile([96, TOK], F32, tag="pvA")
            pvB = pspv.tile([96, TOK], F32, tag="pvB")
            for kcp in range(NKCP):
                is_dve = kcp in dve_set
                if is_dve:
                    ptile = ppool.tile([P, 2, 2, TOK], F8E5, tag="pD")
                else:
                    ptile = ppool.tile([P, 2, 2, TOK], F8E4, tag="pA")
                for sub in range(2):
                    kc = 2 * kcp + sub
                    ps = psqk.tile([P, 2, TOK], F32, tag="s")
                    for h in range(2):
                        base = 64 * half + 32 * h
                        nc.tensor.matmul(
                            ps[:, h, :],
                            lhsT=k8sb[base : base + 32, g, :, kc * P : (kc + 1) * P],
                            rhs=q8sb[base : base + 32, g, :, :],
                            start=True,
                            stop=is_dve,
                            perf_mode=DR,
                            tile_position=(base, 0),
                        )
                        if not is_dve:
                            nc.tensor.matmul(
                                ps[:, h, :],
                                lhsT=identg2,
                                rhs=b8sb[:, ACT_SLOT[kc], :, :],
                                start=False,
                                stop=True,
                                perf_mode=DR,
                                skip_group_check=True,
                            )
                    if is_dve:
                        nc.vector.scalar_tensor_tensor(
                            out=ptile[:, :, sub, :].bitcast(I8),
                            in0=ps,
                            scalar=STT_CLAMP,
                            in1=btsb[:, DVE_SLOT[kc], :]
                            .rearrange("p (o n) -> p o n", o=1)
                            .broadcast_to([P, 2, TOK]),
                            op0=ALU.max,
                            op1=ALU.add,
                        )
                    else:
                        nc.scalar.activation(
                            out=ptile[:, :, sub, :],
                            in_=ps,
                            func=AF.Exp,
                            bias=expb_ap,
                            scale=EXP_SCALE,
                        )
                nc.tensor.matmul(
                    pvA,
                    lhsT=v8sb[:, 2 * kcp : 2 * kcp + 2, 130 * j : 130 * j + 96],
                    rhs=ptile[:, 0, :, :],
                    start=(kcp == 0),
                    stop=(kcp == NKCP - 1),
                    perf_mode=DR,
                )
                nc.tensor.matmul(
                    pvB,
                    lhsT=v8sb[:, 2 * kcp : 2 * kcp + 2, 130 * j + 65 : 130 * j + 161],
                    rhs=ptile[:, 1, :, :],
                    start=(kcp == 0),
                    stop=(kcp == NKCP - 1),
                    perf_mode=DR,
                )

            recipA = small.tile([1, TOK], BF16, tag="recA")
            recipB = small.tile([1, TOK], BF16, tag="recB")
            with nc.allow_low_precision("bf16 softmax denominators, ~0.4% scale"):
                nc.vector.reciprocal(recipA, pvA[64:65, :])
                nc.vector.reciprocal(recipB, pvB[64:65, :])
            ps_bc = psqk.tile([P, 2, TOK], F32, tag="s")
            nc.tensor.matmul(
                ps_bc[0:64, 0, :],
                lhsT=ones64,
                rhs=recipA,
                start=True,
                stop=True,
                tile_position=(0, 0),
            )
            nc.tensor.matmul(
                ps_bc[64:128, 0, :],
                lhsT=ones64,
                rhs=recipB,
                start=True,
                stop=True,
                tile_position=(0, 64),
            )
            bc_sb = small.tile([P, TOK], F32, tag="bcsb", bufs=2)
            nc.scalar.copy(bc_sb, ps_bc[:, 0, :])
            nc.vector.tensor_mul(attnT8[0:64, j, :], pvA[0:64, :], bc_sb[0:64, :])
            nc.vector.tensor_mul(
                attnT8[64:128, j, :], pvB[0:64, :], bc_sb[64:128, :]
            )

        # output projection (fp8 DR) + residual add (f32)
        o_sb = consts.tile([P, NTC, N_STATE], F32)
        for qc in range(NTC):
            ps_o = psqk.tile([P, 2, TOK], F32, tag="s")
            po = ps_o.rearrange("p a b -> p (a b)")
            for pc in range(2):
                for u in range(4):
                    nc.tensor.matmul(
                        ps_o[:, pc, :],
                        lhsT=attnT8[:, 2 * u : 2 * u + 2, qc * P : (qc + 1) * P],
                        rhs=wc_sb[:, u, :, pc * TOK : (pc + 1) * TOK],
                        start=(u == 0),
                        stop=(u == 3),
                        perf_mode=DR,
                    )
            nc.vector.scalar_tensor_tensor(
                out=o_sb[:, qc, :],
                in0=po,
                scalar=G_OUT,
                in1=m_sb[:, qc, :],
                op0=ALU.mult,
                op1=ALU.add,
            )
        for qc in range(NTC):
            nc.sync.dma_start(
                out=o_out.rearrange("(c p) s -> p c s", p=P)[:, qc, :],
                in_=o_sb[:, qc, :],
            )
    nc.compile()
    return nc


_NC_CACHE = {}


def _get_nc(which):
    if which not in _NC_CACHE:
        _NC_CACHE[which] = _build_phase1() if which == 1 else _build_phase2()
    return _NC_CACHE[which]


def _perm_cols():
    """Column permutation for q/k weights: per pair j, [hA d0:32 | hB d0:32 |
    hA d32:64 | hB d32:64]."""
    order = []
    for j in range(NSC):
        hA, hB = 2 * j, 2 * j + 1
        order.extend(range(hA * 64, hA * 64 + 32))
        order.extend(range(hB * 64, hB * 64 + 32))
        order.extend(range(hA * 64 + 32, hA * 64 + 64))
        order.extend(range(hB * 64 + 32, hB * 64 + 64))
    return np.array(order)


def _w_dr_layout(w8):
    """[1024, C] -> [128, 4, 2, C] DoubleRow lhsT layout."""
    return np.ascontiguousarray(
        w8.reshape(4, 2, P, -1).transpose(2, 0, 1, 3)
    )


def kernel(m, bias, gamma, beta, Wq, bq, Wk, Wv, bv, Wc, bc, _want_timing=None):
    m = np.asarray(m, dtype=np.float32).reshape(N_CTX, N_STATE)
    bias = np.asarray(bias, np.float32)
    gamma = np.asarray(gamma, np.float32)
    beta = np.asarray(beta, np.float32)
    Wq = np.asarray(Wq, np.float32)
    Wk = np.asarray(Wk, np.float32)
    Wv = np.asarray(Wv, np.float32)
    Wc = np.asarray(Wc, np.float32)
    bq = np.asarray(bq, np.float32)
    bv = np.asarray(bv, np.float32)
    bc = np.asarray(bc, np.float32)

    m_pad = np.zeros((T_PAD, N_STATE), np.float32)
    m_pad[:N_CTX] = m

    # fold gamma into weights, beta into biases; bv and bc fold into residual
    Wqf = gamma[:, None] * Wq
    Wkf = gamma[:, None] * Wk
    Wvf = gamma[:, None] * Wv
    bqf = bq + beta @ Wq
    # beta@Wk shifts all logits of a query equally -> softmax invariant; drop.
    # beta@Wv + bv shift attention output -> fold into residual with bc.
    perm = _perm_cols()
    Wq8 = _w_dr_layout((LAM_W * Wqf[:, perm]).astype(E4NP))
    Wk8 = _w_dr_layout((LAM_W * Wkf[:, perm]).astype(E4NP))
    Wv8 = _w_dr_layout((LAM_W * Wvf).astype(E4NP))
    bqs = (LAM_Q * bqf[perm]).astype(np.float32)
    Wc8 = _w_dr_layout((LAM_WC * Wc).astype(E4NP))
    mres_full = m_pad + (bc + (bv + beta @ Wv) @ Wc)[None, :]

    import sys as _sys

    def _log(*a):
        print("[kernel]", *a, file=_sys.stderr, flush=True)

    nc1 = _get_nc(1)
    _log("phase1 built")
    in_maps1 = []
    for c in range(N_CORES):
        in_maps1.append(
            {
                "m_blk": np.ascontiguousarray(m_pad[c * TOK : (c + 1) * TOK]),
                "Wq8": Wq8,
                "Wk8": Wk8,
                "Wv8": Wv8,
                "bqs": bqs,
            }
        )
    res1 = run_bass_kernel_spmd(nc1, in_maps1, core_ids=list(range(N_CORES)))
    _log("phase1 done")

    q8_blks = [r["q8_out"] for r in res1.results]
    k8_full = np.concatenate([r["k8_out"] for r in res1.results], axis=3)
    v8_full = np.concatenate([r["v8_out"] for r in res1.results], axis=0)
    v8_full[N_CTX:] = 0  # pad tokens carry no value

    # v8 pair-tile layout [128, 32, VW] with denominator columns
    v8f = v8_full.astype(np.float32).reshape(NKC, P, N_HEADS, D_HEAD)
    v8h = np.zeros((P, NKC, VW), np.float32)
    for j in range(NPAIR):
        v8h[:, :, 130 * j : 130 * j + 64] = v8f[:, :, 2 * j].transpose(1, 0, 2)
        v8h[:, :, 130 * j + 65 : 130 * j + 129] = v8f[:, :, 2 * j + 1].transpose(1, 0, 2)
        v8h[:, :, 130 * j + 64] = ONES_VAL
        v8h[:, :, 130 * j + 129] = ONES_VAL
    # zero the denominator contribution of padded keys
    keyidx = (np.arange(NKC)[None, :] * P + np.arange(P)[:, None])  # [p, kc]
    padmask = keyidx >= N_CTX
    for j in range(NPAIR):
        v8h[:, :, 130 * j + 64][padmask] = 0.0
        v8h[:, :, 130 * j + 129][padmask] = 0.0
    v8h8 = v8h.astype(E4NP)
    v8_dr = np.ascontiguousarray(
        v8h8.reshape(P, 4, NKC // 4, VW).transpose(1, 0, 2, 3)
    )

    biasT = np.ascontiguousarray(bias.T)  # [k, q]

    nc2 = _get_nc(2)
    _log("phase2 built")
    in_maps2 = []
    for c in range(N_CORES):
        qs = slice(c * TOK, (c + 1) * TOK)
        b8 = np.zeros((len(ACT_KC_LIST), P, 2, TOK), E4NP)
        for i, kc in enumerate(ACT_KC_LIST):
            chunk = (LAM_B * biasT[kc * P : (kc + 1) * P, qs]).astype(E4NP)
            b8[i, :, 0, :] = chunk
            b8[i, :, 1, :] = chunk
        bt = np.zeros((len(DVE_KC_LIST), P, TOK), BFNP)
        for i, kc in enumerate(DVE_KC_LIST):
            bt[i] = (
                ALPHA * biasT[kc * P : (kc + 1) * P, qs] + (BETA - ALPHA * C_SHIFT)
            ).astype(BFNP)
        in_maps2.append(
            {
                "q8_in": np.ascontiguousarray(q8_blks[c]),
                "k8_in": k8_full,
                "v8_in": v8_dr,
                "b8_in": b8,
                "bt_in": bt,
                "mres": np.ascontiguousarray(mres_full[qs]),
                "Wc8": Wc8,
            }
        )
    res2 = run_bass_kernel_spmd(nc2, in_maps2, core_ids=list(range(N_CORES)))
    _log("phase2 done")
    o = np.concatenate([r["o_out"] for r in res2.results], axis=0)[:N_CTX]
    if _want_timing is not None:
        _want_timing["res1"] = res1
        _want_timing["res2"] = res2
    return o.reshape(1, N_CTX, N_STATE).astype(np.float32)


# revision 16
# speedup vs baseline: 1.6354x; 1.2998x over previous
"""AttentionResblock on 8 NeuronCores (Trainium2, Bass/Tile) — fp8 edition.

Sharding: query-token blocks of 512 (T_PAD=4096 = 8 x 512), two launches:
  Phase 1 (per core c): LayerNorm + Q/K/V projections (fp8 DoubleRow matmuls)
    for token rows [512c, 512c+512). Emits q8/k8 in DoubleRow-ready
    [128, 2, 512] head-pair tiles and v8 token-major, all fp8-e4m3.
    gamma/beta/bv/bc are folded into weights/residual on the host.
  Phase 2 (per core c): 16-head attention for its 512 query rows over all
    4096 keys. QK via fp8 DoubleRow (2x32 contraction). Softmax weights are
    produced two ways, split across engines to balance the timeline:
      - ACT chunks: bias added in PSUM via fp8 ident-matmul, then true
        exp -> fp8-e4m3 (premultiplied by 512, shifted by C=9).
      - DVE chunks: fastexp bit trick - y = int8(alpha*s + [alpha*b +
        beta - alpha*C]) bitcast as fp8-e5m2 (2^(y/4-15) ~ 512*e^(s+b-9)).
    The scale/shift cancels in softmax: PV accumulates numerator and
    denominator (ones columns in the fp8 V tiles, value 0.5 = LAM_V/LAM_ATTN)
    with fp8 DoubleRow over key-chunk pairs. Normalize, fp8 DoubleRow output
    projection, f32 residual add.

Numerics validated against the reference in numpy (numerics2.py): rel err
~1.5e-4 vs the 2e-2 gate, dominated by fp8 quantization of the attention
branch, which is scaled by ~1e-3 through Wc so the f32 residual dominates.
"""

import sys

sys.path.insert(0, "/opt/trn_rl_repo")

from contextlib import ExitStack  # noqa: E402

import numpy as np  # noqa: E402
import ml_dtypes  # noqa: E402

import concourse.bass as bass  # noqa: E402
import concourse.bacc as bacc  # noqa: E402
import concourse.tile as tile  # noqa: E402
from concourse import mybir  # noqa: E402
from concourse.bass_utils import run_bass_kernel_spmd  # noqa: E402
from concourse.masks import make_identity  # noqa: E402

F32 = mybir.dt.float32
BF16 = mybir.dt.bfloat16
F8E4 = mybir.dt.float8e4
F8E5 = mybir.dt.float8e5
I8 = mybir.dt.int8
AF = mybir.ActivationFunctionType
ALU = mybir.AluOpType
DR = mybir.MatmulPerfMode.DoubleRow

E4NP = ml_dtypes.float8_e4m3
E5NP = ml_dtypes.float8_e5m2
BFNP = ml_dtypes.bfloat16

N_STATE = 1024
N_HEADS = 16
D_HEAD = 64
N_CTX = 4080
T_PAD = 4096
N_CORES = 8
TOK = 512
P = 128
LN_EPS = 1e-5
NSC = 8  # state chunks of 128
NTC = 4  # token chunks per core
NKC = 32  # key chunks of 128
NKCP = 16  # key-chunk pairs of 256
NPAIR = 8  # head pairs

# fp8 scale plan (see numerics2.py)
ALPHA = 4 * np.log2(np.e)  # logit scale in PSUM: psum = ALPHA*(s)
C_SHIFT = 9.0  # global logit shift (measured max 6.21)
PMULT = 512.0  # weights premultiplier (cancels in softmax)
BETA = 96.0  # 60 + 4*log2(PMULT)
LAM_R = 16.0  # LN output scale
LAM_W = 512.0  # Wq/Wk/Wv scale
LAM_Q = float(np.sqrt(ALPHA / 8.0))  # q/k scales; 8*LAM_Q*LAM_K = ALPHA
LAM_V = 16.0
LAM_B = 369.0  # ACT-path bias quant scale; ident diag g = 2^-6, g*LAM_B ~ ALPHA
G_IDENT = 2.0 ** -6
LAM_ATTN = 32.0
LAM_WC = 32768.0
ONES_VAL = LAM_V / LAM_ATTN  # 0.5, folded into denominator columns
GQ = LAM_Q / (LAM_R * LAM_W)
GV = LAM_V / (LAM_R * LAM_W)
G_OUT = 1.0 / (LAM_ATTN * LAM_WC)
EXP_BIAS = float(np.log(PMULT) - C_SHIFT)  # -2.7616
EXP_SCALE = float(1.0 / ALPHA)
STT_CLAMP = -40.0

# kc-pair -> engine assignment (per head-pair parity), tuned for balance
DVE_EVEN = (3, 5, 7, 9, 11, 13, 15)
DVE_ODD = (3, 5, 7, 9, 11, 13, 15)
ACT_KC_LIST = [kc for kcp in range(NKCP) if kcp not in DVE_EVEN
               for kc in (2 * kcp, 2 * kcp + 1)]  # 20 kcs ever handled by ACT
DVE_KC_LIST = [kc for kcp in DVE_ODD for kc in (2 * kcp, 2 * kcp + 1)]  # 14 kcs
ACT_SLOT = {kc: i for i, kc in enumerate(ACT_KC_LIST)}
DVE_SLOT = {kc: i for i, kc in enumerate(DVE_KC_LIST)}
VW = NPAIR * 130 + 32  # v8 tile width: per-pair 130 cols + tail padding
PSQK_BUFS = 2
PSPV_BUFS = 1
P_BUFS = 3
DIAG_SKIP_TAIL = False
DIAG_FORCE = None  # None | "ACT" | "DVE"


def _build_phase1() -> bass.Bass:
    nc = bacc.Bacc("TRN2", target_bir_lowering=False, debug=False, num_devices=N_CORES)
    m_blk = nc.dram_tensor("m_blk", [TOK, N_STATE], F32, kind="ExternalInput")
    Wq8 = nc.dram_tensor("Wq8", [P, 4, 2, N_STATE], F8E4, kind="ExternalInput")
    Wk8 = nc.dram_tensor("Wk8", [P, 4, 2, N_STATE], F8E4, kind="ExternalInput")
    Wv8 = nc.dram_tensor("Wv8", [P, 4, 2, N_STATE], F8E4, kind="ExternalInput")
    bqs = nc.dram_tensor("bqs", [N_STATE], F32, kind="ExternalInput")
    q8_out = nc.dram_tensor("q8_out", [4, P, 2, TOK], F8E4, kind="ExternalOutput")
    k8_out = nc.dram_tensor("k8_out", [4, P, 2, TOK], F8E4, kind="ExternalOutput")
    v8_out = nc.dram_tensor("v8_out", [TOK, N_STATE], F8E4, kind="ExternalOutput")

    with ExitStack() as ctx:
        tc = ctx.enter_context(tile.TileContext(nc))
        consts = ctx.enter_context(tc.tile_pool(name="consts", bufs=1))
        small = ctx.enter_context(tc.tile_pool(name="small", bufs=4))
        work = ctx.enter_context(tc.tile_pool(name="work", bufs=2))
        psum = ctx.enter_context(tc.tile_pool(name="psum", bufs=2, space="PSUM"))
        pst_pool = ctx.enter_context(tc.tile_pool(name="pst", bufs=2, space="PSUM"))

        identB = consts.tile([P, P], BF16)
        make_identity(nc, identB)
        eps_sb = consts.tile([P, 1], F32)
        nc.vector.memset(eps_sb, LN_EPS)
        bqs_sb = consts.tile([P, NSC], F32)
        nc.sync.dma_start(out=bqs_sb, in_=bqs.rearrange("(j p) -> p j", p=P))

        m_sb = consts.tile([P, NTC, N_STATE], F32)
        for tcn in range(NTC):
            nc.sync.dma_start(
                out=m_sb[:, tcn, :],
                in_=m_blk.rearrange("(c p) s -> p c s", p=P)[:, tcn, :],
            )
        w_sb = {}
        for name, w in (("Wq8", Wq8), ("Wk8", Wk8), ("Wv8", Wv8)):
            wb = consts.tile([P, 4, 2, N_STATE], F8E4, name=f"{name}_sb")
            nc.sync.dma_start(out=wb, in_=w[:, :, :, :])
            w_sb[name] = wb

        # LayerNorm -> xcB = (m - mu) * rstd * LAM_R in bf16
        xcB = consts.tile([P, NTC, N_STATE], BF16)
        for tcn in range(NTC):
            ssum = small.tile([P, 1], F32, tag="ssum")
            nc.vector.reduce_sum(ssum, m_sb[:, tcn, :], axis=mybir.AxisListType.X)
            negmean = small.tile([P, 1], F32, tag="negmean")
            nc.scalar.mul(negmean, ssum, -1.0 / N_STATE)
            sqscr = work.tile([P, N_STATE], BF16, tag="sqscr")
            sqsum = small.tile([P, 1], F32, tag="sqsum")
            nc.scalar.activation(
                out=sqscr, in_=m_sb[:, tcn, :], func=AF.Square, accum_out=sqsum
            )
            mean = small.tile([P, 1], F32, tag="mean")
            nc.scalar.mul(mean, ssum, 1.0 / N_STATE)
            msq = small.tile([P, 1], F32, tag="msq")
            nc.scalar.activation(out=msq, in_=mean, func=AF.Square)
            exx = small.tile([P, 1], F32, tag="exx")
            nc.scalar.mul(exx, sqsum, 1.0 / N_STATE)
            var = small.tile([P, 1], F32, tag="var")
            nc.vector.tensor_tensor(out=var, in0=exx, in1=msq, op=ALU.subtract)
            std = small.tile([P, 1], F32, tag="std")
            nc.scalar.activation(out=std, in_=var, func=AF.Sqrt, bias=eps_sb)
            rstd = small.tile([P, 1], F32, tag="rstd")
            nc.vector.reciprocal(rstd, std)
            rstdl = small.tile([P, 1], F32, tag="rstdl")
            nc.scalar.mul(rstdl, rstd, LAM_R)
            nc.gpsimd.tensor_scalar(
                out=xcB[:, tcn, :],
                in0=m_sb[:, tcn, :],
                scalar1=negmean,
                scalar2=rstdl,
                op0=ALU.add,
                op1=ALU.mult,
            )

        # transpose to state-major and quantize: rT8 [128, sc, 512] e4m3
        rT8 = consts.tile([P, NSC, TOK], F8E4)
        for sc in range(NSC):
            pst = pst_pool.tile([P, TOK], BF16, tag="pst")
            for tcn in range(NTC):
                nc.tensor.transpose(
                    pst[:, tcn * P : (tcn + 1) * P],
                    xcB[:, tcn, sc * P : (sc + 1) * P],
                    identB,
                )
            nc.vector.tensor_copy(rT8[:, sc, :], pst)

        # q/k: DoubleRow fp8 matmuls, evacuate into [128, 2, 512] pair tiles
        q8g = [consts.tile([P, 2, TOK], F8E4, name=f"q8g{g}") for g in range(4)]
        k8g = [consts.tile([P, 2, TOK], F8E4, name=f"k8g{g}") for g in range(4)]
        for j in range(NSC):
            g, half = j // 2, j % 2
            psq = psum.tile([P, TOK], F32, tag="psq")
            psk = psum.tile([P, TOK], F32, tag="psk")
            for s in range(4):
                nc.tensor.matmul(
                    psq,
                    lhsT=w_sb["Wq8"][:, s, :, j * P : (j + 1) * P],
                    rhs=rT8[:, 2 * s : 2 * s + 2, :],
                    start=(s == 0),
                    stop=(s == 3),
                    perf_mode=DR,
                )
            for s in range(4):
                nc.tensor.matmul(
                    psk,
                    lhsT=w_sb["Wk8"][:, s, :, j * P : (j + 1) * P],
                    rhs=rT8[:, 2 * s : 2 * s + 2, :],
                    start=(s == 0),
                    stop=(s == 3),
                    perf_mode=DR,
                )
            for t in range(2):
                nc.scalar.activation(
                    out=q8g[g][64 * half : 64 * half + 64, t, :],
                    in_=psq[64 * t : 64 * t + 64, :],
                    func=AF.Identity,
                    bias=bqs_sb[64 * t : 64 * t + 64, j : j + 1],
                    scale=GQ,
                )
                nc.vector.tensor_scalar(
                    out=k8g[g][64 * half : 64 * half + 64, t, :],
                    in0=psk[64 * t : 64 * t + 64, :],
                    scalar1=GQ,
                    scalar2=None,
                    op0=ALU.mult,
                )

        # v: token-major, e4m3
        v8sb = consts.tile([P, NTC, N_STATE], F8E4)
        for tcn in range(NTC):
            psv = psum.tile([P, N_STATE], F32, tag="psv", bufs=1)
            for pc in range(2):
                for s in range(4):
                    nc.tensor.matmul(
                        psv[:, pc * TOK : (pc + 1) * TOK],
                        lhsT=rT8[:, 2 * s : 2 * s + 2, tcn * P : (tcn + 1) * P],
                        rhs=w_sb["Wv8"][:, s, :, pc * TOK : (pc + 1) * TOK],
                        start=(s == 0),
                        stop=(s == 3),
                        perf_mode=DR,
                    )
            nc.scalar.mul(v8sb[:, tcn, :], psv, GV)

        for g in range(4):
            nc.sync.dma_start(out=q8_out[g, :, :, :], in_=q8g[g])
            nc.sync.dma_start(out=k8_out[g, :, :, :], in_=k8g[g])
        for tcn in range(NTC):
            nc.sync.dma_start(
                out=v8_out.rearrange("(c p) s -> p c s", p=P)[:, tcn, :],
                in_=v8sb[:, tcn, :],
            )
    nc.compile()
    return nc


def _build_phase2() -> bass.Bass:
    nc = bacc.Bacc("TRN2", target_bir_lowering=False, debug=False, num_devices=N_CORES)
    q8_in = nc.dram_tensor("q8_in", [4, P, 2, TOK], F8E4, kind="ExternalInput")
    k8_in = nc.dram_tensor("k8_in", [4, P, 2, T_PAD], F8E4, kind="ExternalInput")
    v8_in = nc.dram_tensor("v8_in", [4, P, NKC // 4, VW], F8E4, kind="ExternalInput")
    b8_in = nc.dram_tensor("b8_in", [len(ACT_KC_LIST), P, 2, TOK], F8E4, kind="ExternalInput")
    bt_in = nc.dram_tensor("bt_in", [len(DVE_KC_LIST), P, TOK], BF16, kind="ExternalInput")
    mres = nc.dram_tensor("mres", [TOK, N_STATE], F32, kind="ExternalInput")
    Wc8 = nc.dram_tensor("Wc8", [P, 4, 2, N_STATE], F8E4, kind="ExternalInput")
    o_out = nc.dram_tensor("o_out", [TOK, N_STATE], F32, kind="ExternalOutput")

    with ExitStack() as ctx:
        tc = ctx.enter_context(tile.TileContext(nc))
        consts = ctx.enter_context(tc.tile_pool(name="consts", bufs=1))
        small = ctx.enter_context(tc.tile_pool(name="small", bufs=4))
        ppool = ctx.enter_context(tc.tile_pool(name="ppool", bufs=P_BUFS))
        psqk = ctx.enter_context(tc.tile_pool(name="psqk", bufs=PSQK_BUFS, space="PSUM"))
        pspv = ctx.enter_context(tc.tile_pool(name="pspv", bufs=PSPV_BUFS, space="PSUM"))

        identg = consts.tile([P, 2, P], F8E4)
        nc.vector.memset(identg, 0.0)
        make_identity(nc, identg[:, 0, :])
        make_identity(nc, identg[:, 1, :])
        identg2 = consts.tile([P, 2, P], F8E4)
        nc.scalar.mul(identg2, identg, G_IDENT / 2.0)
        expb_ap = consts.tile([P, 1], F32)
        nc.vector.memset(expb_ap, EXP_BIAS)

        q8sb = consts.tile([P, 4, 2, TOK], F8E4)
        k8sb = consts.tile([P, 4, 2, T_PAD], F8E4)
        v8sb = consts.tile([P, NKC, VW], F8E4)
        b8sb = consts.tile([P, len(ACT_KC_LIST), 2, TOK], F8E4)
        btsb = consts.tile([P, len(DVE_KC_LIST), TOK], BF16)
        m_sb = consts.tile([P, NTC, N_STATE], F32)
        wc_sb = consts.tile([P, 4, 2, N_STATE], F8E4)
        nA, nD = len(ACT_KC_LIST), len(DVE_KC_LIST)
        bA = [0, 5, 10, 15, nA]
        bD = [0, 4, 8, 11, nD]
        nc.sync.dma_start(out=q8sb, in_=q8_in[:, :, :, :].rearrange("g p t n -> p g t n"))
        for g in range(4):
            nc.sync.dma_start(out=k8sb[:, g, :, :], in_=k8_in[g, :, :, :])
            nc.sync.dma_start(
                out=b8sb[:, bA[g] : bA[g + 1], :, :],
                in_=b8_in[bA[g] : bA[g + 1], :, :, :].rearrange("k p t n -> p k t n"),
            )
            nc.sync.dma_start(
                out=btsb[:, bD[g] : bD[g + 1], :],
                in_=bt_in[bD[g] : bD[g + 1], :, :].rearrange("k p n -> p k n"),
            )
            nc.sync.dma_start(
                out=v8sb[:, g * 8 : (g + 1) * 8, :], in_=v8_in[g, :, :, :]
            )
        nc.sync.dma_start(out=m_sb, in_=mres.rearrange("(c p) s -> p c s", p=P))
        nc.sync.dma_start(out=wc_sb, in_=Wc8[:, :, :, :])

        attnT8 = consts.tile([P, NPAIR, TOK], F8E4)

        for j in range(NPAIR):
            g, half = j // 2, j % 2
            dve_set = DVE_ODD if (j % 2) else DVE_EVEN
            if DIAG_FORCE == "ACT":
                dve_set = ()
            elif DIAG_FORCE == "DVE":
                dve_set = tuple(range(NKCP))
            pvA = pspv.tile([96, TOK], F32, tag="pvA")
            pvB = pspv.tile([96, TOK], F32, tag="pvB")
            for kcp in range(NKCP):
                is_dve = kcp in dve_set
                if is_dve:
                    ptile = ppool.tile([P, 2, 2, TOK], F8E5, tag="pD")
                else:
                    ptile = ppool.tile([P, 2, 2, TOK], F8E4, tag="pA")
                for sub in range(2):
                    kc = 2 * kcp + sub
                    if is_dve:
                        # DVE stream: per-head 1-bank psums, independent of
                        # the ACT stream so both engines pipeline in parallel
                        for h in range(2):
                            base = 64 * half + 32 * h
                            psd = psqk.tile([P, TOK], F32, tag="sD")
                            nc.tensor.matmul(
                                psd,
                                lhsT=k8sb[base : base + 32, g, :, kc * P : (kc + 1) * P],
                                rhs=q8sb[base : base + 32, g, :, :],
                                start=True,
                                stop=True,
                                perf_mode=DR,
                                tile_position=(base, 0),
                            )
                            nc.vector.scalar_tensor_tensor(
                                out=ptile[:, h, sub, :].bitcast(I8),
                                in0=psd,
                                scalar=STT_CLAMP,
                                in1=btsb[:, DVE_SLOT.get(kc, kc % len(DVE_KC_LIST)), :],
                                op0=ALU.max,
                                op1=ALU.add,
                            )
                    else:
                        ps = psqk.tile([P, 2, TOK], F32, tag="sA")
                        for h in range(2):
                            base = 64 * half + 32 * h
                            nc.tensor.matmul(
                                ps[:, h, :],
                                lhsT=k8sb[base : base + 32, g, :, kc * P : (kc + 1) * P],
                                rhs=q8sb[base : base + 32, g, :, :],
                                start=True,
                                stop=False,
                                perf_mode=DR,
                                tile_position=(base, 0),
                            )
                            nc.tensor.matmul(
                                ps[:, h, :],
                                lhsT=identg2,
                                rhs=b8sb[:, ACT_SLOT.get(kc, kc % len(ACT_KC_LIST)), :, :],
                                start=False,
                                stop=True,
                                perf_mode=DR,
                                skip_group_check=True,
                            )
                        nc.scalar.activation(
                            out=ptile[:, :, sub, :],
                            in_=ps,
                            func=AF.Exp,
                            bias=expb_ap,
                            scale=EXP_SCALE,
                        )
                nc.tensor.matmul(
                    pvA,
                    lhsT=v8sb[:, 2 * kcp : 2 * kcp + 2, 130 * j : 130 * j + 96],
                    rhs=ptile[:, 0, :, :],
                    start=(kcp == 0),
                    stop=(kcp == NKCP - 1),
                    perf_mode=DR,
                )
                nc.tensor.matmul(
                    pvB,
                    lhsT=v8sb[:, 2 * kcp : 2 * kcp + 2, 130 * j + 65 : 130 * j + 161],
                    rhs=ptile[:, 1, :, :],
                    start=(kcp == 0),
                    stop=(kcp == NKCP - 1),
                    perf_mode=DR,
                )

            if DIAG_SKIP_TAIL:
                nc.vector.memset(attnT8[:, j, :], 0.01)
                continue
            # fast evac: stage pv to SBUF (frees the psum banks), then
            # normalize off the critical path using the idle Pool engine
            stA = small.tile([64, TOK], BF16, tag="stA", bufs=2)
            stB = small.tile([64, TOK], BF16, tag="stB", bufs=2)
            nc.scalar.copy(stA, pvA[0:64, :])
            nc.vector.tensor_copy(stB, pvB[0:64, :])
            sd = small.tile([1, 2, TOK], F32, tag="sd", bufs=2)
            nc.scalar.copy(sd[:, 0, :], pvA[64:65, :])
            nc.vector.tensor_copy(sd[:, 1, :], pvB[64:65, :])
            rec = small.tile([1, 2, TOK], BF16, tag="rec", bufs=2)
            with nc.allow_low_precision("bf16 softmax denominators, ~0.4% scale"):
                nc.vector.reciprocal(rec, sd)
            bcastA = small.tile([64, TOK], BF16, tag="bcastA", bufs=2)
            bcastB = small.tile([64, TOK], BF16, tag="bcastB", bufs=2)
            nc.gpsimd.partition_broadcast(bcastA, rec[:, 0, :], channels=64)
            nc.gpsimd.partition_broadcast(bcastB, rec[:, 1, :], channels=64)
            nc.gpsimd.tensor_tensor(
                out=attnT8[0:64, j, :], in0=stA, in1=bcastA, op=ALU.mult
            )
            nc.gpsimd.tensor_tensor(
                out=attnT8[64:128, j, :], in0=stB, in1=bcastB, op=ALU.mult
            )

        # output projection (fp8 DR) + residual add (f32)
        o_sb = consts.tile([P, NTC, N_STATE], F32)
        for qc in range(NTC):
            ps_o = psqk.tile([P, 2, TOK], F32, tag="sA")
            po = ps_o.rearrange("p a b -> p (a b)")
            for pc in range(2):
                for u in range(4):
                    nc.tensor.matmul(
                        ps_o[:, pc, :],
                        lhsT=attnT8[:, 2 * u : 2 * u + 2, qc * P : (qc + 1) * P],
                        rhs=wc_sb[:, u, :, pc * TOK : (pc + 1) * TOK],
                        start=(u == 0),
                        stop=(u == 3),
                        perf_mode=DR,
                    )
            nc.vector.scalar_tensor_tensor(
                out=o_sb[:, qc, :],
                in0=po,
                scalar=G_OUT,
                in1=m_sb[:, qc, :],
                op0=ALU.mult,
                op1=ALU.add,
            )
        for qc in range(NTC):
            nc.sync.dma_start(
                out=o_out.rearrange("(c p) s -> p c s", p=P)[:, qc, :],
                in_=o_sb[:, qc, :],
            )
    nc.compile()
    return nc


_NC_CACHE = {}


def _get_nc(which):
    if which not in _NC_CACHE:
        _NC_CACHE[which] = _build_phase1() if which == 1 else _build_phase2()
    return _NC_CACHE[which]


def _perm_cols():
    """Column permutation for q/k weights: per pair j, [hA d0:32 | hB d0:32 |
    hA d32:64 | hB d32:64]."""
    order = []
    for j in range(NSC):
        hA, hB = 2 * j, 2 * j + 1
        order.extend(range(hA * 64, hA * 64 + 32))
        order.extend(range(hB * 64, hB * 64 + 32))
        order.extend(range(hA * 64 + 32, hA * 64 + 64))
        order.extend(range(hB * 64 + 32, hB * 64 + 64))
    return np.array(order)


def _w_dr_layout(w8):
    """[1024, C] -> [128, 4, 2, C] DoubleRow lhsT layout."""
    return np.ascontiguousarray(
        w8.reshape(4, 2, P, -1).transpose(2, 0, 1, 3)
    )


def kernel(m, bias, gamma, beta, Wq, bq, Wk, Wv, bv, Wc, bc, _want_timing=None):
    m = np.asarray(m, dtype=np.float32).reshape(N_CTX, N_STATE)
    bias = np.asarray(bias, np.float32)
    gamma = np.asarray(gamma, np.float32)
    beta = np.asarray(beta, np.float32)
    Wq = np.asarray(Wq, np.float32)
    Wk = np.asarray(Wk, np.float32)
    Wv = np.asarray(Wv, np.float32)
    Wc = np.asarray(Wc, np.float32)
    bq = np.asarray(bq, np.float32)
    bv = np.asarray(bv, np.float32)
    bc = np.asarray(bc, np.float32)

    m_pad = np.zeros((T_PAD, N_STATE), np.float32)
    m_pad[:N_CTX] = m

    # fold gamma into weights, beta into biases; bv and bc fold into residual
    Wqf = gamma[:, None] * Wq
    Wkf = gamma[:, None] * Wk
    Wvf = gamma[:, None] * Wv
    bqf = bq + beta @ Wq
    # beta@Wk shifts all logits of a query equally -> softmax invariant; drop.
    # beta@Wv + bv shift attention output -> fold into residual with bc.
    perm = _perm_cols()
    Wq8 = _w_dr_layout((LAM_W * Wqf[:, perm]).astype(E4NP))
    Wk8 = _w_dr_layout((LAM_W * Wkf[:, perm]).astype(E4NP))
    Wv8 = _w_dr_layout((LAM_W * Wvf).astype(E4NP))
    bqs = (LAM_Q * bqf[perm]).astype(np.float32)
    Wc8 = _w_dr_layout((LAM_WC * Wc).astype(E4NP))
    mres_full = m_pad + (bc + (bv + beta @ Wv) @ Wc)[None, :]

    import sys as _sys

    def _log(*a):
        print("[kernel]", *a, file=_sys.stderr, flush=True)

    nc1 = _get_nc(1)
    _log("phase1 built")
    in_maps1 = []
    for c in range(N_CORES):
        in_maps1.append(
            {
                "m_blk": np.ascontiguousarray(m_pad[c * TOK : (c + 1) * TOK]),
                "Wq8": Wq8,
                "Wk8": Wk8,
                "Wv8": Wv8,
                "bqs": bqs,
            }
        )
    res1 = run_bass_kernel_spmd(nc1, in_maps1, core_ids=list(range(N_CORES)))
    _log("phase1 done")

    q8_blks = [r["q8_out"] for r in res1.results]
    k8_full = np.concatenate([r["k8_out"] for r in res1.results], axis=3)
    v8_full = np.concatenate([r["v8_out"] for r in res1.results], axis=0)
    v8_full[N_CTX:] = 0  # pad tokens carry no value

    # v8 pair-tile layout [128, 32, VW] with denominator columns
    v8f = v8_full.astype(np.float32).reshape(NKC, P, N_HEADS, D_HEAD)
    v8h = np.zeros((P, NKC, VW), np.float32)
    for j in range(NPAIR):
        v8h[:, :, 130 * j : 130 * j + 64] = v8f[:, :, 2 * j].transpose(1, 0, 2)
        v8h[:, :, 130 * j + 65 : 130 * j + 129] = v8f[:, :, 2 * j + 1].transpose(1, 0, 2)
        v8h[:, :, 130 * j + 64] = ONES_VAL
        v8h[:, :, 130 * j + 129] = ONES_VAL
    # zero the denominator contribution of padded keys
    keyidx = (np.arange(NKC)[None, :] * P + np.arange(P)[:, None])  # [p, kc]
    padmask = keyidx >= N_CTX
    for j in range(NPAIR):
        v8h[:, :, 130 * j + 64][padmask] = 0.0
        v8h[:, :, 130 * j + 129][padmask] = 0.0
    v8h8 = v8h.astype(E4NP)
    v8_dr = np.ascontiguousarray(
        v8h8.reshape(P, 4, NKC // 4, VW).transpose(1, 0, 2, 3)
    )

    biasT = np.ascontiguousarray(bias.T)  # [k, q]

    nc2 = _get_nc(2)
    _log("phase2 built")
    in_maps2 = []
    for c in range(N_CORES):
        qs = slice(c * TOK, (c + 1) * TOK)
        b8 = np.zeros((len(ACT_KC_LIST), P, 2, TOK), E4NP)
        for i, kc in enumerate(ACT_KC_LIST):
            chunk = (LAM_B * biasT[kc * P : (kc + 1) * P, qs]).astype(E4NP)
            b8[i, :, 0, :] = chunk
            b8[i, :, 1, :] = chunk
        bt = np.zeros((len(DVE_KC_LIST), P, TOK), BFNP)
        for i, kc in enumerate(DVE_KC_LIST):
            bt[i] = (
                ALPHA * biasT[kc * P : (kc + 1) * P, qs] + (BETA - ALPHA * C_SHIFT)
            ).astype(BFNP)
        in_maps2.append(
            {
                "q8_in": np.ascontiguousarray(q8_blks[c]),
                "k8_in": k8_full,
                "v8_in": v8_dr,
                "b8_in": b8,
                "bt_in": bt,
                "mres": np.ascontiguousarray(mres_full[qs]),
                "Wc8": Wc8,
            }
        )
    res2 = run_bass_kernel_spmd(nc2, in_maps2, core_ids=list(range(N_CORES)))
    _log("phase2 done")
    o = np.concatenate([r["o_out"] for r in res2.results], axis=0)[:N_CTX]
    if _want_timing is not None:
        _want_timing["res1"] = res1
        _want_timing["res2"] = res2
    return o.reshape(1, N_CTX, N_STATE).astype(np.float32)


# revision 18
# speedup vs baseline: 1.6992x; 1.0390x over previous
"""AttentionResblock on 8 NeuronCores (Trainium2, Bass/Tile) — fp8 edition.

Sharding: query-token blocks of 512 (T_PAD=4096 = 8 x 512), two launches:
  Phase 1 (per core c): LayerNorm + Q/K/V projections (fp8 DoubleRow matmuls)
    for token rows [512c, 512c+512). Emits q8/k8 in DoubleRow-ready
    [128, 2, 512] head-pair tiles and v8 token-major, all fp8-e4m3.
    gamma/beta/bv/bc are folded into weights/residual on the host.
  Phase 2 (per core c): 16-head attention for its 512 query rows over all
    4096 keys. QK via fp8 DoubleRow (2x32 contraction). Softmax weights are
    produced two ways, split across engines to balance the timeline:
      - ACT chunks: bias added in PSUM via fp8 ident-matmul, then true
        exp -> fp8-e4m3 (premultiplied by 512, shifted by C=9).
      - DVE chunks: fastexp bit trick - y = int8(alpha*s + [alpha*b +
        beta - alpha*C]) bitcast as fp8-e5m2 (2^(y/4-15) ~ 512*e^(s+b-9)).
    The scale/shift cancels in softmax: PV accumulates numerator and
    denominator (ones columns in the fp8 V tiles, value 0.5 = LAM_V/LAM_ATTN)
    with fp8 DoubleRow over key-chunk pairs. Normalize, fp8 DoubleRow output
    projection, f32 residual add.

Numerics validated against the reference in numpy (numerics2.py): rel err
~1.5e-4 vs the 2e-2 gate, dominated by fp8 quantization of the attention
branch, which is scaled by ~1e-3 through Wc so the f32 residual dominates.
"""

import sys

sys.path.insert(0, "/opt/trn_rl_repo")

from contextlib import ExitStack  # noqa: E402

import numpy as np  # noqa: E402
import ml_dtypes  # noqa: E402

import concourse.bass as bass  # noqa: E402
import concourse.bacc as bacc  # noqa: E402
import concourse.tile as tile  # noqa: E402
from concourse import mybir  # noqa: E402
from concourse.bass_utils import run_bass_kernel_spmd  # noqa: E402
from concourse.masks import make_identity  # noqa: E402

F32 = mybir.dt.float32
BF16 = mybir.dt.bfloat16
F8E4 = mybir.dt.float8e4
F8E5 = mybir.dt.float8e5
I8 = mybir.dt.int8
AF = mybir.ActivationFunctionType
ALU = mybir.AluOpType
DR = mybir.MatmulPerfMode.DoubleRow

E4NP = ml_dtypes.float8_e4m3
E5NP = ml_dtypes.float8_e5m2
BFNP = ml_dtypes.bfloat16

N_STATE = 1024
N_HEADS = 16
D_HEAD = 64
N_CTX = 4080
T_PAD = 4096
N_CORES = 8
TOK = 512
P = 128
LN_EPS = 1e-5
NSC = 8  # state chunks of 128
NTC = 4  # token chunks per core
NKC = 32  # key chunks of 128
NKCP = 16  # key-chunk pairs of 256
NPAIR = 8  # head pairs

# fp8 scale plan (see numerics2.py)
ALPHA = 4 * np.log2(np.e)  # logit scale in PSUM: psum = ALPHA*(s)
C_SHIFT = 9.0  # global logit shift (measured max 6.21)
PMULT = 512.0  # weights premultiplier (cancels in softmax)
BETA = 96.0  # 60 + 4*log2(PMULT)
LAM_R = 16.0  # LN output scale
LAM_W = 512.0  # Wq/Wk/Wv scale
LAM_Q = float(np.sqrt(ALPHA / 8.0))  # q/k scales; 8*LAM_Q*LAM_K = ALPHA
LAM_V = 16.0
LAM_B = 369.0  # ACT-path bias quant scale; ident diag g = 2^-6, g*LAM_B ~ ALPHA
G_IDENT = 2.0 ** -6
LAM_ATTN = 32.0
LAM_WC = 32768.0
ONES_VAL = LAM_V / LAM_ATTN  # 0.5, folded into denominator columns
GQ = LAM_Q / (LAM_R * LAM_W)
GV = LAM_V / (LAM_R * LAM_W)
G_OUT = 1.0 / (LAM_ATTN * LAM_WC)
EXP_BIAS = float(np.log(PMULT) - C_SHIFT)  # -2.7616
EXP_SCALE = float(1.0 / ALPHA)
STT_CLAMP = -40.0

# kc-pair -> engine assignment (per head-pair parity), tuned for balance
DVE_EVEN = (3, 5, 7, 9, 11, 13, 15)
DVE_ODD = (3, 5, 7, 9, 11, 13, 15)
ACT_KC_LIST = [kc for kcp in range(NKCP) if kcp not in DVE_EVEN
               for kc in (2 * kcp, 2 * kcp + 1)]  # 20 kcs ever handled by ACT
DVE_KC_LIST = [kc for kcp in DVE_ODD for kc in (2 * kcp, 2 * kcp + 1)]  # 14 kcs
ACT_SLOT = {kc: i for i, kc in enumerate(ACT_KC_LIST)}
DVE_SLOT = {kc: i for i, kc in enumerate(DVE_KC_LIST)}
VW = NPAIR * 130 + 32  # v8 tile width: per-pair 130 cols + tail padding
PSQK_BUFS = 2
PSPV_BUFS = 1
P_BUFS = 3
DIAG_SKIP_TAIL = False
DIAG_FORCE = None  # None | "ACT" | "DVE"


def _build_phase1() -> bass.Bass:
    nc = bacc.Bacc("TRN2", target_bir_lowering=False, debug=False, num_devices=N_CORES)
    m_blk = nc.dram_tensor("m_blk", [TOK, N_STATE], F32, kind="ExternalInput")
    Wq8 = nc.dram_tensor("Wq8", [P, 4, 2, N_STATE], F8E4, kind="ExternalInput")
    Wk8 = nc.dram_tensor("Wk8", [P, 4, 2, N_STATE], F8E4, kind="ExternalInput")
    Wv8 = nc.dram_tensor("Wv8", [P, 4, 2, N_STATE], F8E4, kind="ExternalInput")
    bqs = nc.dram_tensor("bqs", [N_STATE], F32, kind="ExternalInput")
    q8_out = nc.dram_tensor("q8_out", [4, P, 2, TOK], F8E4, kind="ExternalOutput")
    k8_out = nc.dram_tensor("k8_out", [4, P, 2, TOK], F8E4, kind="ExternalOutput")
    v8_out = nc.dram_tensor("v8_out", [TOK, N_STATE], F8E4, kind="ExternalOutput")

    with ExitStack() as ctx:
        tc = ctx.enter_context(tile.TileContext(nc))
        consts = ctx.enter_context(tc.tile_pool(name="consts", bufs=1))
        small = ctx.enter_context(tc.tile_pool(name="small", bufs=4))
        work = ctx.enter_context(tc.tile_pool(name="work", bufs=2))
        psum = ctx.enter_context(tc.tile_pool(name="psum", bufs=2, space="PSUM"))
        pst_pool = ctx.enter_context(tc.tile_pool(name="pst", bufs=2, space="PSUM"))

        identB = consts.tile([P, P], BF16)
        make_identity(nc, identB)
        eps_sb = consts.tile([P, 1], F32)
        nc.vector.memset(eps_sb, LN_EPS)
        bqs_sb = consts.tile([P, NSC], F32)
        nc.sync.dma_start(out=bqs_sb, in_=bqs.rearrange("(j p) -> p j", p=P))

        m_sb = consts.tile([P, NTC, N_STATE], F32)
        w_sb = {}
        for name, w in (("Wq8", Wq8), ("Wk8", Wk8), ("Wv8", Wv8)):
            w_sb[name] = consts.tile([P, 4, 2, N_STATE], F8E4, name=f"{name}_sb")

        def ld_m(tcn):
            nc.sync.dma_start(
                out=m_sb[:, tcn, :],
                in_=m_blk.rearrange("(c p) s -> p c s", p=P)[:, tcn, :],
            )

        ld_m(0)
        ld_m(1)
        ld_m(2)
        ld_m(3)
        nc.sync.dma_start(out=w_sb["Wq8"], in_=Wq8[:, :, :, :])
        nc.sync.dma_start(out=w_sb["Wk8"], in_=Wk8[:, :, :, :])
        nc.sync.dma_start(out=w_sb["Wv8"], in_=Wv8[:, :, :, :])

        # LayerNorm -> xcB = (m - mu) * rstd * LAM_R in bf16
        xcB = consts.tile([P, NTC, N_STATE], BF16)
        for tcn in range(NTC):
            ssum = small.tile([P, 1], F32, tag="ssum")
            nc.vector.reduce_sum(ssum, m_sb[:, tcn, :], axis=mybir.AxisListType.X)
            negmean = small.tile([P, 1], F32, tag="negmean")
            nc.scalar.mul(negmean, ssum, -1.0 / N_STATE)
            sqscr = work.tile([P, N_STATE], BF16, tag="sqscr")
            sqsum = small.tile([P, 1], F32, tag="sqsum")
            nc.scalar.activation(
                out=sqscr, in_=m_sb[:, tcn, :], func=AF.Square, accum_out=sqsum
            )
            # 1024*var = sqsum - ssum^2/1024
            musq = small.tile([P, 1], F32, tag="musq")
            nc.vector.scalar_tensor_tensor(
                out=musq, in0=ssum, scalar=-1.0 / N_STATE, in1=ssum,
                op0=ALU.mult, op1=ALU.mult,
            )
            nvar = small.tile([P, 1], F32, tag="nvar")
            nc.vector.tensor_tensor(out=nvar, in0=sqsum, in1=musq, op=ALU.add)
            std = small.tile([P, 1], F32, tag="std")
            nc.scalar.activation(
                out=std, in_=nvar, func=AF.Sqrt, bias=eps_sb, scale=1.0 / N_STATE
            )
            rstd = small.tile([P, 1], F32, tag="rstd")
            nc.vector.reciprocal(rstd, std)
            rstdl = small.tile([P, 1], F32, tag="rstdl")
            nc.scalar.mul(rstdl, rstd, LAM_R)
            nc.gpsimd.tensor_scalar(
                out=xcB[:, tcn, :],
                in0=m_sb[:, tcn, :],
                scalar1=negmean,
                scalar2=rstdl,
                op0=ALU.add,
                op1=ALU.mult,
            )

        # transpose to state-major and quantize: rT8 [128, sc, 512] e4m3
        rT8 = consts.tile([P, NSC, TOK], F8E4)
        for sc in range(NSC):
            pst = pst_pool.tile([P, TOK], BF16, tag="pst")
            for tcn in range(NTC):
                nc.tensor.transpose(
                    pst[:, tcn * P : (tcn + 1) * P],
                    xcB[:, tcn, sc * P : (sc + 1) * P],
                    identB,
                )
            if sc % 2 == 0:
                nc.vector.tensor_copy(rT8[:, sc, :], pst)
            else:
                nc.scalar.copy(rT8[:, sc, :], pst)

        # q/k: DoubleRow fp8 matmuls, evacuate into [128, 2, 512] pair tiles
        q8g = [consts.tile([P, 2, TOK], F8E4, name=f"q8g{g}") for g in range(4)]
        k8g = [consts.tile([P, 2, TOK], F8E4, name=f"k8g{g}") for g in range(4)]
        for j in range(NSC):
            g, half = j // 2, j % 2
            psq = psum.tile([P, TOK], F32, tag="psq")
            psk = psum.tile([P, TOK], F32, tag="psk")
            for s in range(4):
                nc.tensor.matmul(
                    psq,
                    lhsT=w_sb["Wq8"][:, s, :, j * P : (j + 1) * P],
                    rhs=rT8[:, 2 * s : 2 * s + 2, :],
                    start=(s == 0),
                    stop=(s == 3),
                    perf_mode=DR,
                )
            for s in range(4):
                nc.tensor.matmul(
                    psk,
                    lhsT=w_sb["Wk8"][:, s, :, j * P : (j + 1) * P],
                    rhs=rT8[:, 2 * s : 2 * s + 2, :],
                    start=(s == 0),
                    stop=(s == 3),
                    perf_mode=DR,
                )
            for t in range(2):
                nc.scalar.activation(
                    out=q8g[g][64 * half : 64 * half + 64, t, :],
                    in_=psq[64 * t : 64 * t + 64, :],
                    func=AF.Identity,
                    bias=bqs_sb[64 * t : 64 * t + 64, j : j + 1],
                    scale=GQ,
                )
                nc.vector.tensor_scalar(
                    out=k8g[g][64 * half : 64 * half + 64, t, :],
                    in0=psk[64 * t : 64 * t + 64, :],
                    scalar1=GQ,
                    scalar2=None,
                    op0=ALU.mult,
                )

        # v: token-major, e4m3
        v8sb = consts.tile([P, NTC, N_STATE], F8E4)
        for tcn in range(NTC):
            psv = psum.tile([P, N_STATE], F32, tag="psv", bufs=1)
            for pc in range(2):
                for s in range(4):
                    nc.tensor.matmul(
                        psv[:, pc * TOK : (pc + 1) * TOK],
                        lhsT=rT8[:, 2 * s : 2 * s + 2, tcn * P : (tcn + 1) * P],
                        rhs=w_sb["Wv8"][:, s, :, pc * TOK : (pc + 1) * TOK],
                        start=(s == 0),
                        stop=(s == 3),
                        perf_mode=DR,
                    )
            nc.scalar.mul(v8sb[:, tcn, :], psv, GV)

        for g in range(4):
            nc.sync.dma_start(out=q8_out[g, :, :, :], in_=q8g[g])
            nc.sync.dma_start(out=k8_out[g, :, :, :], in_=k8g[g])
        for tcn in range(NTC):
            nc.sync.dma_start(
                out=v8_out.rearrange("(c p) s -> p c s", p=P)[:, tcn, :],
                in_=v8sb[:, tcn, :],
            )
    nc.compile()
    return nc


def _build_phase2() -> bass.Bass:
    nc = bacc.Bacc("TRN2", target_bir_lowering=False, debug=False, num_devices=N_CORES)
    q8_in = nc.dram_tensor("q8_in", [4, P, 2, TOK], F8E4, kind="ExternalInput")
    k8_in = nc.dram_tensor("k8_in", [4, P, 2, T_PAD], F8E4, kind="ExternalInput")
    v8_in = nc.dram_tensor("v8_in", [4, P, NKC // 4, VW], F8E4, kind="ExternalInput")
    b8_in = nc.dram_tensor("b8_in", [len(ACT_KC_LIST), P, 2, TOK], F8E4, kind="ExternalInput")
    bt_in = nc.dram_tensor("bt_in", [len(DVE_KC_LIST), P, TOK], BF16, kind="ExternalInput")
    mres = nc.dram_tensor("mres", [TOK, N_STATE], F32, kind="ExternalInput")
    Wc8 = nc.dram_tensor("Wc8", [P, 4, 2, N_STATE], F8E4, kind="ExternalInput")
    o_out = nc.dram_tensor("o_out", [TOK, N_STATE], F32, kind="ExternalOutput")

    with ExitStack() as ctx:
        tc = ctx.enter_context(tile.TileContext(nc))
        consts = ctx.enter_context(tc.tile_pool(name="consts", bufs=1))
        small = ctx.enter_context(tc.tile_pool(name="small", bufs=4))
        ppool = ctx.enter_context(tc.tile_pool(name="ppool", bufs=P_BUFS))
        psqk = ctx.enter_context(tc.tile_pool(name="psqk", bufs=PSQK_BUFS, space="PSUM"))
        pspv = ctx.enter_context(tc.tile_pool(name="pspv", bufs=PSPV_BUFS, space="PSUM"))

        identg = consts.tile([P, 2, P], F8E4)
        nc.vector.memset(identg, 0.0)
        make_identity(nc, identg[:, 0, :])
        make_identity(nc, identg[:, 1, :])
        identg2 = consts.tile([P, 2, P], F8E4)
        nc.scalar.mul(identg2, identg, G_IDENT / 2.0)
        expb_ap = consts.tile([P, 1], F32)
        nc.vector.memset(expb_ap, EXP_BIAS)

        q8sb = consts.tile([P, 4, 2, TOK], F8E4)
        k8sb = consts.tile([P, 4, 2, T_PAD], F8E4)
        v8sb = consts.tile([P, NKC, VW], F8E4)
        b8sb = consts.tile([P, len(ACT_KC_LIST), 2, TOK], F8E4)
        btsb = consts.tile([P, len(DVE_KC_LIST), TOK], BF16)
        m_sb = consts.tile([P, NTC, N_STATE], F32)
        wc_sb = consts.tile([P, 4, 2, N_STATE], F8E4)
        nA, nD = len(ACT_KC_LIST), len(DVE_KC_LIST)
        bA = [0, 5, 10, 15, nA]
        bD = [0, 4, 8, 11, nD]

        def ld_q8(g):
            nc.sync.dma_start(out=q8sb[:, g, :, :], in_=q8_in[g, :, :, :])

        def ld_k8(g):
            nc.sync.dma_start(out=k8sb[:, g, :, :], in_=k8_in[g, :, :, :])

        def ld_b8(i):
            nc.sync.dma_start(
                out=b8sb[:, bA[i] : bA[i + 1], :, :],
                in_=b8_in[bA[i] : bA[i + 1], :, :, :].rearrange("k p t n -> p k t n"),
            )

        def ld_bt(i):
            nc.sync.dma_start(
                out=btsb[:, bD[i] : bD[i + 1], :],
                in_=bt_in[bD[i] : bD[i + 1], :, :].rearrange("k p n -> p k n"),
            )

        def ld_v8(q):
            nc.sync.dma_start(
                out=v8sb[:, q * 8 : (q + 1) * 8, :], in_=v8_in[q, :, :, :]
            )

        ld_q8(0)
        ld_k8(0)
        ld_b8(0)
        ld_bt(0)
        ld_v8(0)
        ld_b8(1)
        ld_bt(1)
        ld_v8(1)
        ld_b8(2)
        ld_bt(2)
        ld_v8(2)
        ld_b8(3)
        ld_bt(3)
        ld_v8(3)
        ld_k8(1)
        ld_q8(1)
        ld_k8(2)
        ld_q8(2)
        ld_k8(3)
        ld_q8(3)
        nc.sync.dma_start(out=m_sb, in_=mres.rearrange("(c p) s -> p c s", p=P))
        nc.sync.dma_start(out=wc_sb, in_=Wc8[:, :, :, :])

        attnT8 = consts.tile([P, NPAIR, TOK], F8E4)

        for j in range(NPAIR):
            g, half = j // 2, j % 2
            dve_set = DVE_ODD if (j % 2) else DVE_EVEN
            if DIAG_FORCE == "ACT":
                dve_set = ()
            elif DIAG_FORCE == "DVE":
                dve_set = tuple(range(NKCP))
            pvA = pspv.tile([96, TOK], F32, tag="pvA")
            pvB = pspv.tile([96, TOK], F32, tag="pvB")
            for kcp in range(NKCP):
                is_dve = kcp in dve_set
                if is_dve:
                    ptile = ppool.tile([P, 2, 2, TOK], F8E5, tag="pD")
                else:
                    ptile = ppool.tile([P, 2, 2, TOK], F8E4, tag="pA")
                for sub in range(2):
                    kc = 2 * kcp + sub
                    if is_dve:
                        # DVE stream: per-head 1-bank psums, independent of
                        # the ACT stream so both engines pipeline in parallel
                        for h in range(2):
                            base = 64 * half + 32 * h
                            psd = psqk.tile([P, TOK], F32, tag="sD")
                            nc.tensor.matmul(
                                psd,
                                lhsT=k8sb[base : base + 32, g, :, kc * P : (kc + 1) * P],
                                rhs=q8sb[base : base + 32, g, :, :],
                                start=True,
                                stop=True,
                                perf_mode=DR,
                                tile_position=(base, 0),
                            )
                            nc.vector.scalar_tensor_tensor(
                                out=ptile[:, h, sub, :].bitcast(I8),
                                in0=psd,
                                scalar=STT_CLAMP,
                                in1=btsb[:, DVE_SLOT.get(kc, kc % len(DVE_KC_LIST)), :],
                                op0=ALU.max,
                                op1=ALU.add,
                            )
                    else:
                        ps = psqk.tile([P, 2, TOK], F32, tag="sA")
                        for h in range(2):
                            base = 64 * half + 32 * h
                            nc.tensor.matmul(
                                ps[:, h, :],
                                lhsT=k8sb[base : base + 32, g, :, kc * P : (kc + 1) * P],
                                rhs=q8sb[base : base + 32, g, :, :],
                                start=True,
                                stop=False,
                                perf_mode=DR,
                                tile_position=(base, 0),
                            )
                            nc.tensor.matmul(
                                ps[:, h, :],
                                lhsT=identg2,
                                rhs=b8sb[:, ACT_SLOT.get(kc, kc % len(ACT_KC_LIST)), :, :],
                                start=False,
                                stop=True,
                                perf_mode=DR,
                                skip_group_check=True,
                            )
                        nc.scalar.activation(
                            out=ptile[:, :, sub, :],
                            in_=ps,
                            func=AF.Exp,
                            bias=expb_ap,
                            scale=EXP_SCALE,
                        )
                nc.tensor.matmul(
                    pvA,
                    lhsT=v8sb[:, 2 * kcp : 2 * kcp + 2, 130 * j : 130 * j + 96],
                    rhs=ptile[:, 0, :, :],
                    start=(kcp == 0),
                    stop=(kcp == NKCP - 1),
                    perf_mode=DR,
                )
                nc.tensor.matmul(
                    pvB,
                    lhsT=v8sb[:, 2 * kcp : 2 * kcp + 2, 130 * j + 65 : 130 * j + 161],
                    rhs=ptile[:, 1, :, :],
                    start=(kcp == 0),
                    stop=(kcp == NKCP - 1),
                    perf_mode=DR,
                )

            if DIAG_SKIP_TAIL:
                nc.vector.memset(attnT8[:, j, :], 0.01)
                continue
            # fast evac: stage pv to SBUF (frees the psum banks), then
            # normalize off the critical path using the idle Pool engine
            stA = small.tile([64, TOK], BF16, tag="stA", bufs=2)
            stB = small.tile([64, TOK], BF16, tag="stB", bufs=2)
            nc.scalar.copy(stA, pvA[0:64, :])
            nc.scalar.copy(stB, pvB[0:64, :])
            sd = small.tile([1, 2, TOK], BF16, tag="sd", bufs=2)
            nc.scalar.copy(sd[:, 0, :], pvA[64:65, :])
            nc.vector.tensor_copy(sd[:, 1, :], pvB[64:65, :])
            rec = small.tile([1, 2, TOK], BF16, tag="rec", bufs=2)
            with nc.allow_low_precision("bf16 softmax denominators, ~0.4% scale"):
                nc.vector.reciprocal(rec, sd)
            bcastA = small.tile([64, TOK], BF16, tag="bcastA", bufs=2)
            bcastB = small.tile([64, TOK], BF16, tag="bcastB", bufs=2)
            nc.gpsimd.partition_broadcast(bcastA, rec[:, 0, :], channels=64)
            nc.gpsimd.partition_broadcast(bcastB, rec[:, 1, :], channels=64)
            nc.gpsimd.tensor_tensor(
                out=attnT8[0:64, j, :], in0=stA, in1=bcastA, op=ALU.mult
            )
            nc.gpsimd.tensor_tensor(
                out=attnT8[64:128, j, :], in0=stB, in1=bcastB, op=ALU.mult
            )

        # output projection (fp8 DR) + residual add (f32)
        o_sb = consts.tile([P, NTC, N_STATE], F32)
        for qc in range(NTC):
            ps_o = psqk.tile([P, 2, TOK], F32, tag="sA")
            po = ps_o.rearrange("p a b -> p (a b)")
            for pc in range(2):
                for u in range(4):
                    nc.tensor.matmul(
                        ps_o[:, pc, :],
                        lhsT=attnT8[:, 2 * u : 2 * u + 2, qc * P : (qc + 1) * P],
                        rhs=wc_sb[:, u, :, pc * TOK : (pc + 1) * TOK],
                        start=(u == 0),
                        stop=(u == 3),
                        perf_mode=DR,
                    )
            nc.vector.scalar_tensor_tensor(
                out=o_sb[:, qc, :],
                in0=po,
                scalar=G_OUT,
                in1=m_sb[:, qc, :],
                op0=ALU.mult,
                op1=ALU.add,
            )
        for qc in range(NTC):
            nc.sync.dma_start(
                out=o_out.rearrange("(c p) s -> p c s", p=P)[:, qc, :],
                in_=o_sb[:, qc, :],
            )
    nc.compile()
    return nc


_NC_CACHE = {}


def _get_nc(which):
    if which not in _NC_CACHE:
        _NC_CACHE[which] = _build_phase1() if which == 1 else _build_phase2()
    return _NC_CACHE[which]


def _perm_cols():
    """Column permutation for q/k weights: per pair j, [hA d0:32 | hB d0:32 |
    hA d32:64 | hB d32:64]."""
    order = []
    for j in range(NSC):
        hA, hB = 2 * j, 2 * j + 1
        order.extend(range(hA * 64, hA * 64 + 32))
        order.extend(range(hB * 64, hB * 64 + 32))
        order.extend(range(hA * 64 + 32, hA * 64 + 64))
        order.extend(range(hB * 64 + 32, hB * 64 + 64))
    return np.array(order)


def _w_dr_layout(w8):
    """[1024, C] -> [128, 4, 2, C] DoubleRow lhsT layout."""
    return np.ascontiguousarray(
        w8.reshape(4, 2, P, -1).transpose(2, 0, 1, 3)
    )


def kernel(m, bias, gamma, beta, Wq, bq, Wk, Wv, bv, Wc, bc, _want_timing=None):
    m = np.asarray(m, dtype=np.float32).reshape(N_CTX, N_STATE)
    bias = np.asarray(bias, np.float32)
    gamma = np.asarray(gamma, np.float32)
    beta = np.asarray(beta, np.float32)
    Wq = np.asarray(Wq, np.float32)
    Wk = np.asarray(Wk, np.float32)
    Wv = np.asarray(Wv, np.float32)
    Wc = np.asarray(Wc, np.float32)
    bq = np.asarray(bq, np.float32)
    bv = np.asarray(bv, np.float32)
    bc = np.asarray(bc, np.float32)

    m_pad = np.zeros((T_PAD, N_STATE), np.float32)
    m_pad[:N_CTX] = m

    # fold gamma into weights, beta into biases; bv and bc fold into residual
    Wqf = gamma[:, None] * Wq
    Wkf = gamma[:, None] * Wk
    Wvf = gamma[:, None] * Wv
    bqf = bq + beta @ Wq
    # beta@Wk shifts all logits of a query equally -> softmax invariant; drop.
    # beta@Wv + bv shift attention output -> fold into residual with bc.
    perm = _perm_cols()
    Wq8 = _w_dr_layout((LAM_W * Wqf[:, perm]).astype(E4NP))
    Wk8 = _w_dr_layout((LAM_W * Wkf[:, perm]).astype(E4NP))
    Wv8 = _w_dr_layout((LAM_W * Wvf).astype(E4NP))
    bqs = (LAM_Q * bqf[perm]).astype(np.float32)
    Wc8 = _w_dr_layout((LAM_WC * Wc).astype(E4NP))
    mres_full = m_pad + (bc + (bv + beta @ Wv) @ Wc)[None, :]

    import sys as _sys

    def _log(*a):
        print("[kernel]", *a, file=_sys.stderr, flush=True)

    nc1 = _get_nc(1)
    _log("phase1 built")
    in_maps1 = []
    for c in range(N_CORES):
        in_maps1.append(
            {
                "m_blk": np.ascontiguousarray(m_pad[c * TOK : (c + 1) * TOK]),
                "Wq8": Wq8,
                "Wk8": Wk8,
                "Wv8": Wv8,
                "bqs": bqs,
            }
        )
    res1 = run_bass_kernel_spmd(nc1, in_maps1, core_ids=list(range(N_CORES)))
    _log("phase1 done")

    q8_blks = [r["q8_out"] for r in res1.results]
    k8_full = np.concatenate([r["k8_out"] for r in res1.results], axis=3)
    v8_full = np.concatenate([r["v8_out"] for r in res1.results], axis=0)
    v8_full[N_CTX:] = 0  # pad tokens carry no value

    # v8 pair-tile layout [128, 32, VW] with denominator columns
    v8f = v8_full.astype(np.float32).reshape(NKC, P, N_HEADS, D_HEAD)
    v8h = np.zeros((P, NKC, VW), np.float32)
    for j in range(NPAIR):
        v8h[:, :, 130 * j : 130 * j + 64] = v8f[:, :, 2 * j].transpose(1, 0, 2)
        v8h[:, :, 130 * j + 65 : 130 * j + 129] = v8f[:, :, 2 * j + 1].transpose(1, 0, 2)
        v8h[:, :, 130 * j + 64] = ONES_VAL
        v8h[:, :, 130 * j + 129] = ONES_VAL
    # zero the denominator contribution of padded keys
    keyidx = (np.arange(NKC)[None, :] * P + np.arange(P)[:, None])  # [p, kc]
    padmask = keyidx >= N_CTX
    for j in range(NPAIR):
        v8h[:, :, 130 * j + 64][padmask] = 0.0
        v8h[:, :, 130 * j + 129][padmask] = 0.0
    v8h8 = v8h.astype(E4NP)
    v8_dr = np.ascontiguousarray(
        v8h8.reshape(P, 4, NKC // 4, VW).transpose(1, 0, 2, 3)
    )

    biasT = np.ascontiguousarray(bias.T)  # [k, q]

    nc2 = _get_nc(2)
    _log("phase2 built")
    in_maps2 = []
    for c in range(N_CORES):
        qs = slice(c * TOK, (c + 1) * TOK)
        b8 = np.zeros((len(ACT_KC_LIST), P, 2, TOK), E4NP)
        for i, kc in enumerate(ACT_KC_LIST):
            chunk = (LAM_B * biasT[kc * P : (kc + 1) * P, qs]).astype(E4NP)
            b8[i, :, 0, :] = chunk
            b8[i, :, 1, :] = chunk
        bt = np.zeros((len(DVE_KC_LIST), P, TOK), BFNP)
        for i, kc in enumerate(DVE_KC_LIST):
            bt[i] = (
                ALPHA * biasT[kc * P : (kc + 1) * P, qs] + (BETA - ALPHA * C_SHIFT)
            ).astype(BFNP)
        in_maps2.append(
            {
                "q8_in": np.ascontiguousarray(q8_blks[c]),
                "k8_in": k8_full,
                "v8_in": v8_dr,
                "b8_in": b8,
                "bt_in": bt,
                "mres": np.ascontiguousarray(mres_full[qs]),
                "Wc8": Wc8,
            }
        )
    res2 = run_bass_kernel_spmd(nc2, in_maps2, core_ids=list(range(N_CORES)))
    _log("phase2 done")
    o = np.concatenate([r["o_out"] for r in res2.results], axis=0)[:N_CTX]
    if _want_timing is not None:
        _want_timing["res1"] = res1
        _want_timing["res2"] = res2
    return o.reshape(1, N_CTX, N_STATE).astype(np.float32)


# revision 19
# speedup vs baseline: 1.7434x; 1.0260x over previous
"""AttentionResblock on 8 NeuronCores (Trainium2, Bass/Tile) — fp8 edition.

Sharding: query-token blocks of 512 (T_PAD=4096 = 8 x 512), two launches:
  Phase 1 (per core c): LayerNorm + Q/K/V projections (fp8 DoubleRow matmuls)
    for token rows [512c, 512c+512). Emits q8/k8 in DoubleRow-ready
    [128, 2, 512] head-pair tiles and v8 token-major, all fp8-e4m3.
    gamma/beta/bv/bc are folded into weights/residual on the host.
  Phase 2 (per core c): 16-head attention for its 512 query rows over all
    4096 keys. QK via fp8 DoubleRow (2x32 contraction). Softmax weights are
    produced two ways, split across engines to balance the timeline:
      - ACT chunks: bias added in PSUM via fp8 ident-matmul, then true
        exp -> fp8-e4m3 (premultiplied by 512, shifted by C=9).
      - DVE chunks: fastexp bit trick - y = int8(alpha*s + [alpha*b +
        beta - alpha*C]) bitcast as fp8-e5m2 (2^(y/4-15) ~ 512*e^(s+b-9)).
    The scale/shift cancels in softmax: PV accumulates numerator and
    denominator (ones columns in the fp8 V tiles, value 0.5 = LAM_V/LAM_ATTN)
    with fp8 DoubleRow over key-chunk pairs. Normalize, fp8 DoubleRow output
    projection, f32 residual add.

Numerics validated against the reference in numpy (numerics2.py): rel err
~1.5e-4 vs the 2e-2 gate, dominated by fp8 quantization of the attention
branch, which is scaled by ~1e-3 through Wc so the f32 residual dominates.
"""

import sys

sys.path.insert(0, "/opt/trn_rl_repo")

from contextlib import ExitStack  # noqa: E402

import numpy as np  # noqa: E402
import ml_dtypes  # noqa: E402

import concourse.bass as bass  # noqa: E402
import concourse.bacc as bacc  # noqa: E402
import concourse.tile as tile  # noqa: E402
from concourse import mybir  # noqa: E402
from concourse.bass_utils import run_bass_kernel_spmd  # noqa: E402
from concourse.masks import make_identity  # noqa: E402

F32 = mybir.dt.float32
BF16 = mybir.dt.bfloat16
F8E4 = mybir.dt.float8e4
F8E5 = mybir.dt.float8e5
I8 = mybir.dt.int8
AF = mybir.ActivationFunctionType
ALU = mybir.AluOpType
DR = mybir.MatmulPerfMode.DoubleRow

E4NP = ml_dtypes.float8_e4m3
E5NP = ml_dtypes.float8_e5m2
BFNP = ml_dtypes.bfloat16

N_STATE = 1024
N_HEADS = 16
D_HEAD = 64
N_CTX = 4080
T_PAD = 4096
N_CORES = 8
TOK = 512
P = 128
LN_EPS = 1e-5
NSC = 8  # state chunks of 128
NTC = 4  # token chunks per core
NKC = 32  # key chunks of 128
NKCP = 16  # key-chunk pairs of 256
NPAIR = 8  # head pairs

# fp8 scale plan (see numerics2.py)
ALPHA = 4 * np.log2(np.e)  # logit scale in PSUM: psum = ALPHA*(s)
C_SHIFT = 9.0  # global logit shift (measured max 6.21)
PMULT = 512.0  # weights premultiplier (cancels in softmax)
BETA = 96.0  # 60 + 4*log2(PMULT)
LAM_R = 16.0  # LN output scale
LAM_W = 512.0  # Wq/Wk/Wv scale
LAM_Q = float(np.sqrt(ALPHA / 8.0))  # q/k scales; 8*LAM_Q*LAM_K = ALPHA
LAM_V = 16.0
LAM_B = 369.0  # ACT-path bias quant scale; ident diag g = 2^-6, g*LAM_B ~ ALPHA
G_IDENT = 2.0 ** -6
LAM_ATTN = 32.0
LAM_WC = 32768.0
ONES_VAL = LAM_V / LAM_ATTN  # 0.5, folded into denominator columns
GQ = LAM_Q / (LAM_R * LAM_W)
GV = LAM_V / (LAM_R * LAM_W)
G_OUT = 1.0 / (LAM_ATTN * LAM_WC)
EXP_BIAS = float(np.log(PMULT) - C_SHIFT)  # -2.7616
EXP_SCALE = float(1.0 / ALPHA)
STT_CLAMP = -40.0

# kc-pair -> engine assignment (per head-pair parity), tuned for balance
DVE_EVEN = (3, 5, 7, 9, 11, 13, 15)
DVE_ODD = (3, 5, 7, 9, 11, 13, 15)
ACT_KC_LIST = [kc for kcp in range(NKCP) if kcp not in DVE_EVEN
               for kc in (2 * kcp, 2 * kcp + 1)]  # 20 kcs ever handled by ACT
DVE_KC_LIST = [kc for kcp in DVE_ODD for kc in (2 * kcp, 2 * kcp + 1)]  # 14 kcs
ACT_SLOT = {kc: i for i, kc in enumerate(ACT_KC_LIST)}
DVE_SLOT = {kc: i for i, kc in enumerate(DVE_KC_LIST)}
VW = NPAIR * 130 + 32  # v8 tile width: per-pair 130 cols + tail padding
PSQK_BUFS = 2
PSPV_BUFS = 1
P_BUFS = 3
DIAG_SKIP_TAIL = False
DIAG_FORCE = None  # None | "ACT" | "DVE"


def _build_phase1() -> bass.Bass:
    nc = bacc.Bacc("TRN2", target_bir_lowering=False, debug=False, num_devices=N_CORES)
    m_blk = nc.dram_tensor("m_blk", [TOK, N_STATE], F32, kind="ExternalInput")
    Wq8 = nc.dram_tensor("Wq8", [P, 4, 2, N_STATE], F8E4, kind="ExternalInput")
    Wk8 = nc.dram_tensor("Wk8", [P, 4, 2, N_STATE], F8E4, kind="ExternalInput")
    Wv8 = nc.dram_tensor("Wv8", [P, 4, 2, N_STATE], F8E4, kind="ExternalInput")
    bqs = nc.dram_tensor("bqs", [N_STATE], F32, kind="ExternalInput")
    q8_out = nc.dram_tensor("q8_out", [4, P, 2, TOK], F8E4, kind="ExternalOutput")
    k8_out = nc.dram_tensor("k8_out", [4, P, 2, TOK], F8E4, kind="ExternalOutput")
    v8_out = nc.dram_tensor("v8_out", [TOK, N_STATE], F8E4, kind="ExternalOutput")

    with ExitStack() as ctx:
        tc = ctx.enter_context(tile.TileContext(nc))
        consts = ctx.enter_context(tc.tile_pool(name="consts", bufs=1))
        small = ctx.enter_context(tc.tile_pool(name="small", bufs=4))
        work = ctx.enter_context(tc.tile_pool(name="work", bufs=2))
        psum = ctx.enter_context(tc.tile_pool(name="psum", bufs=2, space="PSUM"))
        pst_pool = ctx.enter_context(tc.tile_pool(name="pst", bufs=2, space="PSUM"))

        identB = consts.tile([P, P], BF16)
        make_identity(nc, identB)
        eps_sb = consts.tile([P, 1], F32)
        nc.vector.memset(eps_sb, LN_EPS)
        bqs_sb = consts.tile([P, NSC], F32)
        nc.sync.dma_start(out=bqs_sb, in_=bqs.rearrange("(j p) -> p j", p=P))

        m_sb = consts.tile([P, NTC, N_STATE], F32)
        w_sb = {}
        for name, w in (("Wq8", Wq8), ("Wk8", Wk8), ("Wv8", Wv8)):
            w_sb[name] = consts.tile([P, 4, 2, N_STATE], F8E4, name=f"{name}_sb")

        def ld_m(tcn):
            nc.sync.dma_start(
                out=m_sb[:, tcn, :],
                in_=m_blk.rearrange("(c p) s -> p c s", p=P)[:, tcn, :],
            )

        ld_m(0)
        ld_m(1)
        ld_m(2)
        ld_m(3)
        nc.sync.dma_start(out=w_sb["Wq8"], in_=Wq8[:, :, :, :])
        nc.sync.dma_start(out=w_sb["Wk8"], in_=Wk8[:, :, :, :])
        nc.sync.dma_start(out=w_sb["Wv8"], in_=Wv8[:, :, :, :])

        # LayerNorm -> xcB = (m - mu) * rstd * LAM_R in bf16
        xcB = consts.tile([P, NTC, N_STATE], BF16)
        for tcn in range(NTC):
            ssum = small.tile([P, 1], F32, tag="ssum")
            nc.vector.reduce_sum(ssum, m_sb[:, tcn, :], axis=mybir.AxisListType.X)
            negmean = small.tile([P, 1], F32, tag="negmean")
            nc.scalar.mul(negmean, ssum, -1.0 / N_STATE)
            sqscr = work.tile([P, N_STATE], BF16, tag="sqscr")
            sqsum = small.tile([P, 1], F32, tag="sqsum")
            nc.scalar.activation(
                out=sqscr, in_=m_sb[:, tcn, :], func=AF.Square, accum_out=sqsum
            )
            # 1024*var = sqsum - ssum^2/1024
            musq = small.tile([P, 1], F32, tag="musq")
            nc.vector.scalar_tensor_tensor(
                out=musq, in0=ssum, scalar=-1.0 / N_STATE, in1=ssum,
                op0=ALU.mult, op1=ALU.mult,
            )
            nvar = small.tile([P, 1], F32, tag="nvar")
            nc.vector.tensor_tensor(out=nvar, in0=sqsum, in1=musq, op=ALU.add)
            std = small.tile([P, 1], F32, tag="std")
            nc.scalar.activation(
                out=std, in_=nvar, func=AF.Sqrt, bias=eps_sb, scale=1.0 / N_STATE
            )
            rstd = small.tile([P, 1], F32, tag="rstd")
            nc.vector.reciprocal(rstd, std)
            rstdl = small.tile([P, 1], F32, tag="rstdl")
            nc.scalar.mul(rstdl, rstd, LAM_R)
            nc.gpsimd.tensor_scalar(
                out=xcB[:, tcn, :],
                in0=m_sb[:, tcn, :],
                scalar1=negmean,
                scalar2=rstdl,
                op0=ALU.add,
                op1=ALU.mult,
            )

        # transpose to state-major and quantize: rT8 [128, sc, 512] e4m3
        rT8 = consts.tile([P, NSC, TOK], F8E4)
        for sc in range(NSC):
            pst = pst_pool.tile([P, TOK], BF16, tag="pst")
            for tcn in range(NTC):
                nc.tensor.transpose(
                    pst[:, tcn * P : (tcn + 1) * P],
                    xcB[:, tcn, sc * P : (sc + 1) * P],
                    identB,
                )
            if sc % 2 == 0:
                nc.vector.tensor_copy(rT8[:, sc, :], pst)
            else:
                nc.scalar.copy(rT8[:, sc, :], pst)

        # q/k: DoubleRow fp8 matmuls, evacuate into [128, 2, 512] pair tiles
        q8g = [consts.tile([P, 2, TOK], F8E4, name=f"q8g{g}") for g in range(4)]
        k8g = [consts.tile([P, 2, TOK], F8E4, name=f"k8g{g}") for g in range(4)]
        for j in range(NSC):
            g, half = j // 2, j % 2
            psq = psum.tile([P, TOK], F32, tag="psq")
            psk = psum.tile([P, TOK], F32, tag="psk")
            for s in range(4):
                nc.tensor.matmul(
                    psq,
                    lhsT=w_sb["Wq8"][:, s, :, j * P : (j + 1) * P],
                    rhs=rT8[:, 2 * s : 2 * s + 2, :],
                    start=(s == 0),
                    stop=(s == 3),
                    perf_mode=DR,
                )
            for s in range(4):
                nc.tensor.matmul(
                    psk,
                    lhsT=w_sb["Wk8"][:, s, :, j * P : (j + 1) * P],
                    rhs=rT8[:, 2 * s : 2 * s + 2, :],
                    start=(s == 0),
                    stop=(s == 3),
                    perf_mode=DR,
                )
            for t in range(2):
                nc.scalar.activation(
                    out=q8g[g][64 * half : 64 * half + 64, t, :],
                    in_=psq[64 * t : 64 * t + 64, :],
                    func=AF.Identity,
                    bias=bqs_sb[64 * t : 64 * t + 64, j : j + 1],
                    scale=GQ,
                )
                nc.vector.tensor_scalar(
                    out=k8g[g][64 * half : 64 * half + 64, t, :],
                    in0=psk[64 * t : 64 * t + 64, :],
                    scalar1=GQ,
                    scalar2=None,
                    op0=ALU.mult,
                )

        # v: token-major, e4m3
        v8sb = consts.tile([P, NTC, N_STATE], F8E4)
        for tcn in range(NTC):
            psv = psum.tile([P, N_STATE], F32, tag="psv", bufs=1)
            for pc in range(2):
                for s in range(4):
                    nc.tensor.matmul(
                        psv[:, pc * TOK : (pc + 1) * TOK],
                        lhsT=rT8[:, 2 * s : 2 * s + 2, tcn * P : (tcn + 1) * P],
                        rhs=w_sb["Wv8"][:, s, :, pc * TOK : (pc + 1) * TOK],
                        start=(s == 0),
                        stop=(s == 3),
                        perf_mode=DR,
                    )
            nc.scalar.mul(v8sb[:, tcn, :], psv, GV)

        for g in range(4):
            nc.sync.dma_start(out=q8_out[g, :, :, :], in_=q8g[g])
            nc.sync.dma_start(out=k8_out[g, :, :, :], in_=k8g[g])
        for tcn in range(NTC):
            nc.sync.dma_start(
                out=v8_out.rearrange("(c p) s -> p c s", p=P)[:, tcn, :],
                in_=v8sb[:, tcn, :],
            )
    nc.compile()
    return nc


def _build_phase2() -> bass.Bass:
    nc = bacc.Bacc("TRN2", target_bir_lowering=False, debug=False, num_devices=N_CORES)
    q8_in = nc.dram_tensor("q8_in", [4, P, 2, TOK], F8E4, kind="ExternalInput")
    k8_in = nc.dram_tensor("k8_in", [4, P, 2, T_PAD], F8E4, kind="ExternalInput")
    v8_in = nc.dram_tensor("v8_in", [4, P, NKC // 4, VW], F8E4, kind="ExternalInput")
    b8_in = nc.dram_tensor("b8_in", [len(ACT_KC_LIST), P, 2, TOK], F8E4, kind="ExternalInput")
    bt_in = nc.dram_tensor("bt_in", [len(DVE_KC_LIST), P, TOK], BF16, kind="ExternalInput")
    mres = nc.dram_tensor("mres", [TOK, N_STATE], F32, kind="ExternalInput")
    Wc8 = nc.dram_tensor("Wc8", [P, 4, 2, N_STATE], F8E4, kind="ExternalInput")
    o_out = nc.dram_tensor("o_out", [TOK, N_STATE], BF16, kind="ExternalOutput")

    with ExitStack() as ctx:
        tc = ctx.enter_context(tile.TileContext(nc))
        consts = ctx.enter_context(tc.tile_pool(name="consts", bufs=1))
        small = ctx.enter_context(tc.tile_pool(name="small", bufs=4))
        ppool = ctx.enter_context(tc.tile_pool(name="ppool", bufs=P_BUFS))
        psqk = ctx.enter_context(tc.tile_pool(name="psqk", bufs=PSQK_BUFS, space="PSUM"))
        pspv = ctx.enter_context(tc.tile_pool(name="pspv", bufs=PSPV_BUFS, space="PSUM"))

        identg = consts.tile([P, 2, P], F8E4)
        nc.vector.memset(identg, 0.0)
        make_identity(nc, identg[:, 0, :])
        make_identity(nc, identg[:, 1, :])
        identg2 = consts.tile([P, 2, P], F8E4)
        nc.scalar.mul(identg2, identg, G_IDENT / 2.0)
        expb_ap = consts.tile([P, 1], F32)
        nc.vector.memset(expb_ap, EXP_BIAS)

        q8sb = consts.tile([P, 4, 2, TOK], F8E4)
        k8sb = consts.tile([P, 4, 2, T_PAD], F8E4)
        v8sb = consts.tile([P, NKC, VW], F8E4)
        b8sb = consts.tile([P, len(ACT_KC_LIST), 2, TOK], F8E4)
        btsb = consts.tile([P, len(DVE_KC_LIST), TOK], BF16)
        m_sb = consts.tile([P, NTC, N_STATE], F32)
        wc_sb = consts.tile([P, 4, 2, N_STATE], F8E4)
        nA, nD = len(ACT_KC_LIST), len(DVE_KC_LIST)
        bA = [0, 5, 10, 15, nA]
        bD = [0, 4, 8, 11, nD]

        def ld_q8(g):
            nc.sync.dma_start(out=q8sb[:, g, :, :], in_=q8_in[g, :, :, :])

        def ld_k8(g):
            nc.sync.dma_start(out=k8sb[:, g, :, :], in_=k8_in[g, :, :, :])

        def ld_b8(i):
            nc.sync.dma_start(
                out=b8sb[:, bA[i] : bA[i + 1], :, :],
                in_=b8_in[bA[i] : bA[i + 1], :, :, :].rearrange("k p t n -> p k t n"),
            )

        def ld_bt(i):
            nc.sync.dma_start(
                out=btsb[:, bD[i] : bD[i + 1], :],
                in_=bt_in[bD[i] : bD[i + 1], :, :].rearrange("k p n -> p k n"),
            )

        def ld_v8(q):
            nc.sync.dma_start(
                out=v8sb[:, q * 8 : (q + 1) * 8, :], in_=v8_in[q, :, :, :]
            )

        ld_q8(0)
        ld_k8(0)
        ld_b8(0)
        ld_bt(0)
        ld_v8(0)
        ld_b8(1)
        ld_bt(1)
        ld_v8(1)
        ld_b8(2)
        ld_bt(2)
        ld_v8(2)
        ld_b8(3)
        ld_bt(3)
        ld_v8(3)
        ld_k8(1)
        ld_q8(1)
        ld_k8(2)
        ld_q8(2)
        ld_k8(3)
        ld_q8(3)
        nc.sync.dma_start(out=m_sb, in_=mres.rearrange("(c p) s -> p c s", p=P))
        nc.sync.dma_start(out=wc_sb, in_=Wc8[:, :, :, :])

        attnT8 = consts.tile([P, NPAIR, TOK], F8E4)

        for j in range(NPAIR):
            g, half = j // 2, j % 2
            dve_set = DVE_ODD if (j % 2) else DVE_EVEN
            if DIAG_FORCE == "ACT":
                dve_set = ()
            elif DIAG_FORCE == "DVE":
                dve_set = tuple(range(NKCP))
            pvA = pspv.tile([96, TOK], F32, tag="pvA")
            pvB = pspv.tile([96, TOK], F32, tag="pvB")
            for kcp in range(NKCP):
                is_dve = kcp in dve_set
                if is_dve:
                    ptile = ppool.tile([P, 2, 2, TOK], F8E5, tag="pD")
                else:
                    ptile = ppool.tile([P, 2, 2, TOK], F8E4, tag="pA")
                for sub in range(2):
                    kc = 2 * kcp + sub
                    if is_dve:
                        # DVE stream: per-head 1-bank psums, independent of
                        # the ACT stream so both engines pipeline in parallel
                        for h in range(2):
                            base = 64 * half + 32 * h
                            psd = psqk.tile([P, TOK], F32, tag="sD")
                            nc.tensor.matmul(
                                psd,
                                lhsT=k8sb[base : base + 32, g, :, kc * P : (kc + 1) * P],
                                rhs=q8sb[base : base + 32, g, :, :],
                                start=True,
                                stop=True,
                                perf_mode=DR,
                                tile_position=(base, 0),
                            )
                            nc.vector.scalar_tensor_tensor(
                                out=ptile[:, h, sub, :].bitcast(I8),
                                in0=psd,
                                scalar=STT_CLAMP,
                                in1=btsb[:, DVE_SLOT.get(kc, kc % len(DVE_KC_LIST)), :],
                                op0=ALU.max,
                                op1=ALU.add,
                            )
                    else:
                        ps = psqk.tile([P, 2, TOK], F32, tag="sA")
                        for h in range(2):
                            base = 64 * half + 32 * h
                            nc.tensor.matmul(
                                ps[:, h, :],
                                lhsT=k8sb[base : base + 32, g, :, kc * P : (kc + 1) * P],
                                rhs=q8sb[base : base + 32, g, :, :],
                                start=True,
                                stop=False,
                                perf_mode=DR,
                                tile_position=(base, 0),
                            )
                            nc.tensor.matmul(
                                ps[:, h, :],
                                lhsT=identg2,
                                rhs=b8sb[:, ACT_SLOT.get(kc, kc % len(ACT_KC_LIST)), :, :],
                                start=False,
                                stop=True,
                                perf_mode=DR,
                                skip_group_check=True,
                            )
                        nc.scalar.activation(
                            out=ptile[:, :, sub, :],
                            in_=ps,
                            func=AF.Exp,
                            bias=expb_ap,
                            scale=EXP_SCALE,
                        )
                nc.tensor.matmul(
                    pvA,
                    lhsT=v8sb[:, 2 * kcp : 2 * kcp + 2, 130 * j : 130 * j + 96],
                    rhs=ptile[:, 0, :, :],
                    start=(kcp == 0),
                    stop=(kcp == NKCP - 1),
                    perf_mode=DR,
                )
                nc.tensor.matmul(
                    pvB,
                    lhsT=v8sb[:, 2 * kcp : 2 * kcp + 2, 130 * j + 65 : 130 * j + 161],
                    rhs=ptile[:, 1, :, :],
                    start=(kcp == 0),
                    stop=(kcp == NKCP - 1),
                    perf_mode=DR,
                )

            if DIAG_SKIP_TAIL:
                nc.vector.memset(attnT8[:, j, :], 0.01)
                continue
            # fast evac: stage pv to SBUF (frees the psum banks), then
            # normalize off the critical path using the idle Pool engine
            stA = small.tile([65, TOK], BF16, tag="stA", bufs=2)
            stB = small.tile([65, TOK], BF16, tag="stB", bufs=2)
            nc.scalar.copy(stA, pvA[0:65, :])
            nc.vector.tensor_copy(stB, pvB[0:65, :])
            recA = small.tile([1, TOK], BF16, tag="recA", bufs=2)
            recB = small.tile([1, TOK], BF16, tag="recB", bufs=2)
            with nc.allow_low_precision("bf16 softmax denominators, ~0.4% scale"):
                nc.vector.reciprocal(recA, stA[64:65, :])
                nc.vector.reciprocal(recB, stB[64:65, :])
            bcastA = small.tile([64, TOK], BF16, tag="bcastA", bufs=2)
            bcastB = small.tile([64, TOK], BF16, tag="bcastB", bufs=2)
            nc.gpsimd.partition_broadcast(bcastA, recA, channels=64)
            nc.gpsimd.partition_broadcast(bcastB, recB, channels=64)
            nc.gpsimd.tensor_tensor(
                out=attnT8[0:64, j, :], in0=stA[0:64, :], in1=bcastA, op=ALU.mult
            )
            nc.gpsimd.tensor_tensor(
                out=attnT8[64:128, j, :], in0=stB[0:64, :], in1=bcastB, op=ALU.mult
            )

        # output projection (fp8 DR) + residual add (f32)
        o_sb = consts.tile([P, NTC, N_STATE], BF16)
        for qc in range(NTC):
            ps_o = psqk.tile([P, 2, TOK], F32, tag="sA")
            po = ps_o.rearrange("p a b -> p (a b)")
            for pc in range(2):
                for u in range(4):
                    nc.tensor.matmul(
                        ps_o[:, pc, :],
                        lhsT=attnT8[:, 2 * u : 2 * u + 2, qc * P : (qc + 1) * P],
                        rhs=wc_sb[:, u, :, pc * TOK : (pc + 1) * TOK],
                        start=(u == 0),
                        stop=(u == 3),
                        perf_mode=DR,
                    )
            nc.vector.scalar_tensor_tensor(
                out=o_sb[:, qc, :],
                in0=po,
                scalar=G_OUT,
                in1=m_sb[:, qc, :],
                op0=ALU.mult,
                op1=ALU.add,
            )
        for qc in range(NTC):
            nc.sync.dma_start(
                out=o_out.rearrange("(c p) s -> p c s", p=P)[:, qc, :],
                in_=o_sb[:, qc, :],
            )
    nc.compile()
    return nc


_NC_CACHE = {}


def _get_nc(which):
    if which not in _NC_CACHE:
        _NC_CACHE[which] = _build_phase1() if which == 1 else _build_phase2()
    return _NC_CACHE[which]


def _perm_cols():
    """Column permutation for q/k weights: per pair j, [hA d0:32 | hB d0:32 |
    hA d32:64 | hB d32:64]."""
    order = []
    for j in range(NSC):
        hA, hB = 2 * j, 2 * j + 1
        order.extend(range(hA * 64, hA * 64 + 32))
        order.extend(range(hB * 64, hB * 64 + 32))
        order.extend(range(hA * 64 + 32, hA * 64 + 64))
        order.extend(range(hB * 64 + 32, hB * 64 + 64))
    return np.array(order)


def _w_dr_layout(w8):
    """[1024, C] -> [128, 4, 2, C] DoubleRow lhsT layout."""
    return np.ascontiguousarray(
        w8.reshape(4, 2, P, -1).transpose(2, 0, 1, 3)
    )


def kernel(m, bias, gamma, beta, Wq, bq, Wk, Wv, bv, Wc, bc, _want_timing=None):
    m = np.asarray(m, dtype=np.float32).reshape(N_CTX, N_STATE)
    bias = np.asarray(bias, np.float32)
    gamma = np.asarray(gamma, np.float32)
    beta = np.asarray(beta, np.float32)
    Wq = np.asarray(Wq, np.float32)
    Wk = np.asarray(Wk, np.float32)
    Wv = np.asarray(Wv, np.float32)
    Wc = np.asarray(Wc, np.float32)
    bq = np.asarray(bq, np.float32)
    bv = np.asarray(bv, np.float32)
    bc = np.asarray(bc, np.float32)

    m_pad = np.zeros((T_PAD, N_STATE), np.float32)
    m_pad[:N_CTX] = m

    # fold gamma into weights, beta into biases; bv and bc fold into residual
    Wqf = gamma[:, None] * Wq
    Wkf = gamma[:, None] * Wk
    Wvf = gamma[:, None] * Wv
    bqf = bq + beta @ Wq
    # beta@Wk shifts all logits of a query equally -> softmax invariant; drop.
    # beta@Wv + bv shift attention output -> fold into residual with bc.
    perm = _perm_cols()
    Wq8 = _w_dr_layout((LAM_W * Wqf[:, perm]).astype(E4NP))
    Wk8 = _w_dr_layout((LAM_W * Wkf[:, perm]).astype(E4NP))
    Wv8 = _w_dr_layout((LAM_W * Wvf).astype(E4NP))
    bqs = (LAM_Q * bqf[perm]).astype(np.float32)
    Wc8 = _w_dr_layout((LAM_WC * Wc).astype(E4NP))
    mres_full = m_pad + (bc + (bv + beta @ Wv) @ Wc)[None, :]

    import sys as _sys

    def _log(*a):
        print("[kernel]", *a, file=_sys.stderr, flush=True)

    nc1 = _get_nc(1)
    _log("phase1 built")
    in_maps1 = []
    for c in range(N_CORES):
        in_maps1.append(
            {
                "m_blk": np.ascontiguousarray(m_pad[c * TOK : (c + 1) * TOK]),
                "Wq8": Wq8,
                "Wk8": Wk8,
                "Wv8": Wv8,
                "bqs": bqs,
            }
        )
    res1 = run_bass_kernel_spmd(nc1, in_maps1, core_ids=list(range(N_CORES)))
    _log("phase1 done")

    q8_blks = [r["q8_out"] for r in res1.results]
    k8_full = np.concatenate([r["k8_out"] for r in res1.results], axis=3)
    v8_full = np.concatenate([r["v8_out"] for r in res1.results], axis=0)
    v8_full[N_CTX:] = 0  # pad tokens carry no value

    # v8 pair-tile layout [128, 32, VW] with denominator columns
    v8f = v8_full.astype(np.float32).reshape(NKC, P, N_HEADS, D_HEAD)
    v8h = np.zeros((P, NKC, VW), np.float32)
    for j in range(NPAIR):
        v8h[:, :, 130 * j : 130 * j + 64] = v8f[:, :, 2 * j].transpose(1, 0, 2)
        v8h[:, :, 130 * j + 65 : 130 * j + 129] = v8f[:, :, 2 * j + 1].transpose(1, 0, 2)
        v8h[:, :, 130 * j + 64] = ONES_VAL
        v8h[:, :, 130 * j + 129] = ONES_VAL
    # zero the denominator contribution of padded keys
    keyidx = (np.arange(NKC)[None, :] * P + np.arange(P)[:, None])  # [p, kc]
    padmask = keyidx >= N_CTX
    for j in range(NPAIR):
        v8h[:, :, 130 * j + 64][padmask] = 0.0
        v8h[:, :, 130 * j + 129][padmask] = 0.0
    v8h8 = v8h.astype(E4NP)
    v8_dr = np.ascontiguousarray(
        v8h8.reshape(P, 4, NKC // 4, VW).transpose(1, 0, 2, 3)
    )

    biasT = np.ascontiguousarray(bias.T)  # [k, q]

    nc2 = _get_nc(2)
    _log("phase2 built")
    in_maps2 = []
    for c in range(N_CORES):
        qs = slice(c * TOK, (c + 1) * TOK)
        b8 = np.zeros((len(ACT_KC_LIST), P, 2, TOK), E4NP)
        for i, kc in enumerate(ACT_KC_LIST):
            chunk = (LAM_B * biasT[kc * P : (kc + 1) * P, qs]).astype(E4NP)
            b8[i, :, 0, :] = chunk
            b8[i, :, 1, :] = chunk
        bt = np.zeros((len(DVE_KC_LIST), P, TOK), BFNP)
        for i, kc in enumerate(DVE_KC_LIST):
            bt[i] = (
                ALPHA * biasT[kc * P : (kc + 1) * P, qs] + (BETA - ALPHA * C_SHIFT)
            ).astype(BFNP)
        in_maps2.append(
            {
                "q8_in": np.ascontiguousarray(q8_blks[c]),
                "k8_in": k8_full,
                "v8_in": v8_dr,
                "b8_in": b8,
                "bt_in": bt,
                "mres": np.ascontiguousarray(mres_full[qs]),
                "Wc8": Wc8,
            }
        )
    res2 = run_bass_kernel_spmd(nc2, in_maps2, core_ids=list(range(N_CORES)))
    _log("phase2 done")
    o = np.concatenate([r["o_out"] for r in res2.results], axis=0)[:N_CTX]
    if _want_timing is not None:
        _want_timing["res1"] = res1
        _want_timing["res2"] = res2
    return o.reshape(1, N_CTX, N_STATE).astype(np.float32)


# revision 22
# speedup vs baseline: 1.7543x; 1.0063x over previous
"""AttentionResblock on 8 NeuronCores (Trainium2, Bass/Tile) — fp8 edition.

Sharding: query-token blocks of 512 (T_PAD=4096 = 8 x 512), two launches:
  Phase 1 (per core c): LayerNorm + Q/K/V projections (fp8 DoubleRow matmuls)
    for token rows [512c, 512c+512). Emits q8/k8 in DoubleRow-ready
    [128, 2, 512] head-pair tiles and v8 token-major, all fp8-e4m3.
    gamma/beta/bv/bc are folded into weights/residual on the host.
  Phase 2 (per core c): 16-head attention for its 512 query rows over all
    4096 keys. QK via fp8 DoubleRow (2x32 contraction). Softmax weights are
    produced two ways, split across engines to balance the timeline:
      - ACT chunks: bias added in PSUM via fp8 ident-matmul, then true
        exp -> fp8-e4m3 (premultiplied by 512, shifted by C=9).
      - DVE chunks: fastexp bit trick - y = int8(alpha*s + [alpha*b +
        beta - alpha*C]) bitcast as fp8-e5m2 (2^(y/4-15) ~ 512*e^(s+b-9)).
    The scale/shift cancels in softmax: PV accumulates numerator and
    denominator (ones columns in the fp8 V tiles, value 0.5 = LAM_V/LAM_ATTN)
    with fp8 DoubleRow over key-chunk pairs. Normalize, fp8 DoubleRow output
    projection, f32 residual add.

Numerics validated against the reference in numpy (numerics2.py): rel err
~1.5e-4 vs the 2e-2 gate, dominated by fp8 quantization of the attention
branch, which is scaled by ~1e-3 through Wc so the f32 residual dominates.
"""

import sys

sys.path.insert(0, "/opt/trn_rl_repo")

from contextlib import ExitStack  # noqa: E402

import numpy as np  # noqa: E402
import ml_dtypes  # noqa: E402

import concourse.bass as bass  # noqa: E402
import concourse.bacc as bacc  # noqa: E402
import concourse.tile as tile  # noqa: E402
from concourse import mybir  # noqa: E402
from concourse.bass_utils import run_bass_kernel_spmd  # noqa: E402
from concourse.masks import make_identity  # noqa: E402

F32 = mybir.dt.float32
BF16 = mybir.dt.bfloat16
F8E4 = mybir.dt.float8e4
F8E5 = mybir.dt.float8e5
I8 = mybir.dt.int8
AF = mybir.ActivationFunctionType
ALU = mybir.AluOpType
DR = mybir.MatmulPerfMode.DoubleRow

E4NP = ml_dtypes.float8_e4m3
E5NP = ml_dtypes.float8_e5m2
BFNP = ml_dtypes.bfloat16

N_STATE = 1024
N_HEADS = 16
D_HEAD = 64
N_CTX = 4080
T_PAD = 4096
N_CORES = 8
TOK = 512
P = 128
LN_EPS = 1e-5
NSC = 8  # state chunks of 128
NTC = 4  # token chunks per core
NKC = 32  # key chunks of 128
NKCP = 16  # key-chunk pairs of 256
NPAIR = 8  # head pairs

# fp8 scale plan (see numerics2.py)
ALPHA = 4 * np.log2(np.e)  # logit scale in PSUM: psum = ALPHA*(s)
C_SHIFT = 9.0  # global logit shift (measured max 6.21)
PMULT = 512.0  # weights premultiplier (cancels in softmax)
BETA = 96.0  # 60 + 4*log2(PMULT)
LAM_R = 16.0  # LN output scale
LAM_W = 512.0  # Wq/Wk/Wv scale
LAM_Q = float(np.sqrt(ALPHA / 8.0))  # q/k scales; 8*LAM_Q*LAM_K = ALPHA
LAM_V = 16.0
LAM_B = 369.0  # ACT-path bias quant scale; ident diag g = 2^-6, g*LAM_B ~ ALPHA
G_IDENT = 2.0 ** -6
LAM_ATTN = 32.0
LAM_WC = 32768.0
ONES_VAL = LAM_V / LAM_ATTN  # 0.5, folded into denominator columns
GQ = LAM_Q / (LAM_R * LAM_W)
GV = LAM_V / (LAM_R * LAM_W)
G_OUT = 1.0 / (LAM_ATTN * LAM_WC)
EXP_BIAS = float(np.log(PMULT) - C_SHIFT)  # -2.7616
EXP_SCALE = float(1.0 / ALPHA)
STT_CLAMP = -40.0

# kc-pair -> engine assignment (per head-pair parity), tuned for balance
DVE_EVEN = (3, 5, 7, 9, 11, 13, 15)
DVE_ODD = (3, 5, 7, 9, 11, 13, 15)
ACT_KC_LIST = [kc for kcp in range(NKCP) if kcp not in DVE_EVEN
               for kc in (2 * kcp, 2 * kcp + 1)]  # 20 kcs ever handled by ACT
DVE_KC_LIST = [kc for kcp in DVE_ODD for kc in (2 * kcp, 2 * kcp + 1)]  # 14 kcs
ACT_SLOT = {kc: i for i, kc in enumerate(ACT_KC_LIST)}
DVE_SLOT = {kc: i for i, kc in enumerate(DVE_KC_LIST)}
VW = NPAIR * 130 + 32  # v8 tile width: per-pair 130 cols + tail padding
PSQK_BUFS = 2
PSPV_BUFS = 1
P_BUFS = 3
DIAG_SKIP_TAIL = False
DIAG_FORCE = None  # None | "ACT" | "DVE"


def _build_phase1() -> bass.Bass:
    nc = bacc.Bacc("TRN2", target_bir_lowering=False, debug=False, num_devices=N_CORES)
    m_blk = nc.dram_tensor("m_blk", [TOK, N_STATE], F32, kind="ExternalInput")
    Wq8 = nc.dram_tensor("Wq8", [P, 4, 2, N_STATE], F8E4, kind="ExternalInput")
    Wk8 = nc.dram_tensor("Wk8", [P, 4, 2, N_STATE], F8E4, kind="ExternalInput")
    Wv8 = nc.dram_tensor("Wv8", [P, 4, 2, N_STATE], F8E4, kind="ExternalInput")
    bqs = nc.dram_tensor("bqs", [N_STATE], F32, kind="ExternalInput")
    q8_out = nc.dram_tensor("q8_out", [4, P, 2, TOK], F8E4, kind="ExternalOutput")
    k8_out = nc.dram_tensor("k8_out", [4, P, 2, TOK], F8E4, kind="ExternalOutput")
    v8_out = nc.dram_tensor("v8_out", [TOK, N_STATE], F8E4, kind="ExternalOutput")

    with ExitStack() as ctx:
        tc = ctx.enter_context(tile.TileContext(nc))
        consts = ctx.enter_context(tc.tile_pool(name="consts", bufs=1))
        small = ctx.enter_context(tc.tile_pool(name="small", bufs=4))
        work = ctx.enter_context(tc.tile_pool(name="work", bufs=2))
        psum = ctx.enter_context(tc.tile_pool(name="psum", bufs=2, space="PSUM"))
        pst_pool = ctx.enter_context(tc.tile_pool(name="pst", bufs=2, space="PSUM"))

        identB = consts.tile([P, P], BF16)
        make_identity(nc, identB)
        eps_sb = consts.tile([P, 1], F32)
        nc.vector.memset(eps_sb, LN_EPS / (LAM_R * LAM_R))
        bqs_sb = consts.tile([P, NSC], F32)

        m_sb = consts.tile([P, NTC, N_STATE], F32)
        w_sb = {}
        for name, w in (("Wq8", Wq8), ("Wk8", Wk8), ("Wv8", Wv8)):
            w_sb[name] = consts.tile([P, 4, 2, N_STATE], F8E4, name=f"{name}_sb")

        def ld_m(tcn):
            nc.sync.dma_start(
                out=m_sb[:, tcn, :],
                in_=m_blk.rearrange("(c p) s -> p c s", p=P)[:, tcn, :],
            )

        ld_m(0)
        ld_m(1)
        ld_m(2)
        ld_m(3)
        nc.sync.dma_start(out=bqs_sb, in_=bqs.rearrange("(j p) -> p j", p=P))
        nc.sync.dma_start(out=w_sb["Wq8"], in_=Wq8[:, :, :, :])
        nc.sync.dma_start(out=w_sb["Wk8"], in_=Wk8[:, :, :, :])
        nc.sync.dma_start(out=w_sb["Wv8"], in_=Wv8[:, :, :, :])

        # LayerNorm -> xcB = (m - mu) * rstd * LAM_R in bf16
        xcB = consts.tile([P, NTC, N_STATE], BF16)
        for tcn in range(NTC):
            ssum = small.tile([P, 1], F32, tag="ssum")
            nc.vector.reduce_sum(ssum, m_sb[:, tcn, :], axis=mybir.AxisListType.X)
            negmean = small.tile([P, 1], F32, tag="negmean")
            nc.scalar.mul(negmean, ssum, -1.0 / N_STATE)
            sqscr = work.tile([P, N_STATE], BF16, tag="sqscr")
            sqsum = small.tile([P, 1], F32, tag="sqsum")
            nc.scalar.activation(
                out=sqscr, in_=m_sb[:, tcn, :], func=AF.Square, accum_out=sqsum
            )
            # 1024*var = sqsum - ssum^2/1024
            musq = small.tile([P, 1], F32, tag="musq")
            nc.vector.scalar_tensor_tensor(
                out=musq, in0=ssum, scalar=-1.0 / N_STATE, in1=ssum,
                op0=ALU.mult, op1=ALU.mult,
            )
            nvar = small.tile([P, 1], F32, tag="nvar")
            nc.vector.tensor_tensor(out=nvar, in0=sqsum, in1=musq, op=ALU.add)
            std = small.tile([P, 1], F32, tag="std")
            nc.scalar.activation(
                out=std, in_=nvar, func=AF.Sqrt, bias=eps_sb,
                scale=1.0 / (N_STATE * LAM_R * LAM_R),
            )
            rstdl = small.tile([P, 1], F32, tag="rstdl")
            nc.vector.reciprocal(rstdl, std)
            nc.gpsimd.tensor_scalar(
                out=xcB[:, tcn, :],
                in0=m_sb[:, tcn, :],
                scalar1=negmean,
                scalar2=rstdl,
                op0=ALU.add,
                op1=ALU.mult,
            )

        # transpose to state-major and quantize: rT8 [128, sc, 512] e4m3
        rT8 = consts.tile([P, NSC, TOK], F8E4)
        for sc in range(NSC):
            pst = pst_pool.tile([P, TOK], BF16, tag="pst")
            for tcn in range(NTC):
                nc.tensor.transpose(
                    pst[:, tcn * P : (tcn + 1) * P],
                    xcB[:, tcn, sc * P : (sc + 1) * P],
                    identB,
                )
            if sc % 2 == 0:
                nc.vector.tensor_copy(rT8[:, sc, :], pst)
            else:
                nc.scalar.copy(rT8[:, sc, :], pst)

        # q/k: DoubleRow fp8 matmuls, evacuate into [128, 2, 512] pair tiles
        q8g = [consts.tile([P, 2, TOK], F8E4, name=f"q8g{g}") for g in range(4)]
        k8g = [consts.tile([P, 2, TOK], F8E4, name=f"k8g{g}") for g in range(4)]
        v8sb = consts.tile([P, NTC, N_STATE], F8E4)

        def emit_v(tcn):
            psv = psum.tile([P, N_STATE], F32, tag="psv", bufs=1)
            for pc in range(2):
                for s in range(4):
                    nc.tensor.matmul(
                        psv[:, pc * TOK : (pc + 1) * TOK],
                        lhsT=rT8[:, 2 * s : 2 * s + 2, tcn * P : (tcn + 1) * P],
                        rhs=w_sb["Wv8"][:, s, :, pc * TOK : (pc + 1) * TOK],
                        start=(s == 0),
                        stop=(s == 3),
                        perf_mode=DR,
                    )
            if tcn % 2 == 0:
                nc.scalar.mul(v8sb[:, tcn, :], psv, GV)
            else:
                nc.vector.tensor_scalar(
                    out=v8sb[:, tcn, :], in0=psv, scalar1=GV, scalar2=None,
                    op0=ALU.mult,
                )
            nc.sync.dma_start(
                out=v8_out.rearrange("(c p) s -> p c s", p=P)[:, tcn, :],
                in_=v8sb[:, tcn, :],
            )

        for j in range(NSC):
            g, half = j // 2, j % 2
            psq = psum.tile([P, TOK], F32, tag="psq")
            psk = psum.tile([P, TOK], F32, tag="psk")
            for s in range(4):
                nc.tensor.matmul(
                    psq,
                    lhsT=w_sb["Wq8"][:, s, :, j * P : (j + 1) * P],
                    rhs=rT8[:, 2 * s : 2 * s + 2, :],
                    start=(s == 0),
                    stop=(s == 3),
                    perf_mode=DR,
                )
            for s in range(4):
                nc.tensor.matmul(
                    psk,
                    lhsT=w_sb["Wk8"][:, s, :, j * P : (j + 1) * P],
                    rhs=rT8[:, 2 * s : 2 * s + 2, :],
                    start=(s == 0),
                    stop=(s == 3),
                    perf_mode=DR,
                )
            for t in range(2):
                nc.scalar.activation(
                    out=q8g[g][64 * half : 64 * half + 64, t, :],
                    in_=psq[64 * t : 64 * t + 64, :],
                    func=AF.Identity,
                    bias=bqs_sb[64 * t : 64 * t + 64, j : j + 1],
                    scale=GQ,
                )
                nc.vector.tensor_scalar(
                    out=k8g[g][64 * half : 64 * half + 64, t, :],
                    in0=psk[64 * t : 64 * t + 64, :],
                    scalar1=GQ,
                    scalar2=None,
                    op0=ALU.mult,
                )
            if j % 2 == 1:
                emit_v(j // 2)
                nc.sync.dma_start(out=q8_out[g, :, :, :], in_=q8g[g])
                nc.sync.dma_start(out=k8_out[g, :, :, :], in_=k8g[g])
    nc.compile()
    return nc


def _build_phase2() -> bass.Bass:
    nc = bacc.Bacc("TRN2", target_bir_lowering=False, debug=False, num_devices=N_CORES)
    q8_in = nc.dram_tensor("q8_in", [4, P, 2, TOK], F8E4, kind="ExternalInput")
    k8_in = nc.dram_tensor("k8_in", [4, P, 2, T_PAD], F8E4, kind="ExternalInput")
    v8_in = nc.dram_tensor("v8_in", [4, P, NKC // 4, VW], F8E4, kind="ExternalInput")
    b8_in = nc.dram_tensor("b8_in", [len(ACT_KC_LIST), P, 2, TOK], F8E4, kind="ExternalInput")
    bt_in = nc.dram_tensor("bt_in", [len(DVE_KC_LIST), P, TOK], BF16, kind="ExternalInput")
    mres = nc.dram_tensor("mres", [TOK, N_STATE], F32, kind="ExternalInput")
    Wc8 = nc.dram_tensor("Wc8", [P, 4, 2, N_STATE], F8E4, kind="ExternalInput")
    o_out = nc.dram_tensor("o_out", [TOK, N_STATE], BF16, kind="ExternalOutput")

    with ExitStack() as ctx:
        tc = ctx.enter_context(tile.TileContext(nc))
        consts = ctx.enter_context(tc.tile_pool(name="consts", bufs=1))
        small = ctx.enter_context(tc.tile_pool(name="small", bufs=4))
        ppool = ctx.enter_context(tc.tile_pool(name="ppool", bufs=P_BUFS))
        psqk = ctx.enter_context(tc.tile_pool(name="psqk", bufs=PSQK_BUFS, space="PSUM"))
        pspv = ctx.enter_context(tc.tile_pool(name="pspv", bufs=PSPV_BUFS, space="PSUM"))

        identg = consts.tile([P, 2, P], F8E4)
        nc.vector.memset(identg, 0.0)
        make_identity(nc, identg[:, 0, :])
        make_identity(nc, identg[:, 1, :])
        identg2 = consts.tile([P, 2, P], F8E4)
        nc.scalar.mul(identg2, identg, G_IDENT / 2.0)
        expb_ap = consts.tile([P, 1], F32)
        nc.vector.memset(expb_ap, EXP_BIAS)

        q8sb = consts.tile([P, 4, 2, TOK], F8E4)
        k8sb = consts.tile([P, 4, 2, T_PAD], F8E4)
        v8sb = consts.tile([P, NKC, VW], F8E4)
        b8sb = consts.tile([P, len(ACT_KC_LIST), 2, TOK], F8E4)
        btsb = consts.tile([P, len(DVE_KC_LIST), TOK], BF16)
        m_sb = consts.tile([P, NTC, N_STATE], F32)
        wc_sb = consts.tile([P, 4, 2, N_STATE], F8E4)
        nA, nD = len(ACT_KC_LIST), len(DVE_KC_LIST)
        bA = [0, 5, 10, 15, nA]
        bD = [0, 4, 8, 11, nD]

        def ld_q8(g):
            nc.sync.dma_start(out=q8sb[:, g, :, :], in_=q8_in[g, :, :, :])

        def ld_k8(g, split=False):
            if split:
                nc.sync.dma_start(
                    out=k8sb[:, g, :, 0 : T_PAD // 2],
                    in_=k8_in[g, :, :, 0 : T_PAD // 2],
                )
                nc.sync.dma_start(
                    out=k8sb[:, g, :, T_PAD // 2 :],
                    in_=k8_in[g, :, :, T_PAD // 2 :],
                )
            else:
                nc.sync.dma_start(out=k8sb[:, g, :, :], in_=k8_in[g, :, :, :])

        def ld_b8(i):
            nc.sync.dma_start(
                out=b8sb[:, bA[i] : bA[i + 1], :, :],
                in_=b8_in[bA[i] : bA[i + 1], :, :, :].rearrange("k p t n -> p k t n"),
            )

        def ld_bt(i):
            nc.sync.dma_start(
                out=btsb[:, bD[i] : bD[i + 1], :],
                in_=bt_in[bD[i] : bD[i + 1], :, :].rearrange("k p n -> p k n"),
            )

        def ld_v8(q):
            nc.sync.dma_start(
                out=v8sb[:, q * 8 : (q + 1) * 8, :], in_=v8_in[q, :, :, :]
            )

        ld_q8(0)
        ld_k8(0, split=True)
        ld_b8(0)
        ld_bt(0)
        ld_v8(0)
        ld_b8(1)
        ld_bt(1)
        ld_v8(1)
        ld_b8(2)
        ld_bt(2)
        ld_v8(2)
        ld_b8(3)
        ld_bt(3)
        ld_v8(3)
        ld_k8(1)
        ld_q8(1)
        ld_k8(2)
        ld_q8(2)
        ld_k8(3)
        ld_q8(3)
        nc.sync.dma_start(out=m_sb, in_=mres.rearrange("(c p) s -> p c s", p=P))
        nc.sync.dma_start(out=wc_sb, in_=Wc8[:, :, :, :])

        attnT8 = consts.tile([P, NPAIR, TOK], F8E4)

        for j in range(NPAIR):
            g, half = j // 2, j % 2
            dve_set = DVE_ODD if (j % 2) else DVE_EVEN
            if DIAG_FORCE == "ACT":
                dve_set = ()
            elif DIAG_FORCE == "DVE":
                dve_set = tuple(range(NKCP))
            pvA = pspv.tile([96, TOK], F32, tag="pvA")
            pvB = pspv.tile([96, TOK], F32, tag="pvB")
            for kcp in range(NKCP):
                is_dve = kcp in dve_set
                if is_dve:
                    ptile = ppool.tile([P, 2, 2, TOK], F8E5, tag="pD")
                else:
                    ptile = ppool.tile([P, 2, 2, TOK], F8E4, tag="pA")
                for sub in range(2):
                    kc = 2 * kcp + sub
                    if is_dve:
                        # DVE stream: per-head 1-bank psums, independent of
                        # the ACT stream so both engines pipeline in parallel
                        for h in range(2):
                            base = 64 * half + 32 * h
                            psd = psqk.tile([P, TOK], F32, tag="sD")
                            nc.tensor.matmul(
                                psd,
                                lhsT=k8sb[base : base + 32, g, :, kc * P : (kc + 1) * P],
                                rhs=q8sb[base : base + 32, g, :, :],
                                start=True,
                                stop=True,
                                perf_mode=DR,
                                tile_position=(base, 0),
                            )
                            nc.vector.scalar_tensor_tensor(
                                out=ptile[:, h, sub, :].bitcast(I8),
                                in0=psd,
                                scalar=STT_CLAMP,
                                in1=btsb[:, DVE_SLOT.get(kc, kc % len(DVE_KC_LIST)), :],
                                op0=ALU.max,
                                op1=ALU.add,
                            )
                    else:
                        ps = psqk.tile([P, 2, TOK], F32, tag="sA")
                        for h in range(2):
                            base = 64 * half + 32 * h
                            nc.tensor.matmul(
                                ps[:, h, :],
                                lhsT=k8sb[base : base + 32, g, :, kc * P : (kc + 1) * P],
                                rhs=q8sb[base : base + 32, g, :, :],
                                start=True,
                                stop=False,
                                perf_mode=DR,
                                tile_position=(base, 0),
                            )
                            nc.tensor.matmul(
                                ps[:, h, :],
                                lhsT=identg2,
                                rhs=b8sb[:, ACT_SLOT.get(kc, kc % len(ACT_KC_LIST)), :, :],
                                start=False,
                                stop=True,
                                perf_mode=DR,
                                skip_group_check=True,
                            )
                        nc.scalar.activation(
                            out=ptile[:, :, sub, :],
                            in_=ps,
                            func=AF.Exp,
                            bias=expb_ap,
                            scale=EXP_SCALE,
                        )
                nc.tensor.matmul(
                    pvA,
                    lhsT=v8sb[:, 2 * kcp : 2 * kcp + 2, 130 * j : 130 * j + 96],
                    rhs=ptile[:, 0, :, :],
                    start=(kcp == 0),
                    stop=(kcp == NKCP - 1),
                    perf_mode=DR,
                )
                nc.tensor.matmul(
                    pvB,
                    lhsT=v8sb[:, 2 * kcp : 2 * kcp + 2, 130 * j + 65 : 130 * j + 161],
                    rhs=ptile[:, 1, :, :],
                    start=(kcp == 0),
                    stop=(kcp == NKCP - 1),
                    perf_mode=DR,
                )

            if DIAG_SKIP_TAIL:
                nc.vector.memset(attnT8[:, j, :], 0.01)
                continue
            # fast evac: stage pv to SBUF (frees the psum banks), then
            # normalize off the critical path using the idle Pool engine
            stA = small.tile([65, TOK], BF16, tag="stA", bufs=2)
            stB = small.tile([65, TOK], BF16, tag="stB", bufs=2)
            nc.scalar.copy(stA, pvA[0:65, :])
            nc.vector.tensor_copy(stB, pvB[0:65, :])
            recA = small.tile([1, TOK], BF16, tag="recA", bufs=2)
            recB = small.tile([1, TOK], BF16, tag="recB", bufs=2)
            with nc.allow_low_precision("bf16 softmax denominators, ~0.4% scale"):
                nc.vector.reciprocal(recA, stA[64:65, :])
                nc.vector.reciprocal(recB, stB[64:65, :])
            bcastA = small.tile([64, TOK], BF16, tag="bcastA", bufs=2)
            bcastB = small.tile([64, TOK], BF16, tag="bcastB", bufs=2)
            nc.gpsimd.partition_broadcast(bcastA, recA, channels=64)
            nc.gpsimd.partition_broadcast(bcastB, recB, channels=64)
            nc.gpsimd.tensor_tensor(
                out=attnT8[0:64, j, :], in0=stA[0:64, :], in1=bcastA, op=ALU.mult
            )
            nc.gpsimd.tensor_tensor(
                out=attnT8[64:128, j, :], in0=stB[0:64, :], in1=bcastB, op=ALU.mult
            )

        # output projection (fp8 DR) + residual add (f32)
        o_sb = consts.tile([P, NTC, N_STATE], BF16)
        for qc in range(NTC):
            ps_o = psqk.tile([P, 2, TOK], F32, tag="sA")
            po = ps_o.rearrange("p a b -> p (a b)")
            for pc in range(2):
                for u in range(4):
                    nc.tensor.matmul(
                        ps_o[:, pc, :],
                        lhsT=attnT8[:, 2 * u : 2 * u + 2, qc * P : (qc + 1) * P],
                        rhs=wc_sb[:, u, :, pc * TOK : (pc + 1) * TOK],
                        start=(u == 0),
                        stop=(u == 3),
                        perf_mode=DR,
                    )
            nc.vector.scalar_tensor_tensor(
                out=o_sb[:, qc, :],
                in0=po,
                scalar=G_OUT,
                in1=m_sb[:, qc, :],
                op0=ALU.mult,
                op1=ALU.add,
            )
        for qc in range(NTC):
            nc.sync.dma_start(
                out=o_out.rearrange("(c p) s -> p c s", p=P)[:, qc, :],
                in_=o_sb[:, qc, :],
            )
    nc.compile()
    return nc


_NC_CACHE = {}


def _get_nc(which):
    if which not in _NC_CACHE:
        _NC_CACHE[which] = _build_phase1() if which == 1 else _build_phase2()
    return _NC_CACHE[which]


def _perm_cols():
    """Column permutation for q/k weights: per pair j, [hA d0:32 | hB d0:32 |
    hA d32:64 | hB d32:64]."""
    order = []
    for j in range(NSC):
        hA, hB = 2 * j, 2 * j + 1
        order.extend(range(hA * 64, hA * 64 + 32))
        order.extend(range(hB * 64, hB * 64 + 32))
        order.extend(range(hA * 64 + 32, hA * 64 + 64))
        order.extend(range(hB * 64 + 32, hB * 64 + 64))
    return np.array(order)


def _w_dr_layout(w8):
    """[1024, C] -> [128, 4, 2, C] DoubleRow lhsT layout."""
    return np.ascontiguousarray(
        w8.reshape(4, 2, P, -1).transpose(2, 0, 1, 3)
    )


def kernel(m, bias, gamma, beta, Wq, bq, Wk, Wv, bv, Wc, bc, _want_timing=None):
    m = np.asarray(m, dtype=np.float32).reshape(N_CTX, N_STATE)
    bias = np.asarray(bias, np.float32)
    gamma = np.asarray(gamma, np.float32)
    beta = np.asarray(beta, np.float32)
    Wq = np.asarray(Wq, np.float32)
    Wk = np.asarray(Wk, np.float32)
    Wv = np.asarray(Wv, np.float32)
    Wc = np.asarray(Wc, np.float32)
    bq = np.asarray(bq, np.float32)
    bv = np.asarray(bv, np.float32)
    bc = np.asarray(bc, np.float32)

    m_pad = np.zeros((T_PAD, N_STATE), np.float32)
    m_pad[:N_CTX] = m

    # fold gamma into weights, beta into biases; bv and bc fold into residual
    Wqf = gamma[:, None] * Wq
    Wkf = gamma[:, None] * Wk
    Wvf = gamma[:, None] * Wv
    bqf = bq + beta @ Wq
    # beta@Wk shifts all logits of a query equally -> softmax invariant; drop.
    # beta@Wv + bv shift attention output -> fold into residual with bc.
    perm = _perm_cols()
    Wq8 = _w_dr_layout((LAM_W * Wqf[:, perm]).astype(E4NP))
    Wk8 = _w_dr_layout((LAM_W * Wkf[:, perm]).astype(E4NP))
    Wv8 = _w_dr_layout((LAM_W * Wvf).astype(E4NP))
    bqs = (LAM_Q * bqf[perm]).astype(np.float32)
    Wc8 = _w_dr_layout((LAM_WC * Wc).astype(E4NP))
    mres_full = m_pad + (bc + (bv + beta @ Wv) @ Wc)[None, :]

    import sys as _sys

    def _log(*a):
        print("[kernel]", *a, file=_sys.stderr, flush=True)

    nc1 = _get_nc(1)
    _log("phase1 built")
    in_maps1 = []
    for c in range(N_CORES):
        in_maps1.append(
            {
                "m_blk": np.ascontiguousarray(m_pad[c * TOK : (c + 1) * TOK]),
                "Wq8": Wq8,
                "Wk8": Wk8,
                "Wv8": Wv8,
                "bqs": bqs,
            }
        )
    res1 = run_bass_kernel_spmd(nc1, in_maps1, core_ids=list(range(N_CORES)))
    _log("phase1 done")

    q8_blks = [r["q8_out"] for r in res1.results]
    k8_full = np.concatenate([r["k8_out"] for r in res1.results], axis=3)
    v8_full = np.concatenate([r["v8_out"] for r in res1.results], axis=0)
    v8_full[N_CTX:] = 0  # pad tokens carry no value

    # v8 pair-tile layout [128, 32, VW] with denominator columns
    v8f = v8_full.astype(np.float32).reshape(NKC, P, N_HEADS, D_HEAD)
    v8h = np.zeros((P, NKC, VW), np.float32)
    for j in range(NPAIR):
        v8h[:, :, 130 * j : 130 * j + 64] = v8f[:, :, 2 * j].transpose(1, 0, 2)
        v8h[:, :, 130 * j + 65 : 130 * j + 129] = v8f[:, :, 2 * j + 1].transpose(1, 0, 2)
        v8h[:, :, 130 * j + 64] = ONES_VAL
        v8h[:, :, 130 * j + 129] = ONES_VAL
    # zero the denominator contribution of padded keys
    keyidx = (np.arange(NKC)[None, :] * P + np.arange(P)[:, None])  # [p, kc]
    padmask = keyidx >= N_CTX
    for j in range(NPAIR):
        v8h[:, :, 130 * j + 64][padmask] = 0.0
        v8h[:, :, 130 * j + 129][padmask] = 0.0
    v8h8 = v8h.astype(E4NP)
    v8_dr = np.ascontiguousarray(
        v8h8.reshape(P, 4, NKC // 4, VW).transpose(1, 0, 2, 3)
    )

    biasT = np.ascontiguousarray(bias.T)  # [k, q]

    nc2 = _get_nc(2)
    _log("phase2 built")
    in_maps2 = []
    for c in range(N_CORES):
        qs = slice(c * TOK, (c + 1) * TOK)
        b8 = np.zeros((len(ACT_KC_LIST), P, 2, TOK), E4NP)
        for i, kc in enumerate(ACT_KC_LIST):
            chunk = (LAM_B * biasT[kc * P : (kc + 1) * P, qs]).astype(E4NP)
            b8[i, :, 0, :] = chunk
            b8[i, :, 1, :] = chunk
        bt = np.zeros((len(DVE_KC_LIST), P, TOK), BFNP)
        for i, kc in enumerate(DVE_KC_LIST):
            bt[i] = (
                ALPHA * biasT[kc * P : (kc + 1) * P, qs] + (BETA - ALPHA * C_SHIFT)
            ).astype(BFNP)
        in_maps2.append(
            {
                "q8_in": np.ascontiguousarray(q8_blks[c]),
                "k8_in": k8_full,
                "v8_in": v8_dr,
                "b8_in": b8,
                "bt_in": bt,
                "mres": np.ascontiguousarray(mres_full[qs]),
                "Wc8": Wc8,
            }
        )
    res2 = run_bass_kernel_spmd(nc2, in_maps2, core_ids=list(range(N_CORES)))
    _log("phase2 done")
    o = np.concatenate([r["o_out"] for r in res2.results], axis=0)[:N_CTX]
    if _want_timing is not None:
        _want_timing["res1"] = res1
        _want_timing["res2"] = res2
    return o.reshape(1, N_CTX, N_STATE).astype(np.float32)


# revision 23
# speedup vs baseline: 1.7688x; 1.0082x over previous
"""AttentionResblock on 8 NeuronCores (Trainium2, Bass/Tile) — fp8 edition.

Sharding: query-token blocks of 512 (T_PAD=4096 = 8 x 512), two launches:
  Phase 1 (per core c): LayerNorm + Q/K/V projections (fp8 DoubleRow matmuls)
    for token rows [512c, 512c+512). Emits q8/k8 in DoubleRow-ready
    [128, 2, 512] head-pair tiles and v8 token-major, all fp8-e4m3.
    gamma/beta/bv/bc are folded into weights/residual on the host.
  Phase 2 (per core c): 16-head attention for its 512 query rows over all
    4096 keys. QK via fp8 DoubleRow (2x32 contraction). Softmax weights are
    produced two ways, split across engines to balance the timeline:
      - ACT chunks: bias added in PSUM via fp8 ident-matmul, then true
        exp -> fp8-e4m3 (premultiplied by 512, shifted by C=9).
      - DVE chunks: fastexp bit trick - y = int8(alpha*s + [alpha*b +
        beta - alpha*C]) bitcast as fp8-e5m2 (2^(y/4-15) ~ 512*e^(s+b-9)).
    The scale/shift cancels in softmax: PV accumulates numerator and
    denominator (ones columns in the fp8 V tiles, value 0.5 = LAM_V/LAM_ATTN)
    with fp8 DoubleRow over key-chunk pairs. Normalize, fp8 DoubleRow output
    projection, f32 residual add.

Numerics validated against the reference in numpy (numerics2.py): rel err
~1.5e-4 vs the 2e-2 gate, dominated by fp8 quantization of the attention
branch, which is scaled by ~1e-3 through Wc so the f32 residual dominates.
"""

import sys

sys.path.insert(0, "/opt/trn_rl_repo")

from contextlib import ExitStack  # noqa: E402

import numpy as np  # noqa: E402
import ml_dtypes  # noqa: E402

import concourse.bass as bass  # noqa: E402
import concourse.bacc as bacc  # noqa: E402
import concourse.tile as tile  # noqa: E402
from concourse import mybir  # noqa: E402
from concourse.bass_utils import run_bass_kernel_spmd  # noqa: E402
from concourse.masks import make_identity  # noqa: E402

F32 = mybir.dt.float32
BF16 = mybir.dt.bfloat16
F8E4 = mybir.dt.float8e4
F8E5 = mybir.dt.float8e5
I8 = mybir.dt.int8
AF = mybir.ActivationFunctionType
ALU = mybir.AluOpType
DR = mybir.MatmulPerfMode.DoubleRow

E4NP = ml_dtypes.float8_e4m3
E5NP = ml_dtypes.float8_e5m2
BFNP = ml_dtypes.bfloat16

N_STATE = 1024
N_HEADS = 16
D_HEAD = 64
N_CTX = 4080
T_PAD = 4096
N_CORES = 8
TOK = 512
P = 128
LN_EPS = 1e-5
NSC = 8  # state chunks of 128
NTC = 4  # token chunks per core
NKC = 32  # key chunks of 128
NKCP = 16  # key-chunk pairs of 256
NPAIR = 8  # head pairs

# fp8 scale plan (see numerics2.py)
ALPHA = 4 * np.log2(np.e)  # logit scale in PSUM: psum = ALPHA*(s)
C_SHIFT = 9.0  # global logit shift (measured max 6.21)
PMULT = 512.0  # weights premultiplier (cancels in softmax)
BETA = 96.0  # 60 + 4*log2(PMULT)
LAM_R = 16.0  # LN output scale
LAM_W = 512.0  # Wq/Wk/Wv scale
LAM_Q = float(np.sqrt(ALPHA / 8.0))  # q/k scales; 8*LAM_Q*LAM_K = ALPHA
LAM_V = 16.0
LAM_B = 369.0  # ACT-path bias quant scale; ident diag g = 2^-6, g*LAM_B ~ ALPHA
G_IDENT = 2.0 ** -6
LAM_ATTN = 32.0
LAM_WC = 32768.0
ONES_VAL = LAM_V / LAM_ATTN  # 0.5, folded into denominator columns
GQ = LAM_Q / (LAM_R * LAM_W)
GV = LAM_V / (LAM_R * LAM_W)
G_OUT = 1.0 / (LAM_ATTN * LAM_WC)
EXP_BIAS = float(np.log(PMULT) - C_SHIFT)  # -2.7616
EXP_SCALE = float(1.0 / ALPHA)
STT_CLAMP = -40.0

# kc-pair -> engine assignment (per head-pair parity), tuned for balance
DVE_EVEN = (3, 5, 7, 9, 11, 13, 15)
DVE_ODD = (3, 5, 7, 9, 11, 13, 15)
ACT_KC_LIST = [kc for kcp in range(NKCP) if kcp not in DVE_EVEN
               for kc in (2 * kcp, 2 * kcp + 1)]  # 20 kcs ever handled by ACT
DVE_KC_LIST = [kc for kcp in DVE_ODD for kc in (2 * kcp, 2 * kcp + 1)]  # 14 kcs
ACT_SLOT = {kc: i for i, kc in enumerate(ACT_KC_LIST)}
DVE_SLOT = {kc: i for i, kc in enumerate(DVE_KC_LIST)}
VW = NPAIR * 130 + 32  # v8 tile width: per-pair 130 cols + tail padding
PSQK_BUFS = 2
PSPV_BUFS = 1
P_BUFS = 3
DIAG_SKIP_TAIL = False
DIAG_FORCE = None  # None | "ACT" | "DVE"


def _build_phase1() -> bass.Bass:
    nc = bacc.Bacc("TRN2", target_bir_lowering=False, debug=False, num_devices=N_CORES)
    m_blk = nc.dram_tensor("m_blk", [TOK, N_STATE], BF16, kind="ExternalInput")
    Wq8 = nc.dram_tensor("Wq8", [P, 4, 2, N_STATE], F8E4, kind="ExternalInput")
    Wk8 = nc.dram_tensor("Wk8", [P, 4, 2, N_STATE], F8E4, kind="ExternalInput")
    Wv8 = nc.dram_tensor("Wv8", [P, 4, 2, N_STATE], F8E4, kind="ExternalInput")
    bqs = nc.dram_tensor("bqs", [N_STATE], F32, kind="ExternalInput")
    q8_out = nc.dram_tensor("q8_out", [4, P, 2, TOK], F8E4, kind="ExternalOutput")
    k8_out = nc.dram_tensor("k8_out", [4, P, 2, TOK], F8E4, kind="ExternalOutput")
    v8_out = nc.dram_tensor("v8_out", [TOK, N_STATE], F8E4, kind="ExternalOutput")

    with ExitStack() as ctx:
        tc = ctx.enter_context(tile.TileContext(nc))
        consts = ctx.enter_context(tc.tile_pool(name="consts", bufs=1))
        small = ctx.enter_context(tc.tile_pool(name="small", bufs=4))
        work = ctx.enter_context(tc.tile_pool(name="work", bufs=2))
        psum = ctx.enter_context(tc.tile_pool(name="psum", bufs=2, space="PSUM"))
        pst_pool = ctx.enter_context(tc.tile_pool(name="pst", bufs=2, space="PSUM"))

        identB = consts.tile([P, P], BF16)
        make_identity(nc, identB)
        eps_sb = consts.tile([P, 1], F32)
        nc.vector.memset(eps_sb, LN_EPS / (LAM_R * LAM_R))
        bqs_sb = consts.tile([P, NSC], F32)

        m_sb = consts.tile([P, NTC, N_STATE], BF16)
        w_sb = {}
        for name, w in (("Wq8", Wq8), ("Wk8", Wk8), ("Wv8", Wv8)):
            w_sb[name] = consts.tile([P, 4, 2, N_STATE], F8E4, name=f"{name}_sb")

        def ld_m(tcn):
            nc.sync.dma_start(
                out=m_sb[:, tcn, :],
                in_=m_blk.rearrange("(c p) s -> p c s", p=P)[:, tcn, :],
            )

        ld_m(0)
        ld_m(1)
        ld_m(2)
        ld_m(3)
        nc.sync.dma_start(out=bqs_sb, in_=bqs.rearrange("(j p) -> p j", p=P))
        nc.sync.dma_start(out=w_sb["Wq8"], in_=Wq8[:, :, :, :])
        nc.sync.dma_start(out=w_sb["Wk8"], in_=Wk8[:, :, :, :])
        nc.sync.dma_start(out=w_sb["Wv8"], in_=Wv8[:, :, :, :])

        # LayerNorm -> xcB = (m - mu) * rstd * LAM_R in bf16
        xcB = consts.tile([P, NTC, N_STATE], BF16)
        for tcn in range(NTC):
            ssum = small.tile([P, 1], F32, tag="ssum")
            nc.vector.reduce_sum(ssum, m_sb[:, tcn, :], axis=mybir.AxisListType.X)
            negmean = small.tile([P, 1], F32, tag="negmean")
            nc.scalar.mul(negmean, ssum, -1.0 / N_STATE)
            sqscr = work.tile([P, N_STATE], BF16, tag="sqscr")
            sqsum = small.tile([P, 1], F32, tag="sqsum")
            nc.scalar.activation(
                out=sqscr, in_=m_sb[:, tcn, :], func=AF.Square, accum_out=sqsum
            )
            # 1024*var = sqsum - ssum^2/1024
            musq = small.tile([P, 1], F32, tag="musq")
            nc.vector.scalar_tensor_tensor(
                out=musq, in0=ssum, scalar=-1.0 / N_STATE, in1=ssum,
                op0=ALU.mult, op1=ALU.mult,
            )
            nvar = small.tile([P, 1], F32, tag="nvar")
            nc.vector.tensor_tensor(out=nvar, in0=sqsum, in1=musq, op=ALU.add)
            std = small.tile([P, 1], F32, tag="std")
            nc.scalar.activation(
                out=std, in_=nvar, func=AF.Sqrt, bias=eps_sb,
                scale=1.0 / (N_STATE * LAM_R * LAM_R),
            )
            rstdl = small.tile([P, 1], F32, tag="rstdl")
            nc.vector.reciprocal(rstdl, std)
            eng = nc.gpsimd if tcn % 2 == 0 else nc.vector
            eng.tensor_scalar(
                out=xcB[:, tcn, :],
                in0=m_sb[:, tcn, :],
                scalar1=negmean,
                scalar2=rstdl,
                op0=ALU.add,
                op1=ALU.mult,
            )

        # transpose to state-major and quantize: rT8 [128, sc, 512] e4m3
        rT8 = consts.tile([P, NSC, TOK], F8E4)
        for sc in range(NSC):
            pst = pst_pool.tile([P, TOK], BF16, tag="pst")
            for tcn in range(NTC):
                nc.tensor.transpose(
                    pst[:, tcn * P : (tcn + 1) * P],
                    xcB[:, tcn, sc * P : (sc + 1) * P],
                    identB,
                )
            if sc % 2 == 0:
                nc.vector.tensor_copy(rT8[:, sc, :], pst)
            else:
                nc.scalar.copy(rT8[:, sc, :], pst)

        # q/k: DoubleRow fp8 matmuls, evacuate into [128, 2, 512] pair tiles
        q8g = [consts.tile([P, 2, TOK], F8E4, name=f"q8g{g}") for g in range(4)]
        k8g = [consts.tile([P, 2, TOK], F8E4, name=f"k8g{g}") for g in range(4)]
        v8sb = consts.tile([P, NTC, N_STATE], F8E4)

        def emit_v(tcn):
            psv = psum.tile([P, N_STATE], F32, tag="psv", bufs=1)
            for pc in range(2):
                for s in range(4):
                    nc.tensor.matmul(
                        psv[:, pc * TOK : (pc + 1) * TOK],
                        lhsT=rT8[:, 2 * s : 2 * s + 2, tcn * P : (tcn + 1) * P],
                        rhs=w_sb["Wv8"][:, s, :, pc * TOK : (pc + 1) * TOK],
                        start=(s == 0),
                        stop=(s == 3),
                        perf_mode=DR,
                    )
            if tcn % 2 == 0:
                nc.scalar.mul(v8sb[:, tcn, :], psv, GV)
            else:
                nc.vector.tensor_scalar(
                    out=v8sb[:, tcn, :], in0=psv, scalar1=GV, scalar2=None,
                    op0=ALU.mult,
                )
            nc.sync.dma_start(
                out=v8_out.rearrange("(c p) s -> p c s", p=P)[:, tcn, :],
                in_=v8sb[:, tcn, :],
            )

        for j in range(NSC):
            g, half = j // 2, j % 2
            psq = psum.tile([P, TOK], F32, tag="psq")
            psk = psum.tile([P, TOK], F32, tag="psk")
            for s in range(4):
                nc.tensor.matmul(
                    psq,
                    lhsT=w_sb["Wq8"][:, s, :, j * P : (j + 1) * P],
                    rhs=rT8[:, 2 * s : 2 * s + 2, :],
                    start=(s == 0),
                    stop=(s == 3),
                    perf_mode=DR,
                )
            for s in range(4):
                nc.tensor.matmul(
                    psk,
                    lhsT=w_sb["Wk8"][:, s, :, j * P : (j + 1) * P],
                    rhs=rT8[:, 2 * s : 2 * s + 2, :],
                    start=(s == 0),
                    stop=(s == 3),
                    perf_mode=DR,
                )
            for t in range(2):
                nc.scalar.activation(
                    out=q8g[g][64 * half : 64 * half + 64, t, :],
                    in_=psq[64 * t : 64 * t + 64, :],
                    func=AF.Identity,
                    bias=bqs_sb[64 * t : 64 * t + 64, j : j + 1],
                    scale=GQ,
                )
                nc.vector.tensor_scalar(
                    out=k8g[g][64 * half : 64 * half + 64, t, :],
                    in0=psk[64 * t : 64 * t + 64, :],
                    scalar1=GQ,
                    scalar2=None,
                    op0=ALU.mult,
                )
            if j % 2 == 1:
                emit_v(j // 2)
                nc.sync.dma_start(out=q8_out[g, :, :, :], in_=q8g[g])
                nc.sync.dma_start(out=k8_out[g, :, :, :], in_=k8g[g])
    nc.compile()
    return nc


def _build_phase2() -> bass.Bass:
    nc = bacc.Bacc("TRN2", target_bir_lowering=False, debug=False, num_devices=N_CORES)
    q8_in = nc.dram_tensor("q8_in", [4, P, 2, TOK], F8E4, kind="ExternalInput")
    k8_in = nc.dram_tensor("k8_in", [4, P, 2, T_PAD], F8E4, kind="ExternalInput")
    v8_in = nc.dram_tensor("v8_in", [4, P, NKC // 4, VW], F8E4, kind="ExternalInput")
    b8_in = nc.dram_tensor("b8_in", [len(ACT_KC_LIST), P, 2, TOK], F8E4, kind="ExternalInput")
    bt_in = nc.dram_tensor("bt_in", [len(DVE_KC_LIST), P, TOK], BF16, kind="ExternalInput")
    mres = nc.dram_tensor("mres", [TOK, N_STATE], F32, kind="ExternalInput")
    Wc8 = nc.dram_tensor("Wc8", [P, 4, 2, N_STATE], F8E4, kind="ExternalInput")
    o_out = nc.dram_tensor("o_out", [TOK, N_STATE], BF16, kind="ExternalOutput")

    with ExitStack() as ctx:
        tc = ctx.enter_context(tile.TileContext(nc))
        consts = ctx.enter_context(tc.tile_pool(name="consts", bufs=1))
        small = ctx.enter_context(tc.tile_pool(name="small", bufs=4))
        ppool = ctx.enter_context(tc.tile_pool(name="ppool", bufs=P_BUFS))
        psqk = ctx.enter_context(tc.tile_pool(name="psqk", bufs=PSQK_BUFS, space="PSUM"))
        pspv = ctx.enter_context(tc.tile_pool(name="pspv", bufs=PSPV_BUFS, space="PSUM"))

        identg = consts.tile([P, 2, P], F8E4)
        nc.vector.memset(identg, 0.0)
        make_identity(nc, identg[:, 0, :])
        make_identity(nc, identg[:, 1, :])
        identg2 = consts.tile([P, 2, P], F8E4)
        nc.scalar.mul(identg2, identg, G_IDENT / 2.0)
        expb_ap = consts.tile([P, 1], F32)
        nc.vector.memset(expb_ap, EXP_BIAS)

        q8sb = consts.tile([P, 4, 2, TOK], F8E4)
        k8sb = consts.tile([P, 4, 2, T_PAD], F8E4)
        v8sb = consts.tile([P, NKC, VW], F8E4)
        b8sb = consts.tile([P, len(ACT_KC_LIST), 2, TOK], F8E4)
        btsb = consts.tile([P, len(DVE_KC_LIST), TOK], BF16)
        m_sb = consts.tile([P, NTC, N_STATE], F32)
        wc_sb = consts.tile([P, 4, 2, N_STATE], F8E4)
        nA, nD = len(ACT_KC_LIST), len(DVE_KC_LIST)
        bA = [0, 5, 10, 15, nA]
        bD = [0, 4, 8, 11, nD]

        def ld_q8(g):
            nc.sync.dma_start(out=q8sb[:, g, :, :], in_=q8_in[g, :, :, :])

        def ld_k8(g, split=False):
            if split:
                nc.sync.dma_start(
                    out=k8sb[:, g, :, 0 : T_PAD // 2],
                    in_=k8_in[g, :, :, 0 : T_PAD // 2],
                )
                nc.sync.dma_start(
                    out=k8sb[:, g, :, T_PAD // 2 :],
                    in_=k8_in[g, :, :, T_PAD // 2 :],
                )
            else:
                nc.sync.dma_start(out=k8sb[:, g, :, :], in_=k8_in[g, :, :, :])

        def ld_b8(i):
            nc.sync.dma_start(
                out=b8sb[:, bA[i] : bA[i + 1], :, :],
                in_=b8_in[bA[i] : bA[i + 1], :, :, :].rearrange("k p t n -> p k t n"),
            )

        def ld_bt(i):
            nc.sync.dma_start(
                out=btsb[:, bD[i] : bD[i + 1], :],
                in_=bt_in[bD[i] : bD[i + 1], :, :].rearrange("k p n -> p k n"),
            )

        def ld_v8(q):
            nc.sync.dma_start(
                out=v8sb[:, q * 8 : (q + 1) * 8, :], in_=v8_in[q, :, :, :]
            )

        ld_q8(0)
        ld_k8(0, split=True)
        ld_b8(0)
        ld_bt(0)
        ld_v8(0)
        ld_b8(1)
        ld_bt(1)
        ld_v8(1)
        ld_b8(2)
        ld_bt(2)
        ld_v8(2)
        ld_b8(3)
        ld_bt(3)
        ld_v8(3)
        ld_k8(1)
        ld_q8(1)
        ld_k8(2)
        ld_q8(2)
        ld_k8(3)
        ld_q8(3)
        nc.sync.dma_start(out=m_sb, in_=mres.rearrange("(c p) s -> p c s", p=P))
        nc.sync.dma_start(out=wc_sb, in_=Wc8[:, :, :, :])

        attnT8 = consts.tile([P, NPAIR, TOK], F8E4)

        for j in range(NPAIR):
            g, half = j // 2, j % 2
            dve_set = DVE_ODD if (j % 2) else DVE_EVEN
            if DIAG_FORCE == "ACT":
                dve_set = ()
            elif DIAG_FORCE == "DVE":
                dve_set = tuple(range(NKCP))
            pvA = pspv.tile([96, TOK], F32, tag="pvA")
            pvB = pspv.tile([96, TOK], F32, tag="pvB")
            for kcp in range(NKCP):
                is_dve = kcp in dve_set
                if is_dve:
                    ptile = ppool.tile([P, 2, 2, TOK], F8E5, tag="pD")
                else:
                    ptile = ppool.tile([P, 2, 2, TOK], F8E4, tag="pA")
                for sub in range(2):
                    kc = 2 * kcp + sub
                    if is_dve:
                        # DVE stream: per-head 1-bank psums, independent of
                        # the ACT stream so both engines pipeline in parallel
                        for h in range(2):
                            base = 64 * half + 32 * h
                            psd = psqk.tile([P, TOK], F32, tag="sD")
                            nc.tensor.matmul(
                                psd,
                                lhsT=k8sb[base : base + 32, g, :, kc * P : (kc + 1) * P],
                                rhs=q8sb[base : base + 32, g, :, :],
                                start=True,
                                stop=True,
                                perf_mode=DR,
                                tile_position=(base, 0),
                            )
                            nc.vector.scalar_tensor_tensor(
                                out=ptile[:, h, sub, :].bitcast(I8),
                                in0=psd,
                                scalar=STT_CLAMP,
                                in1=btsb[:, DVE_SLOT.get(kc, kc % len(DVE_KC_LIST)), :],
                                op0=ALU.max,
                                op1=ALU.add,
                            )
                    else:
                        ps = psqk.tile([P, 2, TOK], F32, tag="sA")
                        for h in range(2):
                            base = 64 * half + 32 * h
                            nc.tensor.matmul(
                                ps[:, h, :],
                                lhsT=k8sb[base : base + 32, g, :, kc * P : (kc + 1) * P],
                                rhs=q8sb[base : base + 32, g, :, :],
                                start=True,
                                stop=False,
                                perf_mode=DR,
                                tile_position=(base, 0),
                            )
                            nc.tensor.matmul(
                                ps[:, h, :],
                                lhsT=identg2,
                                rhs=b8sb[:, ACT_SLOT.get(kc, kc % len(ACT_KC_LIST)), :, :],
                                start=False,
                                stop=True,
                                perf_mode=DR,
                                skip_group_check=True,
                            )
                        nc.scalar.activation(
                            out=ptile[:, :, sub, :],
                            in_=ps,
                            func=AF.Exp,
                            bias=expb_ap,
                            scale=EXP_SCALE,
                        )
                nc.tensor.matmul(
                    pvA,
                    lhsT=v8sb[:, 2 * kcp : 2 * kcp + 2, 130 * j : 130 * j + 96],
                    rhs=ptile[:, 0, :, :],
                    start=(kcp == 0),
                    stop=(kcp == NKCP - 1),
                    perf_mode=DR,
                )
                nc.tensor.matmul(
                    pvB,
                    lhsT=v8sb[:, 2 * kcp : 2 * kcp + 2, 130 * j + 65 : 130 * j + 161],
                    rhs=ptile[:, 1, :, :],
                    start=(kcp == 0),
                    stop=(kcp == NKCP - 1),
                    perf_mode=DR,
                )

            if DIAG_SKIP_TAIL:
                nc.vector.memset(attnT8[:, j, :], 0.01)
                continue
            # fast evac: stage pv to SBUF (frees the psum banks), then
            # normalize off the critical path using the idle Pool engine
            stA = small.tile([65, TOK], BF16, tag="stA", bufs=2)
            stB = small.tile([65, TOK], BF16, tag="stB", bufs=2)
            nc.scalar.copy(stA, pvA[0:65, :])
            nc.vector.tensor_copy(stB, pvB[0:65, :])
            recA = small.tile([1, TOK], BF16, tag="recA", bufs=2)
            recB = small.tile([1, TOK], BF16, tag="recB", bufs=2)
            with nc.allow_low_precision("bf16 softmax denominators, ~0.4% scale"):
                nc.vector.reciprocal(recA, stA[64:65, :])
                nc.vector.reciprocal(recB, stB[64:65, :])
            bcastA = small.tile([64, TOK], BF16, tag="bcastA", bufs=2)
            bcastB = small.tile([64, TOK], BF16, tag="bcastB", bufs=2)
            nc.gpsimd.partition_broadcast(bcastA, recA, channels=64)
            nc.gpsimd.partition_broadcast(bcastB, recB, channels=64)
            nc.gpsimd.tensor_tensor(
                out=attnT8[0:64, j, :], in0=stA[0:64, :], in1=bcastA, op=ALU.mult
            )
            nc.gpsimd.tensor_tensor(
                out=attnT8[64:128, j, :], in0=stB[0:64, :], in1=bcastB, op=ALU.mult
            )

        # output projection (fp8 DR) + residual add (f32)
        o_sb = consts.tile([P, NTC, N_STATE], BF16)
        for qc in range(NTC):
            ps_o = psqk.tile([P, 2, TOK], F32, tag="sA")
            po = ps_o.rearrange("p a b -> p (a b)")
            for pc in range(2):
                for u in range(4):
                    nc.tensor.matmul(
                        ps_o[:, pc, :],
                        lhsT=attnT8[:, 2 * u : 2 * u + 2, qc * P : (qc + 1) * P],
                        rhs=wc_sb[:, u, :, pc * TOK : (pc + 1) * TOK],
                        start=(u == 0),
                        stop=(u == 3),
                        perf_mode=DR,
                    )
            nc.vector.scalar_tensor_tensor(
                out=o_sb[:, qc, :],
                in0=po,
                scalar=G_OUT,
                in1=m_sb[:, qc, :],
                op0=ALU.mult,
                op1=ALU.add,
            )
        for qc in range(NTC):
            nc.sync.dma_start(
                out=o_out.rearrange("(c p) s -> p c s", p=P)[:, qc, :],
                in_=o_sb[:, qc, :],
            )
    nc.compile()
    return nc


_NC_CACHE = {}


def _get_nc(which):
    if which not in _NC_CACHE:
        _NC_CACHE[which] = _build_phase1() if which == 1 else _build_phase2()
    return _NC_CACHE[which]


def _perm_cols():
    """Column permutation for q/k weights: per pair j, [hA d0:32 | hB d0:32 |
    hA d32:64 | hB d32:64]."""
    order = []
    for j in range(NSC):
        hA, hB = 2 * j, 2 * j + 1
        order.extend(range(hA * 64, hA * 64 + 32))
        order.extend(range(hB * 64, hB * 64 + 32))
        order.extend(range(hA * 64 + 32, hA * 64 + 64))
        order.extend(range(hB * 64 + 32, hB * 64 + 64))
    return np.array(order)


def _w_dr_layout(w8):
    """[1024, C] -> [128, 4, 2, C] DoubleRow lhsT layout."""
    return np.ascontiguousarray(
        w8.reshape(4, 2, P, -1).transpose(2, 0, 1, 3)
    )


def kernel(m, bias, gamma, beta, Wq, bq, Wk, Wv, bv, Wc, bc, _want_timing=None):
    m = np.asarray(m, dtype=np.float32).reshape(N_CTX, N_STATE)
    bias = np.asarray(bias, np.float32)
    gamma = np.asarray(gamma, np.float32)
    beta = np.asarray(beta, np.float32)
    Wq = np.asarray(Wq, np.float32)
    Wk = np.asarray(Wk, np.float32)
    Wv = np.asarray(Wv, np.float32)
    Wc = np.asarray(Wc, np.float32)
    bq = np.asarray(bq, np.float32)
    bv = np.asarray(bv, np.float32)
    bc = np.asarray(bc, np.float32)

    m_pad = np.zeros((T_PAD, N_STATE), np.float32)
    m_pad[:N_CTX] = m

    # fold gamma into weights, beta into biases; bv and bc fold into residual
    Wqf = gamma[:, None] * Wq
    Wkf = gamma[:, None] * Wk
    Wvf = gamma[:, None] * Wv
    bqf = bq + beta @ Wq
    # beta@Wk shifts all logits of a query equally -> softmax invariant; drop.
    # beta@Wv + bv shift attention output -> fold into residual with bc.
    perm = _perm_cols()
    Wq8 = _w_dr_layout((LAM_W * Wqf[:, perm]).astype(E4NP))
    Wk8 = _w_dr_layout((LAM_W * Wkf[:, perm]).astype(E4NP))
    Wv8 = _w_dr_layout((LAM_W * Wvf).astype(E4NP))
    bqs = (LAM_Q * bqf[perm]).astype(np.float32)
    Wc8 = _w_dr_layout((LAM_WC * Wc).astype(E4NP))
    mres_full = m_pad + (bc + (bv + beta @ Wv) @ Wc)[None, :]

    import sys as _sys

    def _log(*a):
        print("[kernel]", *a, file=_sys.stderr, flush=True)

    nc1 = _get_nc(1)
    _log("phase1 built")
    in_maps1 = []
    for c in range(N_CORES):
        in_maps1.append(
            {
                "m_blk": np.ascontiguousarray(
                    m_pad[c * TOK : (c + 1) * TOK].astype(BFNP)
                ),
                "Wq8": Wq8,
                "Wk8": Wk8,
                "Wv8": Wv8,
                "bqs": bqs,
            }
        )
    res1 = run_bass_kernel_spmd(nc1, in_maps1, core_ids=list(range(N_CORES)))
    _log("phase1 done")

    q8_blks = [r["q8_out"] for r in res1.results]
    k8_full = np.concatenate([r["k8_out"] for r in res1.results], axis=3)
    v8_full = np.concatenate([r["v8_out"] for r in res1.results], axis=0)
    v8_full[N_CTX:] = 0  # pad tokens carry no value

    # v8 pair-tile layout [128, 32, VW] with denominator columns
    v8f = v8_full.astype(np.float32).reshape(NKC, P, N_HEADS, D_HEAD)
    v8h = np.zeros((P, NKC, VW), np.float32)
    for j in range(NPAIR):
        v8h[:, :, 130 * j : 130 * j + 64] = v8f[:, :, 2 * j].transpose(1, 0, 2)
        v8h[:, :, 130 * j + 65 : 130 * j + 129] = v8f[:, :, 2 * j + 1].transpose(1, 0, 2)
        v8h[:, :, 130 * j + 64] = ONES_VAL
        v8h[:, :, 130 * j + 129] = ONES_VAL
    # zero the denominator contribution of padded keys
    keyidx = (np.arange(NKC)[None, :] * P + np.arange(P)[:, None])  # [p, kc]
    padmask = keyidx >= N_CTX
    for j in range(NPAIR):
        v8h[:, :, 130 * j + 64][padmask] = 0.0
        v8h[:, :, 130 * j + 129][padmask] = 0.0
    v8h8 = v8h.astype(E4NP)
    v8_dr = np.ascontiguousarray(
        v8h8.reshape(P, 4, NKC // 4, VW).transpose(1, 0, 2, 3)
    )

    biasT = np.ascontiguousarray(bias.T)  # [k, q]

    nc2 = _get_nc(2)
    _log("phase2 built")
    in_maps2 = []
    for c in range(N_CORES):
        qs = slice(c * TOK, (c + 1) * TOK)
        b8 = np.zeros((len(ACT_KC_LIST), P, 2, TOK), E4NP)
        for i, kc in enumerate(ACT_KC_LIST):
            chunk = (LAM_B * biasT[kc * P : (kc + 1) * P, qs]).astype(E4NP)
            b8[i, :, 0, :] = chunk
            b8[i, :, 1, :] = chunk
        bt = np.zeros((len(DVE_KC_LIST), P, TOK), BFNP)
        for i, kc in enumerate(DVE_KC_LIST):
            bt[i] = (
                ALPHA * biasT[kc * P : (kc + 1) * P, qs] + (BETA - ALPHA * C_SHIFT)
            ).astype(BFNP)
        in_maps2.append(
            {
                "q8_in": np.ascontiguousarray(q8_blks[c]),
                "k8_in": k8_full,
                "v8_in": v8_dr,
                "b8_in": b8,
                "bt_in": bt,
                "mres": np.ascontiguousarray(mres_full[qs]),
                "Wc8": Wc8,
            }
        )
    res2 = run_bass_kernel_spmd(nc2, in_maps2, core_ids=list(range(N_CORES)))
    _log("phase2 done")
    o = np.concatenate([r["o_out"] for r in res2.results], axis=0)[:N_CTX]
    if _want_timing is not None:
        _want_timing["res1"] = res1
        _want_timing["res2"] = res2
    return o.reshape(1, N_CTX, N_STATE).astype(np.float32)


# revision 24
# speedup vs baseline: 1.7753x; 1.0037x over previous
"""AttentionResblock on 8 NeuronCores (Trainium2, Bass/Tile) — fp8 edition.

Sharding: query-token blocks of 512 (T_PAD=4096 = 8 x 512), two launches:
  Phase 1 (per core c): LayerNorm + Q/K/V projections (fp8 DoubleRow matmuls)
    for token rows [512c, 512c+512). Emits q8/k8 in DoubleRow-ready
    [128, 2, 512] head-pair tiles and v8 token-major, all fp8-e4m3.
    gamma/beta/bv/bc are folded into weights/residual on the host.
  Phase 2 (per core c): 16-head attention for its 512 query rows over all
    4096 keys. QK via fp8 DoubleRow (2x32 contraction). Softmax weights are
    produced two ways, split across engines to balance the timeline:
      - ACT chunks: bias added in PSUM via fp8 ident-matmul, then true
        exp -> fp8-e4m3 (premultiplied by 512, shifted by C=9).
      - DVE chunks: fastexp bit trick - y = int8(alpha*s + [alpha*b +
        beta - alpha*C]) bitcast as fp8-e5m2 (2^(y/4-15) ~ 512*e^(s+b-9)).
    The scale/shift cancels in softmax: PV accumulates numerator and
    denominator (ones columns in the fp8 V tiles, value 0.5 = LAM_V/LAM_ATTN)
    with fp8 DoubleRow over key-chunk pairs. Normalize, fp8 DoubleRow output
    projection, f32 residual add.

Numerics validated against the reference in numpy (numerics2.py): rel err
~1.5e-4 vs the 2e-2 gate, dominated by fp8 quantization of the attention
branch, which is scaled by ~1e-3 through Wc so the f32 residual dominates.
"""

import sys

sys.path.insert(0, "/opt/trn_rl_repo")

from contextlib import ExitStack  # noqa: E402

import numpy as np  # noqa: E402
import ml_dtypes  # noqa: E402

import concourse.bass as bass  # noqa: E402
import concourse.bacc as bacc  # noqa: E402
import concourse.tile as tile  # noqa: E402
from concourse import mybir  # noqa: E402
from concourse.bass_utils import run_bass_kernel_spmd  # noqa: E402
from concourse.masks import make_identity  # noqa: E402

F32 = mybir.dt.float32
BF16 = mybir.dt.bfloat16
F8E4 = mybir.dt.float8e4
F8E5 = mybir.dt.float8e5
I8 = mybir.dt.int8
AF = mybir.ActivationFunctionType
ALU = mybir.AluOpType
DR = mybir.MatmulPerfMode.DoubleRow

E4NP = ml_dtypes.float8_e4m3
E5NP = ml_dtypes.float8_e5m2
BFNP = ml_dtypes.bfloat16

N_STATE = 1024
N_HEADS = 16
D_HEAD = 64
N_CTX = 4080
T_PAD = 4096
N_CORES = 8
TOK = 512
P = 128
LN_EPS = 1e-5
NSC = 8  # state chunks of 128
NTC = 4  # token chunks per core
NKC = 32  # key chunks of 128
NKCP = 16  # key-chunk pairs of 256
NPAIR = 8  # head pairs

# fp8 scale plan (see numerics2.py)
ALPHA = 4 * np.log2(np.e)  # logit scale in PSUM: psum = ALPHA*(s)
C_SHIFT = 9.0  # global logit shift (measured max 6.21)
PMULT = 512.0  # weights premultiplier (cancels in softmax)
BETA = 96.0  # 60 + 4*log2(PMULT)
LAM_R = 16.0  # LN output scale
LAM_W = 512.0  # Wq/Wk/Wv scale
LAM_Q = float(np.sqrt(ALPHA / 8.0))  # q/k scales; 8*LAM_Q*LAM_K = ALPHA
LAM_V = 16.0
LAM_B = 369.0  # ACT-path bias quant scale; ident diag g = 2^-6, g*LAM_B ~ ALPHA
G_IDENT = 2.0 ** -6
LAM_ATTN = 32.0
LAM_WC = 32768.0
ONES_VAL = LAM_V / LAM_ATTN  # 0.5, folded into denominator columns
GQ = LAM_Q / (LAM_R * LAM_W)
GV = LAM_V / (LAM_R * LAM_W)
G_OUT = 1.0 / (LAM_ATTN * LAM_WC)
EXP_BIAS = float(np.log(PMULT) - C_SHIFT)  # -2.7616
EXP_SCALE = float(1.0 / ALPHA)
STT_CLAMP = -40.0

# kc-pair -> engine assignment (per head-pair parity), tuned for balance
DVE_EVEN = (3, 5, 7, 9, 11, 13, 15)
DVE_ODD = (3, 5, 7, 9, 11, 13, 15)
ACT_KC_LIST = [kc for kcp in range(NKCP) if kcp not in DVE_EVEN
               for kc in (2 * kcp, 2 * kcp + 1)]  # 20 kcs ever handled by ACT
DVE_KC_LIST = [kc for kcp in DVE_ODD for kc in (2 * kcp, 2 * kcp + 1)]  # 14 kcs
ACT_SLOT = {kc: i for i, kc in enumerate(ACT_KC_LIST)}
DVE_SLOT = {kc: i for i, kc in enumerate(DVE_KC_LIST)}
VW = NPAIR * 130 + 32  # v8 tile width: per-pair 130 cols + tail padding
PSQK_BUFS = 2
PSPV_BUFS = 1
P_BUFS = 4
DIAG_SKIP_TAIL = False
DIAG_FORCE = None  # None | "ACT" | "DVE"


def _build_phase1() -> bass.Bass:
    nc = bacc.Bacc("TRN2", target_bir_lowering=False, debug=False, num_devices=N_CORES)
    m_blk = nc.dram_tensor("m_blk", [TOK, N_STATE], BF16, kind="ExternalInput")
    Wq8 = nc.dram_tensor("Wq8", [P, 4, 2, N_STATE], F8E4, kind="ExternalInput")
    Wk8 = nc.dram_tensor("Wk8", [P, 4, 2, N_STATE], F8E4, kind="ExternalInput")
    Wv8 = nc.dram_tensor("Wv8", [P, 4, 2, N_STATE], F8E4, kind="ExternalInput")
    bqs = nc.dram_tensor("bqs", [N_STATE], F32, kind="ExternalInput")
    q8_out = nc.dram_tensor("q8_out", [4, P, 2, TOK], F8E4, kind="ExternalOutput")
    k8_out = nc.dram_tensor("k8_out", [4, P, 2, TOK], F8E4, kind="ExternalOutput")
    v8_out = nc.dram_tensor("v8_out", [TOK, N_STATE], F8E4, kind="ExternalOutput")

    with ExitStack() as ctx:
        tc = ctx.enter_context(tile.TileContext(nc))
        consts = ctx.enter_context(tc.tile_pool(name="consts", bufs=1))
        small = ctx.enter_context(tc.tile_pool(name="small", bufs=4))
        work = ctx.enter_context(tc.tile_pool(name="work", bufs=2))
        psum = ctx.enter_context(tc.tile_pool(name="psum", bufs=2, space="PSUM"))
        pst_pool = ctx.enter_context(tc.tile_pool(name="pst", bufs=2, space="PSUM"))

        identB = consts.tile([P, P], BF16)
        make_identity(nc, identB)
        eps_sb = consts.tile([P, 1], F32)
        nc.vector.memset(eps_sb, LN_EPS / (LAM_R * LAM_R))
        bqs_sb = consts.tile([P, NSC], F32)

        m_sb = consts.tile([P, NTC, N_STATE], BF16)
        w_sb = {}
        for name, w in (("Wq8", Wq8), ("Wk8", Wk8), ("Wv8", Wv8)):
            w_sb[name] = consts.tile([P, 4, 2, N_STATE], F8E4, name=f"{name}_sb")

        def ld_m(tcn):
            nc.sync.dma_start(
                out=m_sb[:, tcn, :],
                in_=m_blk.rearrange("(c p) s -> p c s", p=P)[:, tcn, :],
            )

        ld_m(0)
        ld_m(1)
        ld_m(2)
        ld_m(3)
        nc.sync.dma_start(out=bqs_sb, in_=bqs.rearrange("(j p) -> p j", p=P))
        nc.sync.dma_start(out=w_sb["Wq8"], in_=Wq8[:, :, :, :])
        nc.sync.dma_start(out=w_sb["Wk8"], in_=Wk8[:, :, :, :])
        nc.sync.dma_start(out=w_sb["Wv8"], in_=Wv8[:, :, :, :])

        # LayerNorm -> xcB = (m - mu) * rstd * LAM_R in bf16
        xcB = consts.tile([P, NTC, N_STATE], BF16)
        for tcn in range(NTC):
            ssum = small.tile([P, 1], F32, tag="ssum")
            nc.vector.reduce_sum(ssum, m_sb[:, tcn, :], axis=mybir.AxisListType.X)
            negmean = small.tile([P, 1], F32, tag="negmean")
            nc.scalar.mul(negmean, ssum, -1.0 / N_STATE)
            sqscr = work.tile([P, N_STATE], BF16, tag="sqscr")
            sqsum = small.tile([P, 1], F32, tag="sqsum")
            nc.scalar.activation(
                out=sqscr, in_=m_sb[:, tcn, :], func=AF.Square, accum_out=sqsum
            )
            # 1024*var = sqsum - ssum^2/1024
            musq = small.tile([P, 1], F32, tag="musq")
            nc.vector.scalar_tensor_tensor(
                out=musq, in0=ssum, scalar=-1.0 / N_STATE, in1=ssum,
                op0=ALU.mult, op1=ALU.mult,
            )
            nvar = small.tile([P, 1], F32, tag="nvar")
            nc.vector.tensor_tensor(out=nvar, in0=sqsum, in1=musq, op=ALU.add)
            std = small.tile([P, 1], F32, tag="std")
            nc.scalar.activation(
                out=std, in_=nvar, func=AF.Sqrt, bias=eps_sb,
                scale=1.0 / (N_STATE * LAM_R * LAM_R),
            )
            rstdl = small.tile([P, 1], F32, tag="rstdl")
            nc.vector.reciprocal(rstdl, std)
            eng = nc.gpsimd if tcn % 2 == 0 else nc.vector
            eng.tensor_scalar(
                out=xcB[:, tcn, :],
                in0=m_sb[:, tcn, :],
                scalar1=negmean,
                scalar2=rstdl,
                op0=ALU.add,
                op1=ALU.mult,
            )

        # transpose to state-major and quantize: rT8 [128, sc, 512] e4m3
        rT8 = consts.tile([P, NSC, TOK], F8E4)
        for sc in range(NSC):
            pst = pst_pool.tile([P, TOK], BF16, tag="pst")
            for tcn in range(NTC):
                nc.tensor.transpose(
                    pst[:, tcn * P : (tcn + 1) * P],
                    xcB[:, tcn, sc * P : (sc + 1) * P],
                    identB,
                )
            if sc % 2 == 0:
                nc.vector.tensor_copy(rT8[:, sc, :], pst)
            else:
                nc.scalar.copy(rT8[:, sc, :], pst)

        # q/k: DoubleRow fp8 matmuls, evacuate into [128, 2, 512] pair tiles
        q8g = [consts.tile([P, 2, TOK], F8E4, name=f"q8g{g}") for g in range(4)]
        k8g = [consts.tile([P, 2, TOK], F8E4, name=f"k8g{g}") for g in range(4)]
        v8sb = consts.tile([P, NTC, N_STATE], F8E4)

        def emit_v(tcn):
            psv = psum.tile([P, N_STATE], F32, tag="psv", bufs=1)
            for pc in range(2):
                for s in range(4):
                    nc.tensor.matmul(
                        psv[:, pc * TOK : (pc + 1) * TOK],
                        lhsT=rT8[:, 2 * s : 2 * s + 2, tcn * P : (tcn + 1) * P],
                        rhs=w_sb["Wv8"][:, s, :, pc * TOK : (pc + 1) * TOK],
                        start=(s == 0),
                        stop=(s == 3),
                        perf_mode=DR,
                    )
            if tcn % 2 == 0:
                nc.scalar.mul(v8sb[:, tcn, :], psv, GV)
            else:
                nc.vector.tensor_scalar(
                    out=v8sb[:, tcn, :], in0=psv, scalar1=GV, scalar2=None,
                    op0=ALU.mult,
                )
            nc.sync.dma_start(
                out=v8_out.rearrange("(c p) s -> p c s", p=P)[:, tcn, :],
                in_=v8sb[:, tcn, :],
            )

        for j in range(NSC):
            g, half = j // 2, j % 2
            psq = psum.tile([P, TOK], F32, tag="psq")
            psk = psum.tile([P, TOK], F32, tag="psk")
            for s in range(4):
                nc.tensor.matmul(
                    psq,
                    lhsT=w_sb["Wq8"][:, s, :, j * P : (j + 1) * P],
                    rhs=rT8[:, 2 * s : 2 * s + 2, :],
                    start=(s == 0),
                    stop=(s == 3),
                    perf_mode=DR,
                )
            for s in range(4):
                nc.tensor.matmul(
                    psk,
                    lhsT=w_sb["Wk8"][:, s, :, j * P : (j + 1) * P],
                    rhs=rT8[:, 2 * s : 2 * s + 2, :],
                    start=(s == 0),
                    stop=(s == 3),
                    perf_mode=DR,
                )
            for t in range(2):
                nc.scalar.activation(
                    out=q8g[g][64 * half : 64 * half + 64, t, :],
                    in_=psq[64 * t : 64 * t + 64, :],
                    func=AF.Identity,
                    bias=bqs_sb[64 * t : 64 * t + 64, j : j + 1],
                    scale=GQ,
                )
                nc.vector.tensor_scalar(
                    out=k8g[g][64 * half : 64 * half + 64, t, :],
                    in0=psk[64 * t : 64 * t + 64, :],
                    scalar1=GQ,
                    scalar2=None,
                    op0=ALU.mult,
                )
            if j % 2 == 1:
                emit_v(j // 2)
                nc.sync.dma_start(out=q8_out[g, :, :, :], in_=q8g[g])
                nc.sync.dma_start(out=k8_out[g, :, :, :], in_=k8g[g])
    nc.compile()
    return nc


def _build_phase2() -> bass.Bass:
    nc = bacc.Bacc("TRN2", target_bir_lowering=False, debug=False, num_devices=N_CORES)
    q8_in = nc.dram_tensor("q8_in", [4, P, 2, TOK], F8E4, kind="ExternalInput")
    k8_in = nc.dram_tensor("k8_in", [4, P, 2, T_PAD], F8E4, kind="ExternalInput")
    v8_in = nc.dram_tensor("v8_in", [4, P, NKC // 4, VW], F8E4, kind="ExternalInput")
    b8_in = nc.dram_tensor("b8_in", [len(ACT_KC_LIST), P, 2, TOK], F8E4, kind="ExternalInput")
    bt_in = nc.dram_tensor("bt_in", [len(DVE_KC_LIST), P, TOK], BF16, kind="ExternalInput")
    mres = nc.dram_tensor("mres", [TOK, N_STATE], F32, kind="ExternalInput")
    Wc8 = nc.dram_tensor("Wc8", [P, 4, 2, N_STATE], F8E4, kind="ExternalInput")
    o_out = nc.dram_tensor("o_out", [TOK, N_STATE], BF16, kind="ExternalOutput")

    with ExitStack() as ctx:
        tc = ctx.enter_context(tile.TileContext(nc))
        consts = ctx.enter_context(tc.tile_pool(name="consts", bufs=1))
        small = ctx.enter_context(tc.tile_pool(name="small", bufs=4))
        ppool = ctx.enter_context(tc.tile_pool(name="ppool", bufs=P_BUFS))
        psqk = ctx.enter_context(tc.tile_pool(name="psqk", bufs=PSQK_BUFS, space="PSUM"))
        pspv = ctx.enter_context(tc.tile_pool(name="pspv", bufs=PSPV_BUFS, space="PSUM"))

        identg = consts.tile([P, 2, P], F8E4)
        nc.vector.memset(identg, 0.0)
        make_identity(nc, identg[:, 0, :])
        make_identity(nc, identg[:, 1, :])
        identg2 = consts.tile([P, 2, P], F8E4)
        nc.scalar.mul(identg2, identg, G_IDENT / 2.0)
        expb_ap = consts.tile([P, 1], F32)
        nc.vector.memset(expb_ap, EXP_BIAS)

        q8sb = consts.tile([P, 4, 2, TOK], F8E4)
        k8sb = consts.tile([P, 4, 2, T_PAD], F8E4)
        v8sb = consts.tile([P, NKC, VW], F8E4)
        b8sb = consts.tile([P, len(ACT_KC_LIST), 2, TOK], F8E4)
        btsb = consts.tile([P, len(DVE_KC_LIST), TOK], BF16)
        m_sb = consts.tile([P, NTC, N_STATE], F32)
        wc_sb = consts.tile([P, 4, 2, N_STATE], F8E4)
        nA, nD = len(ACT_KC_LIST), len(DVE_KC_LIST)
        bA = [0, 5, 10, 15, nA]
        bD = [0, 4, 8, 11, nD]

        def ld_q8(g):
            nc.sync.dma_start(out=q8sb[:, g, :, :], in_=q8_in[g, :, :, :])

        def ld_k8(g, split=False):
            if split:
                nc.sync.dma_start(
                    out=k8sb[:, g, :, 0 : T_PAD // 2],
                    in_=k8_in[g, :, :, 0 : T_PAD // 2],
                )
                nc.sync.dma_start(
                    out=k8sb[:, g, :, T_PAD // 2 :],
                    in_=k8_in[g, :, :, T_PAD // 2 :],
                )
            else:
                nc.sync.dma_start(out=k8sb[:, g, :, :], in_=k8_in[g, :, :, :])

        def ld_b8(i):
            nc.sync.dma_start(
                out=b8sb[:, bA[i] : bA[i + 1], :, :],
                in_=b8_in[bA[i] : bA[i + 1], :, :, :].rearrange("k p t n -> p k t n"),
            )

        def ld_bt(i):
            nc.sync.dma_start(
                out=btsb[:, bD[i] : bD[i + 1], :],
                in_=bt_in[bD[i] : bD[i + 1], :, :].rearrange("k p n -> p k n"),
            )

        def ld_v8(q):
            nc.sync.dma_start(
                out=v8sb[:, q * 8 : (q + 1) * 8, :], in_=v8_in[q, :, :, :]
            )

        ld_q8(0)
        ld_k8(0, split=True)
        ld_b8(0)
        ld_bt(0)
        ld_v8(0)
        ld_b8(1)
        ld_bt(1)
        ld_v8(1)
        ld_b8(2)
        ld_bt(2)
        ld_v8(2)
        ld_b8(3)
        ld_bt(3)
        ld_v8(3)
        ld_k8(1)
        ld_q8(1)
        ld_k8(2)
        ld_q8(2)
        ld_k8(3)
        ld_q8(3)
        nc.sync.dma_start(out=m_sb, in_=mres.rearrange("(c p) s -> p c s", p=P))
        nc.sync.dma_start(out=wc_sb, in_=Wc8[:, :, :, :])

        attnT8 = consts.tile([P, NPAIR, TOK], F8E4)

        for j in range(NPAIR):
            g, half = j // 2, j % 2
            dve_set = DVE_ODD if (j % 2) else DVE_EVEN
            if DIAG_FORCE == "ACT":
                dve_set = ()
            elif DIAG_FORCE == "DVE":
                dve_set = tuple(range(NKCP))
            pvA = pspv.tile([96, TOK], F32, tag="pvA")
            pvB = pspv.tile([96, TOK], F32, tag="pvB")
            for kcp in range(NKCP):
                is_dve = kcp in dve_set
                if is_dve:
                    ptile = ppool.tile([P, 2, 2, TOK], F8E5, tag="pD")
                else:
                    ptile = ppool.tile([P, 2, 2, TOK], F8E4, tag="pA")
                for sub in range(2):
                    kc = 2 * kcp + sub
                    if is_dve:
                        # DVE stream: per-head 1-bank psums, independent of
                        # the ACT stream so both engines pipeline in parallel
                        for h in range(2):
                            base = 64 * half + 32 * h
                            psd = psqk.tile([P, TOK], F32, tag="sD")
                            nc.tensor.matmul(
                                psd,
                                lhsT=k8sb[base : base + 32, g, :, kc * P : (kc + 1) * P],
                                rhs=q8sb[base : base + 32, g, :, :],
                                start=True,
                                stop=True,
                                perf_mode=DR,
                                tile_position=(base, 0),
                            )
                            nc.vector.scalar_tensor_tensor(
                                out=ptile[:, h, sub, :].bitcast(I8),
                                in0=psd,
                                scalar=STT_CLAMP,
                                in1=btsb[:, DVE_SLOT.get(kc, kc % len(DVE_KC_LIST)), :],
                                op0=ALU.max,
                                op1=ALU.add,
                            )
                    else:
                        ps = psqk.tile([P, 2, TOK], F32, tag="sA")
                        for h in range(2):
                            base = 64 * half + 32 * h
                            nc.tensor.matmul(
                                ps[:, h, :],
                                lhsT=k8sb[base : base + 32, g, :, kc * P : (kc + 1) * P],
                                rhs=q8sb[base : base + 32, g, :, :],
                                start=True,
                                stop=False,
                                perf_mode=DR,
                                tile_position=(base, 0),
                            )
                            nc.tensor.matmul(
                                ps[:, h, :],
                                lhsT=identg2,
                                rhs=b8sb[:, ACT_SLOT.get(kc, kc % len(ACT_KC_LIST)), :, :],
                                start=False,
                                stop=True,
                                perf_mode=DR,
                                skip_group_check=True,
                            )
                        nc.scalar.activation(
                            out=ptile[:, :, sub, :],
                            in_=ps,
                            func=AF.Exp,
                            bias=expb_ap,
                            scale=EXP_SCALE,
                        )
                nc.tensor.matmul(
                    pvA,
                    lhsT=v8sb[:, 2 * kcp : 2 * kcp + 2, 130 * j : 130 * j + 96],
                    rhs=ptile[:, 0, :, :],
                    start=(kcp == 0),
                    stop=(kcp == NKCP - 1),
                    perf_mode=DR,
                )
                nc.tensor.matmul(
                    pvB,
                    lhsT=v8sb[:, 2 * kcp : 2 * kcp + 2, 130 * j + 65 : 130 * j + 161],
                    rhs=ptile[:, 1, :, :],
                    start=(kcp == 0),
                    stop=(kcp == NKCP - 1),
                    perf_mode=DR,
                )

            if DIAG_SKIP_TAIL:
                nc.vector.memset(attnT8[:, j, :], 0.01)
                continue
            # fast evac: stage pv to SBUF (frees the psum banks), then
            # normalize off the critical path using the idle Pool engine
            stA = small.tile([65, TOK], BF16, tag="stA", bufs=2)
            stB = small.tile([65, TOK], BF16, tag="stB", bufs=2)
            nc.scalar.copy(stA, pvA[0:65, :])
            nc.vector.tensor_copy(stB, pvB[0:65, :])
            recA = small.tile([1, TOK], BF16, tag="recA", bufs=2)
            recB = small.tile([1, TOK], BF16, tag="recB", bufs=2)
            with nc.allow_low_precision("bf16 softmax denominators, ~0.4% scale"):
                nc.vector.reciprocal(recA, stA[64:65, :])
                nc.vector.reciprocal(recB, stB[64:65, :])
            bcastA = small.tile([64, TOK], BF16, tag="bcastA", bufs=2)
            bcastB = small.tile([64, TOK], BF16, tag="bcastB", bufs=2)
            nc.gpsimd.partition_broadcast(bcastA, recA, channels=64)
            nc.gpsimd.partition_broadcast(bcastB, recB, channels=64)
            nc.gpsimd.tensor_tensor(
                out=attnT8[0:64, j, :], in0=stA[0:64, :], in1=bcastA, op=ALU.mult
            )
            nc.gpsimd.tensor_tensor(
                out=attnT8[64:128, j, :], in0=stB[0:64, :], in1=bcastB, op=ALU.mult
            )

        # output projection (fp8 DR) + residual add (f32)
        o_sb = consts.tile([P, NTC, N_STATE], BF16)
        for qc in range(NTC):
            ps_o = psqk.tile([P, 2, TOK], F32, tag="sA")
            po = ps_o.rearrange("p a b -> p (a b)")
            for pc in range(2):
                for u in range(4):
                    nc.tensor.matmul(
                        ps_o[:, pc, :],
                        lhsT=attnT8[:, 2 * u : 2 * u + 2, qc * P : (qc + 1) * P],
                        rhs=wc_sb[:, u, :, pc * TOK : (pc + 1) * TOK],
                        start=(u == 0),
                        stop=(u == 3),
                        perf_mode=DR,
                    )
            nc.vector.scalar_tensor_tensor(
                out=o_sb[:, qc, :],
                in0=po,
                scalar=G_OUT,
                in1=m_sb[:, qc, :],
                op0=ALU.mult,
                op1=ALU.add,
            )
        for qc in range(NTC):
            nc.sync.dma_start(
                out=o_out.rearrange("(c p) s -> p c s", p=P)[:, qc, :],
                in_=o_sb[:, qc, :],
            )
    nc.compile()
    return nc


_NC_CACHE = {}


def _get_nc(which):
    if which not in _NC_CACHE:
        _NC_CACHE[which] = _build_phase1() if which == 1 else _build_phase2()
    return _NC_CACHE[which]


def _perm_cols():
    """Column permutation for q/k weights: per pair j, [hA d0:32 | hB d0:32 |
    hA d32:64 | hB d32:64]."""
    order = []
    for j in range(NSC):
        hA, hB = 2 * j, 2 * j + 1
        order.extend(range(hA * 64, hA * 64 + 32))
        order.extend(range(hB * 64, hB * 64 + 32))
        order.extend(range(hA * 64 + 32, hA * 64 + 64))
        order.extend(range(hB * 64 + 32, hB * 64 + 64))
    return np.array(order)


def _w_dr_layout(w8):
    """[1024, C] -> [128, 4, 2, C] DoubleRow lhsT layout."""
    return np.ascontiguousarray(
        w8.reshape(4, 2, P, -1).transpose(2, 0, 1, 3)
    )


def kernel(m, bias, gamma, beta, Wq, bq, Wk, Wv, bv, Wc, bc, _want_timing=None):
    m = np.asarray(m, dtype=np.float32).reshape(N_CTX, N_STATE)
    bias = np.asarray(bias, np.float32)
    gamma = np.asarray(gamma, np.float32)
    beta = np.asarray(beta, np.float32)
    Wq = np.asarray(Wq, np.float32)
    Wk = np.asarray(Wk, np.float32)
    Wv = np.asarray(Wv, np.float32)
    Wc = np.asarray(Wc, np.float32)
    bq = np.asarray(bq, np.float32)
    bv = np.asarray(bv, np.float32)
    bc = np.asarray(bc, np.float32)

    m_pad = np.zeros((T_PAD, N_STATE), np.float32)
    m_pad[:N_CTX] = m

    # fold gamma into weights, beta into biases; bv and bc fold into residual
    Wqf = gamma[:, None] * Wq
    Wkf = gamma[:, None] * Wk
    Wvf = gamma[:, None] * Wv
    bqf = bq + beta @ Wq
    # beta@Wk shifts all logits of a query equally -> softmax invariant; drop.
    # beta@Wv + bv shift attention output -> fold into residual with bc.
    perm = _perm_cols()
    Wq8 = _w_dr_layout((LAM_W * Wqf[:, perm]).astype(E4NP))
    Wk8 = _w_dr_layout((LAM_W * Wkf[:, perm]).astype(E4NP))
    Wv8 = _w_dr_layout((LAM_W * Wvf).astype(E4NP))
    bqs = (LAM_Q * bqf[perm]).astype(np.float32)
    Wc8 = _w_dr_layout((LAM_WC * Wc).astype(E4NP))
    mres_full = m_pad + (bc + (bv + beta @ Wv) @ Wc)[None, :]

    import sys as _sys

    def _log(*a):
        print("[kernel]", *a, file=_sys.stderr, flush=True)

    nc1 = _get_nc(1)
    _log("phase1 built")
    in_maps1 = []
    for c in range(N_CORES):
        in_maps1.append(
            {
                "m_blk": np.ascontiguousarray(
                    m_pad[c * TOK : (c + 1) * TOK].astype(BFNP)
                ),
                "Wq8": Wq8,
                "Wk8": Wk8,
                "Wv8": Wv8,
                "bqs": bqs,
            }
        )
    res1 = run_bass_kernel_spmd(nc1, in_maps1, core_ids=list(range(N_CORES)))
    _log("phase1 done")

    q8_blks = [r["q8_out"] for r in res1.results]
    k8_full = np.concatenate([r["k8_out"] for r in res1.results], axis=3)
    v8_full = np.concatenate([r["v8_out"] for r in res1.results], axis=0)
    v8_full[N_CTX:] = 0  # pad tokens carry no value

    # v8 pair-tile layout [128, 32, VW] with denominator columns
    v8f = v8_full.astype(np.float32).reshape(NKC, P, N_HEADS, D_HEAD)
    v8h = np.zeros((P, NKC, VW), np.float32)
    for j in range(NPAIR):
        v8h[:, :, 130 * j : 130 * j + 64] = v8f[:, :, 2 * j].transpose(1, 0, 2)
        v8h[:, :, 130 * j + 65 : 130 * j + 129] = v8f[:, :, 2 * j + 1].transpose(1, 0, 2)
        v8h[:, :, 130 * j + 64] = ONES_VAL
        v8h[:, :, 130 * j + 129] = ONES_VAL
    # zero the denominator contribution of padded keys
    keyidx = (np.arange(NKC)[None, :] * P + np.arange(P)[:, None])  # [p, kc]
    padmask = keyidx >= N_CTX
    for j in range(NPAIR):
        v8h[:, :, 130 * j + 64][padmask] = 0.0
        v8h[:, :, 130 * j + 129][padmask] = 0.0
    v8h8 = v8h.astype(E4NP)
    v8_dr = np.ascontiguousarray(
        v8h8.reshape(P, 4, NKC // 4, VW).transpose(1, 0, 2, 3)
    )

    biasT = np.ascontiguousarray(bias.T)  # [k, q]

    nc2 = _get_nc(2)
    _log("phase2 built")
    in_maps2 = []
    for c in range(N_CORES):
        qs = slice(c * TOK, (c + 1) * TOK)
        b8 = np.zeros((len(ACT_KC_LIST), P, 2, TOK), E4NP)
        for i, kc in enumerate(ACT_KC_LIST):
            chunk = (LAM_B * biasT[kc * P : (kc + 1) * P, qs]).astype(E4NP)
            b8[i, :, 0, :] = chunk
            b8[i, :, 1, :] = chunk
        bt = np.zeros((len(DVE_KC_LIST), P, TOK), BFNP)
        for i, kc in enumerate(DVE_KC_LIST):
            bt[i] = (
                ALPHA * biasT[kc * P : (kc + 1) * P, qs] + (BETA - ALPHA * C_SHIFT)
            ).astype(BFNP)
        in_maps2.append(
            {
                "q8_in": np.ascontiguousarray(q8_blks[c]),
                "k8_in": k8_full,
                "v8_in": v8_dr,
                "b8_in": b8,
                "bt_in": bt,
                "mres": np.ascontiguousarray(mres_full[qs]),
                "Wc8": Wc8,
            }
        )
    res2 = run_bass_kernel_spmd(nc2, in_maps2, core_ids=list(range(N_CORES)))
    _log("phase2 done")
    o = np.concatenate([r["o_out"] for r in res2.results], axis=0)[:N_CTX]
    if _want_timing is not None:
        _want_timing["res1"] = res1
        _want_timing["res2"] = res2
    return o.reshape(1, N_CTX, N_STATE).astype(np.float32)
